# revision 27
# baseline (speedup 1.0000x reference)
"""Multi-head attention (S=2048, B=4, H=1024, NH=16) on 8 Trainium2 NeuronCores.

Sharding: each core handles 2 batches x 4 heads (batch pairs balanced by
valid length; tensor-parallel over heads). Within a core everything is bf16
matmul / fp32 accumulate:
  1. q,k projected d-major (qT/kT: [dims, seq]), v seq-major ([seq, dims])
  2. scoresT[k,q] per head-pair via row-tiled matmuls (row_grp concurrency)
  3. mask+scale+exp fused on ScalarE (per-partition bias; PAD keys -> exp 0)
  4. PV col-tiled accumulates attnT; Z row-sums via all-ones stationary
     matmul (col_grp-concurrent with PV)
  5. attnT normalized by 1/Z; keepq applied before the Wo output multiply
  6. Wo projection -> yT partial [H, S]; host sums 4 partials/batch

Schedule: slot0 projections stream from HBM across 4 DMA queues; slot0
attention is ScalarE(exp)-paced; slot1's projections are emitted as
deadline-scheduled units interleaved into slot1's attention kc-loop so they
fill TensorE idle slots, with a dedicated 1-bank PSUM pool. The last q-chunk
of each slot is width-trimmed to the valid length (rounded to 32), shrinking
exp/matmul/DVE work on padded queries. Wo accumulates in the scores PSUM
pool (bufs=2) so the output chain pipelines at the kernel tail.
"""
import sys

if "/opt/trn_rl_repo" not in sys.path:
    sys.path.insert(0, "/opt/trn_rl_repo")

import math
from itertools import permutations

import ml_dtypes
import numpy as np

import concourse.bass as bass
import concourse.mybir as mybir
import concourse.tile as tile
from concourse import bacc
from concourse.bass_utils import run_bass_kernel_spmd

S, B, H, NH, DK = 2048, 4, 1024, 16, 64
N_CORES = 8
BF16 = mybir.dt.bfloat16
F32 = mybir.dt.float32
NPBF16 = ml_dtypes.bfloat16
MASK_BIAS = -30000.0

_prog_cache: dict = {}


def _build_program(NQ, NK, VQ):
    """One SPMD program. Per batch-slot s: NQ[s] 512-wide q chunks (last one
    VQ[s] wide), NK[s] 128-wide k chunks. Slot 0 is the smaller workload."""
    NSCK = [(nk * 128 + 511) // 512 for nk in NK]
    KW = [nk * 128 for nk in NK]                    # k/v valid width
    QW = [(NQ[s] - 1) * 512 + VQ[s] for s in range(2)]  # q valid width
    nc = bacc.Bacc("TRN2", target_bir_lowering=False, debug=False,
                   num_devices=N_CORES)

    d_in = {}
    for s in range(2):
        d_in[f"qT{s}"] = nc.dram_tensor(f"qT{s}", [H, S], BF16, kind="ExternalInput")
        d_in[f"kT{s}"] = nc.dram_tensor(f"kT{s}", [H, S], BF16, kind="ExternalInput")
        d_in[f"vT{s}"] = nc.dram_tensor(f"vT{s}", [H, S], BF16, kind="ExternalInput")
        d_in[f"mb{s}"] = nc.dram_tensor(f"mb{s}", [128, 16], F32, kind="ExternalInput")
        d_in[f"kq{s}"] = nc.dram_tensor(f"kq{s}", [4, 512], F32, kind="ExternalInput")
    d_in["wqT"] = nc.dram_tensor("wqT", [H, 256], BF16, kind="ExternalInput")
    d_in["wkT"] = nc.dram_tensor("wkT", [H, 256], BF16, kind="ExternalInput")
    d_in["wvT"] = nc.dram_tensor("wvT", [H, 256], BF16, kind="ExternalInput")
    d_in["woT"] = nc.dram_tensor("woT", [256, H], BF16, kind="ExternalInput")
    d_out = [nc.dram_tensor(f"y{s}", [H, S], BF16, kind="ExternalOutput")
             for s in range(2)]

    def qcw(s, sc):
        return 512 if sc < NQ[s] - 1 else VQ[s]

    def kcw(s, sc):
        return min(512, KW[s] - sc * 512)

    with tile.TileContext(nc) as tc:
        with tc.tile_pool(name="wpool", bufs=1) as wpool, \
             tc.tile_pool(name="inp", bufs=6) as inp, \
             tc.tile_pool(name="in8", bufs=1) as in8, \
             tc.tile_pool(name="persist", bufs=1) as persist, \
             tc.tile_pool(name="probs", bufs=3) as probsp, \
             tc.tile_pool(name="small", bufs=2) as small, \
             tc.tile_pool(name="att", bufs=3) as attp, \
             tc.tile_pool(name="yst", bufs=3) as ystp:

            # --- weights: consolidated DMAs on the sync queue ---
            # w*_all[p, ic*256 + j] = w*T[ic*128 + p, j]
            wq_all = wpool.tile([128, 2048], BF16, name="wq_all", tag="wq")
            wk_all = wpool.tile([128, 2048], BF16, name="wk_all", tag="wk")
            wv_all = wpool.tile([128, 2048], BF16, name="wv_all", tag="wv")
            # wo_all[p, j*1024 + c] = woT[j*128 + p, c]
            wo_all = wpool.tile([128, 2048], BF16, name="wo_all", tag="wo")
            _wseen = set()

            def wqkv(t, ic, ft):
                return t[:, ic * 256 + ft * 128: ic * 256 + (ft + 1) * 128]

            mb = [wpool.tile([128, 16], F32, name=f"mbt{s}", tag=f"mbt{s}")
                  for s in range(2)]

            def emit_wo_mb():
                for j in range(2):
                    nc.sync.dma_start(
                        out=wo_all[:, j * 1024:(j + 1) * 1024],
                        in_=d_in["woT"].ap()[j * 128:(j + 1) * 128, :])
                for s in range(2):
                    nc.sync.dma_start(out=mb[s][:], in_=d_in[f"mb{s}"].ap())
            ones = wpool.tile([128, 64], BF16, name="ones", tag="ones")
            nc.vector.memset(ones[:], 1.0)

            # --- persistent projection outputs ---
            qTp = [[persist.tile([128, NQ[s] * 512], BF16, name=f"qTp{s}_{p}",
                                 tag=f"qTp{s}_{p}")
                    for p in range(2)] for s in range(2)]
            kTp = [[persist.tile([128, NSCK[s] * 512], BF16, name=f"kTp{s}_{p}",
                                 tag=f"kTp{s}_{p}")
                    for p in range(2)] for s in range(2)]
            vp = [[persist.tile([128, 256], BF16, name=f"vp{s}_{st}", tag=f"vp{s}_{st}")
                   for st in range(NK[s])] for s in range(2)]

            # DMA queue rotation for input streams (keep Scalar clean once
            # attention starts; Sync carries the weights up front)
            s0_queues = [nc.scalar, nc.gpsimd, nc.sync]
            s1_queues = [nc.sync, nc.gpsimd]
            _qi = [0]

            def dma_rot(queues, out, in_):
                q = queues[_qi[0] % len(queues)]
                _qi[0] += 1
                q.dma_start(out=out, in_=in_)

            def emit_w(t, dname, ic):
                # interleave weight-chunk DMAs with the input stream so the
                # first matmuls aren't gated on the full weight load
                if (dname, ic) in _wseen:
                    return
                _wseen.add((dname, ic))
                dma_rot(s0_queues, t[:, ic * 256:(ic + 1) * 256],
                        d_in[dname].ap()[ic * 128:(ic + 1) * 128, :])

            def emit_proj_streamed(s, pool):
                """ic-outer projections with streamed inputs (slot 0)."""
                for kind, wts, dname, nsc, outtiles, cw in (
                        ("q", wq_all, f"qT{s}", NQ[s], qTp[s], qcw),
                        ("k", wk_all, f"kT{s}", NSCK[s], kTp[s], kcw)):
                    wname = "wqT" if kind == "q" else "wkT"
                    ps = [[pool.tile([128, 512], F32,
                                     name=f"pj{kind}{s}_{ft}_{sc}",
                                     tag=f"pj_{ft}_{sc}")
                           for sc in range(nsc)] for ft in range(2)]
                    tw = sum(cw(s, sc) for sc in range(nsc))
                    for ic in range(8):
                        emit_w(wts, wname, ic)
                        it = inp.tile([128, 2048], BF16,
                                      name=f"in{kind}{s}_{ic}", tag="inp")
                        dma_rot(s0_queues,
                                it[:, 0:tw],
                                d_in[dname].ap()[ic * 128:(ic + 1) * 128, 0:tw])
                        for ft in range(2):
                            for sc in range(nsc):
                                w = cw(s, sc)
                                nc.tensor.matmul(
                                    out=ps[ft][sc][:, 0:w],
                                    lhsT=wqkv(wts, ic, ft),
                                    rhs=it[:, sc * 512: sc * 512 + w],
                                    start=(ic == 0), stop=(ic == 7))
                    for ft in range(2):
                        for sc in range(nsc):
                            w = cw(s, sc)
                            if kind == "q":
                                nc.vector.tensor_copy(
                                    outtiles[ft][:, sc * 512: sc * 512 + w],
                                    ps[ft][sc][:, 0:w])
                            else:
                                nc.scalar.copy(
                                    outtiles[ft][:, sc * 512: sc * 512 + w],
                                    ps[ft][sc][:, 0:w])

            def emit_v_prefetch0():
                tiles = []
                for ic in range(8):
                    emit_w(wv_all, "wvT", ic)
                    it = in8.tile([128, KW[0]], BF16, name=f"v0in_{ic}",
                                  tag=f"v0in{ic}")
                    dma_rot(s0_queues, it[:],
                            d_in["vT0"].ap()[ic * 128:(ic + 1) * 128, 0:KW[0]])
                    tiles.append(it)
                return tiles

            def emit_v_resident(s, pool, tiles):
                for st0 in range(0, NK[s], 4):
                    sts = range(st0, min(st0 + 4, NK[s]))
                    psv = {st: pool.tile([128, 256], F32, name=f"pjv{s}_{st}",
                                         tag=f"pj_0_{(st - st0) % 4}")
                           for st in sts}
                    for ic in range(8):
                        for st in sts:
                            nc.tensor.matmul(
                                out=psv[st][:],
                                lhsT=tiles[ic][:, st * 128:(st + 1) * 128],
                                rhs=wv_all[:, ic * 256: ic * 256 + 256],
                                start=(ic == 0), stop=(ic == 7))
                    for st in sts:
                        if st % 2:
                            nc.scalar.copy(vp[s][st][:], psv[st][:])
                        else:
                            nc.vector.tensor_copy(vp[s][st][:], psv[st][:])

            def emit_prefetch(s):
                """Issue all of slot s's input DMAs into dedicated tiles."""
                tiles = {}
                for kind, dname, w in (("v", f"vT{s}", KW[s]),
                                       ("k", f"kT{s}", KW[s]),
                                       ("q", f"qT{s}", QW[s])):
                    for ic in range(8):
                        it = in8.tile([128, w], BF16, name=f"pf{kind}{s}_{ic}",
                                      tag=f"pf{kind}{ic}")
                        dma_rot(s1_queues,
                                it[:],
                                d_in[dname].ap()[ic * 128:(ic + 1) * 128, 0:w])
                        tiles[(kind, ic)] = it
                return tiles

            def proj_units(s, pool, tiles, nkit, kinds=("v", "k", "q")):
                """Deadline-tagged projection units for slot s, consumed by
                interleaving into the attention kc-loop. Deadline = global
                iteration index ((qc*2)+p)*NK + kc of first use."""
                units = []

                def q_unit(ft, sc):
                    def emit():
                        w = qcw(s, sc)
                        pj = pool.tile([128, 512], F32,
                                       name=f"rpjq{s}_{ft}_{sc}", tag="y")
                        for ic in range(8):
                            nc.tensor.matmul(
                                out=pj[:, 0:w],
                                lhsT=wqkv(wq_all, ic, ft),
                                rhs=tiles[("q", ic)][:, sc * 512: sc * 512 + w],
                                start=(ic == 0), stop=(ic == 7))
                        nc.vector.tensor_copy(
                            qTp[s][ft][:, sc * 512: sc * 512 + w], pj[:, 0:w])
                    return emit

                def k_unit(ft, sc):
                    def emit():
                        w = kcw(s, sc)
                        pj = pool.tile([128, 512], F32,
                                       name=f"rpjk{s}_{ft}_{sc}", tag="y")
                        for ic in range(8):
                            nc.tensor.matmul(
                                out=pj[:, 0:w],
                                lhsT=wqkv(wk_all, ic, ft),
                                rhs=tiles[("k", ic)][:, sc * 512: sc * 512 + w],
                                start=(ic == 0), stop=(ic == 7))
                        nc.vector.tensor_copy(
                            kTp[s][ft][:, sc * 512: sc * 512 + w], pj[:, 0:w])
                    return emit

                def v_unit(st):
                    def emit():
                        pj = pool.tile([128, 512], F32,
                                       name=f"rpjv{s}_{st}", tag="y")
                        for ic in range(8):
                            nc.tensor.matmul(
                                out=pj[:, 0:256],
                                lhsT=tiles[("v", ic)][:, st * 128:(st + 1) * 128],
                                rhs=wv_all[:, ic * 256: ic * 256 + 256],
                                start=(ic == 0), stop=(ic == 7))
                        nc.vector.tensor_copy(vp[s][st][:], pj[:, 0:256])
                    return emit

                vk = []
                if "v" in kinds:
                    for st in range(NK[s]):
                        vk.append(v_unit(st))
                if "k" in kinds:
                    for ft in range(2):
                        for sc in range(NSCK[s]):
                            vk.append(k_unit(ft, sc))
                if "q" in kinds:
                    for ft in range(2):
                        for sc in range(NQ[s]):
                            units.append((max(0, (sc * 2 + ft) * nkit - nkit // 2),
                                          q_unit(ft, sc)))
                units.sort(key=lambda u: u[0])
                return vk, units

            def emit_attention(s, psc, pat, pz, pwo, units=None, carry_wo=None,
                               lead=4):
                units = list(units) if units else []
                wo_pending = list(carry_wo) if carry_wo else []
                nkit = NK[s]

                def drain_units(cur_idx, pre_group=False):
                    # hard deadlines: emit everything overdue; then up to two
                    # early ones to smooth PE load
                    n = 0
                    while units and (units[0][0] <= cur_idx
                                     or (n < 2 and units[0][0] <= cur_idx + lead)):
                        units.pop(0)[1]()
                        n += 1
                    m = 0
                    while wo_pending and wo_pending[0][0] <= cur_idx and m < 2:
                        wo_pending.pop(0)[1](None)
                        m += 1

                def make_wo_unit(qc, ot, ab_pair, W):
                    def emit(tag):
                        if tag is None:
                            yps = pwo.tile([128, 512], F32,
                                           name=f"yp{s}_{qc}_{ot}", tag="y")
                            ypv = yps[:, 0:W]
                        else:
                            yps = psc.tile([128, 1024], F32,
                                           name=f"yp{s}_{qc}_{ot}", tag="sc")
                            ypv = yps[:, 0:W]
                        for j in range(2):
                            nc.tensor.matmul(
                                out=ypv,
                                lhsT=wo_all[:, j * 1024 + ot * 128: j * 1024 + (ot + 1) * 128],
                                rhs=ab_pair[j][:, 0:W],
                                start=(j == 0), stop=(j == 1))
                        ysb = ystp.tile([128, 512], BF16,
                                        name=f"ysb{s}_{qc}_{ot}", tag="ysb")
                        nc.vector.tensor_copy(ysb[:, 0:W], ypv)
                        (nc.gpsimd if ot % 2 else nc.sync).dma_start(
                            out=d_out[s].ap()[ot * 128:(ot + 1) * 128,
                                              qc * 512: qc * 512 + W],
                            in_=ysb[:, 0:W])
                    return emit

                for qc in range(NQ[s]):
                    W = qcw(s, qc)
                    kqr = small.tile([128, 512], F32, name=f"kqr{s}_{qc}",
                                     tag="kqr")
                    nc.gpsimd.dma_start(
                        out=kqr[:, 0:W],
                        in_=bass.AP(tensor=d_in[f"kq{s}"], offset=qc * 512,
                                    ap=[[0, 128], [1, W]]))
                    att_sb = []
                    for p in range(2):
                        base_idx = (qc * 2 + p) * nkit
                        # hard-deadline units must precede this group's
                        # first scores emission (they share the PE queue)
                        drain_units(base_idx)
                        attn = pat.tile([128, 512], F32,
                                        name=f"at{s}_{qc}_{p}", tag="at")
                        zps = pz.tile([128, 512], F32,
                                      name=f"z{s}_{qc}_{p}", tag="z")

                        # software-pipelined: scores(kc+1) emitted before
                        # PV(kc) so the in-order PE queue never stalls on exp
                        def emit_scores(kc):
                            sc_ps = psc.tile([128, 1024], F32,
                                             name=f"s{s}_{qc}_{p}_{kc}",
                                             tag="sc")
                            pr = probsp.tile([128, 1024], BF16,
                                             name=f"pr{s}_{qc}_{p}_{kc}",
                                             tag="pr")
                            for hh in range(2):
                                hsl = slice(hh * 64, hh * 64 + 64)
                                # 512-strided blocks: the two row-group-
                                # concurrent matmuls must drain to
                                # different PSUM banks
                                nc.tensor.matmul(
                                    out=sc_ps[:, hh * 512: hh * 512 + W],
                                    lhsT=kTp[s][p][hsl, kc * 128:(kc + 1) * 128],
                                    rhs=qTp[s][p][hsl, qc * 512: qc * 512 + W],
                                    start=True, stop=True)
                            if W == 512:
                                exp_in = sc_ps[:, 0:1024]
                                exp_out = pr[:, 0:1024]
                            else:
                                exp_in = sc_ps[:].rearrange(
                                    "p (b w) -> p b w", b=2)[:, :, 0:W]
                                exp_out = pr[:, 0:2 * W].rearrange(
                                    "p (b w) -> p b w", b=2)
                            nc.scalar.activation(
                                out=exp_out, in_=exp_in,
                                func=mybir.ActivationFunctionType.Exp,
                                bias=mb[s][:, kc:kc + 1],
                                scale=1.0 / math.sqrt(DK))
                            return pr

                        pr_next = emit_scores(0)
                        for kc in range(nkit):
                            drain_units(base_idx + kc)
                            first, last = kc == 0, kc == nkit - 1
                            pr = pr_next
                            if not last:
                                pr_next = emit_scores(kc + 1)
                            for hh in range(2):
                                hsl = slice(hh * 64, hh * 64 + 64)
                                nc.tensor.matmul(
                                    out=attn[hsl, 0:W],
                                    lhsT=vp[s][kc][:, p * 128 + hh * 64:p * 128 + (hh + 1) * 64],
                                    rhs=pr[:, hh * W:(hh + 1) * W],
                                    start=first, stop=last)
                                nc.tensor.matmul(
                                    out=zps[hsl, 0:W],
                                    lhsT=ones[:, :], rhs=pr[:, hh * W:(hh + 1) * W],
                                    start=first, stop=last)
                        # normalize: attn * keepq / Z
                        rz = small.tile([128, 512], F32,
                                        name=f"rz{s}_{qc}_{p}", tag="rz")
                        nc.vector.reciprocal_approx_fast(out=rz[:, 0:W],
                                                         in_=zps[:, 0:W])
                        nc.vector.tensor_mul(rz[:, 0:W], rz[:, 0:W], kqr[:, 0:W])
                        ab = attp.tile([128, 512], BF16,
                                       name=f"ab{s}_{qc}_{p}", tag=f"ab{p}")
                        nc.vector.tensor_mul(ab[:, 0:W], attn[:, 0:W], rz[:, 0:W])
                        att_sb.append(ab)
                    # Wo is deferred: one unit per iteration of the NEXT
                    # group, so the output chain never gates the next
                    # group's scores/exp pipeline
                    for ot in range(8):
                        wo_pending.append(((qc + 1) * 2 * nkit + 2 * ot,
                                           make_wo_unit(qc, ot, att_sb, W)))
                # flush stragglers; alternate between the 'y' bank and the
                # now-idle scores slots so the tail chain pipelines
                while units:
                    units.pop(0)[1]()
                return wo_pending

            # slot 0 projections use the full PSUM (released afterwards)
            with tc.tile_pool(name="pproj", bufs=1, space="PSUM") as pproj:
                emit_proj_streamed(0, pproj)
                v0tiles = emit_v_prefetch0()
                emit_v_resident(0, pproj, v0tiles)
            emit_wo_mb()
            tiles1 = emit_prefetch(1)
            with tc.tile_pool(name="psc", bufs=2, space="PSUM") as psc, \
                 tc.tile_pool(name="pat", bufs=2, space="PSUM") as pat, \
                 tc.tile_pool(name="pz", bufs=1, space="PSUM") as pz, \
                 tc.tile_pool(name="pwo", bufs=1, space="PSUM") as pwo:
                vk1, units1 = proj_units(1, pwo, tiles1, NK[1])
                n0 = NQ[0] * 2 * NK[0]
                sp = max(1, (n0 - 6) // max(1, len(vk1)))
                units0 = [(2 + sp * i, fn) for i, fn in enumerate(vk1)]
                left0 = emit_attention(0, psc, pat, pz, pwo, units=units0)
                carry = [(i, fn) for i, (_, fn) in enumerate(left0)]
                left1 = emit_attention(1, psc, pat, pz, pwo, units=units1,
                                       carry_wo=carry)
                for i, (_, fn) in enumerate(left1):
                    fn("sc" if i % 2 == 0 else None)
    nc.compile()
    return nc


def _get_program(NQ, NK, VQ):
    key = (tuple(NQ), tuple(NK), tuple(VQ))
    if key not in _prog_cache:
        _prog_cache[key] = _build_program(list(NQ), list(NK), list(VQ))
    return _prog_cache[key]


def kernel(value, key, query, padding_mask, Wq, Wk, Wv, Wo):
    value = np.asarray(value)
    key = np.asarray(key)
    query = np.asarray(query)
    padding_mask = np.asarray(padding_mask)
    Wq, Wk, Wv, Wo = (np.asarray(a) for a in (Wq, Wk, Wv, Wo))

    lengths = (~padding_mask).sum(axis=0).astype(int)  # (B,)

    # --- batch pairing: assign batches to (group, slot) minimizing baked work ---
    def slot_counts(assign):
        nq = [max((int(lengths[assign[g][sl]]) + 511) // 512 for g in range(2))
              for sl in range(2)]
        nk = [max((int(lengths[assign[g][sl]]) + 127) // 128 for g in range(2))
              for sl in range(2)]
        return nq, nk

    best = None
    for perm in permutations(range(B)):
        a = ((perm[0], perm[1]), (perm[2], perm[3]))
        nq, nk = slot_counts(a)
        c = nq[0] * nk[0] + nq[1] * nk[1]
        if best is None or c < best[0]:
            best = (c, a)
    assign = best[1]
    nq, nk = slot_counts(assign)
    # slot 0 should be the smaller workload (its projections can't overlap)
    if nq[0] * nk[0] > nq[1] * nk[1]:
        assign = tuple((g[1], g[0]) for g in assign)
        nq, nk = slot_counts(assign)
    NQ, NK = nq, nk
    # trimmed width of the last q chunk per slot (multiple of 32)
    VQ = []
    for sl in range(2):
        maxlen = max(int(lengths[assign[g][sl]]) for g in range(2))
        v = maxlen - (NQ[sl] - 1) * 512
        VQ.append(min(512, (v + 31) // 32 * 32))

    nc = _get_program(NQ, NK, VQ)

    # --- per-core inputs ---
    WqT = np.ascontiguousarray(Wq.T).astype(NPBF16)
    WkT = np.ascontiguousarray(Wk.T).astype(NPBF16)
    WvT = np.ascontiguousarray(Wv.T).astype(NPBF16)
    WoT = np.ascontiguousarray(Wo.T).astype(NPBF16)

    batch_qT, batch_kT, batch_vT, batch_mb, batch_kq = {}, {}, {}, {}, {}
    for b in range(B):
        batch_qT[b] = np.ascontiguousarray(query[:, b, :].T).astype(NPBF16)
        batch_kT[b] = np.ascontiguousarray(key[:, b, :].T).astype(NPBF16)
        batch_vT[b] = np.ascontiguousarray(value[:, b, :].T).astype(NPBF16)
        kpos = np.arange(S).reshape(16, 128)  # [kchunk, kpos]
        mbv = np.where(kpos >= lengths[b], np.float32(MASK_BIAS), np.float32(0.0))
        batch_mb[b] = np.ascontiguousarray(mbv.T).astype(np.float32)  # [128, 16]
        batch_kq[b] = (np.arange(S).reshape(4, 512) < lengths[b]).astype(np.float32)

    in_maps = []
    for c in range(N_CORES):
        g, hq = c // 4, c % 4
        f0 = hq * 256
        m = {
            "wqT": np.ascontiguousarray(WqT[:, f0:f0 + 256]),
            "wkT": np.ascontiguousarray(WkT[:, f0:f0 + 256]),
            "wvT": np.ascontiguousarray(WvT[:, f0:f0 + 256]),
            "woT": np.ascontiguousarray(WoT[f0:f0 + 256, :]),
        }
        for sl in range(2):
            b = assign[g][sl]
            m[f"qT{sl}"] = batch_qT[b]
            m[f"kT{sl}"] = batch_kT[b]
            m[f"vT{sl}"] = batch_vT[b]
            m[f"mb{sl}"] = batch_mb[b]
            m[f"kq{sl}"] = batch_kq[b]
        in_maps.append(m)

    res = run_bass_kernel_spmd(nc, in_maps, list(range(N_CORES)))

    # --- gather: sum 4 head-quad partials per batch, transpose ---
    out = np.zeros((S, B, H), dtype=np.float32)
    for g in range(2):
        for sl in range(2):
            b = assign[g][sl]
            acc = np.zeros((H, S), dtype=np.float32)
            for hq in range(4):
                c = g * 4 + hq
                acc += res.results[c][f"y{sl}"].astype(np.float32)
            out[:, b, :] = acc.T
    return out


# revision 28
# speedup vs baseline: 1.1073x; 1.1073x over previous
"""Multi-head attention (S=2048, B=4, H=1024, NH=16) on 8 Trainium2 NeuronCores.

Sharding: each core handles 2 batches x 4 heads (batch pairs balanced by
valid length; tensor-parallel over heads). Within a core everything is bf16
matmul / fp32 accumulate:
  1. q,k projected d-major (qT/kT: [dims, seq]), v seq-major ([seq, dims])
  2. scoresT[k,q] per head-pair via row-tiled matmuls (row_grp concurrency)
  3. mask+scale+exp fused on ScalarE (per-partition bias; PAD keys -> exp 0)
  4. PV col-tiled accumulates attnT; Z row-sums via all-ones stationary
     matmul (col_grp-concurrent with PV)
  5. attnT normalized by 1/Z; keepq applied before the Wo output multiply
  6. Wo projection -> yT partial [H, S]; host sums 4 partials/batch

Schedule: slot0 projections stream from HBM across 4 DMA queues; slot0
attention is ScalarE(exp)-paced; slot1's projections are emitted as
deadline-scheduled units interleaved into slot1's attention kc-loop so they
fill TensorE idle slots, with a dedicated 1-bank PSUM pool. The last q-chunk
of each slot is width-trimmed to the valid length (rounded to 32), shrinking
exp/matmul/DVE work on padded queries. Wo accumulates in the scores PSUM
pool (bufs=2) so the output chain pipelines at the kernel tail.
"""
import sys

if "/opt/trn_rl_repo" not in sys.path:
    sys.path.insert(0, "/opt/trn_rl_repo")

import math
from itertools import permutations

import ml_dtypes
import numpy as np

import concourse.bass as bass
import concourse.mybir as mybir
import concourse.tile as tile
from concourse import bacc
from concourse.bass_utils import run_bass_kernel_spmd

S, B, H, NH, DK = 2048, 4, 1024, 16, 64
N_CORES = 8
BF16 = mybir.dt.bfloat16
F32 = mybir.dt.float32
NPBF16 = ml_dtypes.bfloat16
MASK_BIAS = -30000.0

_prog_cache: dict = {}


def _build_program(NQ, NK, VQ):
    """One SPMD program. Per batch-slot s: NQ[s] 512-wide q chunks (last one
    VQ[s] wide), NK[s] 128-wide k chunks. Slot 0 is the smaller workload."""
    NSCK = [(nk * 128 + 511) // 512 for nk in NK]
    KW = [nk * 128 for nk in NK]                    # k/v valid width
    QW = [(NQ[s] - 1) * 512 + VQ[s] for s in range(2)]  # q valid width
    nc = bacc.Bacc("TRN2", target_bir_lowering=False, debug=False,
                   num_devices=N_CORES)

    d_in = {}
    for s in range(2):
        d_in[f"qT{s}"] = nc.dram_tensor(f"qT{s}", [H, S], BF16, kind="ExternalInput")
        d_in[f"kT{s}"] = nc.dram_tensor(f"kT{s}", [H, S], BF16, kind="ExternalInput")
        d_in[f"vT{s}"] = nc.dram_tensor(f"vT{s}", [H, S], BF16, kind="ExternalInput")
        d_in[f"mb{s}"] = nc.dram_tensor(f"mb{s}", [128, 16], F32, kind="ExternalInput")
        d_in[f"kq{s}"] = nc.dram_tensor(f"kq{s}", [4, 512], F32, kind="ExternalInput")
    d_in["wqT"] = nc.dram_tensor("wqT", [H, 256], BF16, kind="ExternalInput")
    d_in["wkT"] = nc.dram_tensor("wkT", [H, 256], BF16, kind="ExternalInput")
    d_in["wvT"] = nc.dram_tensor("wvT", [H, 256], BF16, kind="ExternalInput")
    d_in["woT"] = nc.dram_tensor("woT", [256, H], BF16, kind="ExternalInput")
    d_out = [nc.dram_tensor(f"y{s}", [H, S], BF16, kind="ExternalOutput")
             for s in range(2)]

    def qcw(s, sc):
        return 512 if sc < NQ[s] - 1 else VQ[s]

    def kcw(s, sc):
        return min(512, KW[s] - sc * 512)

    with tile.TileContext(nc) as tc:
        with tc.tile_pool(name="wpool", bufs=1) as wpool, \
             tc.tile_pool(name="inp", bufs=6) as inp, \
             tc.tile_pool(name="in8", bufs=1) as in8, \
             tc.tile_pool(name="persist", bufs=1) as persist, \
             tc.tile_pool(name="probs", bufs=3) as probsp, \
             tc.tile_pool(name="small", bufs=2) as small, \
             tc.tile_pool(name="att", bufs=3) as attp, \
             tc.tile_pool(name="yst", bufs=3) as ystp:

            # --- weights: consolidated DMAs on the sync queue ---
            # w*_all[p, ic*256 + j] = w*T[ic*128 + p, j]
            wq_all = wpool.tile([128, 2048], BF16, name="wq_all", tag="wq")
            wk_all = wpool.tile([128, 2048], BF16, name="wk_all", tag="wk")
            wv_all = wpool.tile([128, 2048], BF16, name="wv_all", tag="wv")
            # wo_all[p, j*1024 + c] = woT[j*128 + p, c]
            wo_all = wpool.tile([128, 2048], BF16, name="wo_all", tag="wo")
            _wseen = set()

            def wqkv(t, ic, ft):
                return t[:, ic * 256 + ft * 128: ic * 256 + (ft + 1) * 128]

            mb = [wpool.tile([128, 16], F32, name=f"mbt{s}", tag=f"mbt{s}")
                  for s in range(2)]

            def emit_wo_mb():
                for j in range(2):
                    nc.sync.dma_start(
                        out=wo_all[:, j * 1024:(j + 1) * 1024],
                        in_=d_in["woT"].ap()[j * 128:(j + 1) * 128, :])
                for s in range(2):
                    nc.sync.dma_start(out=mb[s][:], in_=d_in[f"mb{s}"].ap())
            ones = wpool.tile([128, 64], BF16, name="ones", tag="ones")
            nc.vector.memset(ones[:], 1.0)

            # --- persistent projection outputs ---
            qTp = [[persist.tile([128, NQ[s] * 512], BF16, name=f"qTp{s}_{p}",
                                 tag=f"qTp{s}_{p}")
                    for p in range(2)] for s in range(2)]
            kTp = [[persist.tile([128, NSCK[s] * 512], BF16, name=f"kTp{s}_{p}",
                                 tag=f"kTp{s}_{p}")
                    for p in range(2)] for s in range(2)]
            vp = [[persist.tile([128, 256], BF16, name=f"vp{s}_{st}", tag=f"vp{s}_{st}")
                   for st in range(NK[s])] for s in range(2)]

            # DMA queue rotation for input streams (keep Scalar clean once
            # attention starts; Sync carries the weights up front)
            s0_queues = [nc.scalar, nc.gpsimd, nc.sync]
            s1_queues = [nc.sync, nc.gpsimd]
            _qi = [0]

            def dma_rot(queues, out, in_):
                q = queues[_qi[0] % len(queues)]
                _qi[0] += 1
                q.dma_start(out=out, in_=in_)

            def emit_w(t, dname, ic):
                # interleave weight-chunk DMAs with the input stream so the
                # first matmuls aren't gated on the full weight load
                if (dname, ic) in _wseen:
                    return
                _wseen.add((dname, ic))
                dma_rot(s0_queues, t[:, ic * 256:(ic + 1) * 256],
                        d_in[dname].ap()[ic * 128:(ic + 1) * 128, :])

            def emit_proj_streamed(s, pool):
                """ic-outer projections with streamed inputs (slot 0)."""
                for kind, wts, dname, nsc, outtiles, cw in (
                        ("q", wq_all, f"qT{s}", NQ[s], qTp[s], qcw),
                        ("k", wk_all, f"kT{s}", NSCK[s], kTp[s], kcw)):
                    wname = "wqT" if kind == "q" else "wkT"
                    ps = [[pool.tile([128, 512], F32,
                                     name=f"pj{kind}{s}_{ft}_{sc}",
                                     tag=f"pj_{ft}_{sc}")
                           for sc in range(nsc)] for ft in range(2)]
                    tw = sum(cw(s, sc) for sc in range(nsc))
                    for ic in range(8):
                        emit_w(wts, wname, ic)
                        it = inp.tile([128, 2048], BF16,
                                      name=f"in{kind}{s}_{ic}", tag="inp")
                        dma_rot(s0_queues,
                                it[:, 0:tw],
                                d_in[dname].ap()[ic * 128:(ic + 1) * 128, 0:tw])
                        for ft in range(2):
                            for sc in range(nsc):
                                w = cw(s, sc)
                                nc.tensor.matmul(
                                    out=ps[ft][sc][:, 0:w],
                                    lhsT=wqkv(wts, ic, ft),
                                    rhs=it[:, sc * 512: sc * 512 + w],
                                    start=(ic == 0), stop=(ic == 7))
                    for ft in range(2):
                        for sc in range(nsc):
                            w = cw(s, sc)
                            if kind == "q":
                                nc.vector.tensor_copy(
                                    outtiles[ft][:, sc * 512: sc * 512 + w],
                                    ps[ft][sc][:, 0:w])
                            else:
                                nc.scalar.copy(
                                    outtiles[ft][:, sc * 512: sc * 512 + w],
                                    ps[ft][sc][:, 0:w])

            def emit_v_prefetch0():
                tiles = []
                for ic in range(8):
                    emit_w(wv_all, "wvT", ic)
                    it = in8.tile([128, KW[0]], BF16, name=f"v0in_{ic}",
                                  tag=f"v0in{ic}")
                    dma_rot(s0_queues, it[:],
                            d_in["vT0"].ap()[ic * 128:(ic + 1) * 128, 0:KW[0]])
                    tiles.append(it)
                return tiles

            def emit_v_resident(s, pool, tiles):
                for st0 in range(0, NK[s], 8):
                    sts = range(st0, min(st0 + 8, NK[s]))
                    psv = {st: pool.tile([128, 256], F32, name=f"pjv{s}_{st}",
                                         tag=f"pj_{(st - st0) // 4}_{(st - st0) % 4}")
                           for st in sts}
                    for ic in range(8):
                        for st in sts:
                            nc.tensor.matmul(
                                out=psv[st][:],
                                lhsT=tiles[ic][:, st * 128:(st + 1) * 128],
                                rhs=wv_all[:, ic * 256: ic * 256 + 256],
                                start=(ic == 0), stop=(ic == 7))
                    for st in sts:
                        if st % 2:
                            nc.scalar.copy(vp[s][st][:], psv[st][:])
                        else:
                            nc.vector.tensor_copy(vp[s][st][:], psv[st][:])

            def emit_prefetch(s):
                """Issue all of slot s's input DMAs into dedicated tiles."""
                tiles = {}
                for kind, dname, w in (("v", f"vT{s}", KW[s]),
                                       ("k", f"kT{s}", KW[s]),
                                       ("q", f"qT{s}", QW[s])):
                    for ic in range(8):
                        it = in8.tile([128, w], BF16, name=f"pf{kind}{s}_{ic}",
                                      tag=f"pf{kind}{ic}")
                        dma_rot(s1_queues,
                                it[:],
                                d_in[dname].ap()[ic * 128:(ic + 1) * 128, 0:w])
                        tiles[(kind, ic)] = it
                return tiles

            def proj_units(s, pool, tiles, nkit, kinds=("v", "k", "q")):
                """Deadline-tagged projection units for slot s, consumed by
                interleaving into the attention kc-loop. Deadline = global
                iteration index ((qc*2)+p)*NK + kc of first use."""
                units = []

                def q_unit(ft, sc):
                    def emit():
                        w = qcw(s, sc)
                        pj = pool.tile([128, 512], F32,
                                       name=f"rpjq{s}_{ft}_{sc}", tag="y")
                        for ic in range(8):
                            nc.tensor.matmul(
                                out=pj[:, 0:w],
                                lhsT=wqkv(wq_all, ic, ft),
                                rhs=tiles[("q", ic)][:, sc * 512: sc * 512 + w],
                                start=(ic == 0), stop=(ic == 7))
                        nc.vector.tensor_copy(
                            qTp[s][ft][:, sc * 512: sc * 512 + w], pj[:, 0:w])
                    return emit

                def k_unit(ft, sc):
                    def emit():
                        w = kcw(s, sc)
                        pj = pool.tile([128, 512], F32,
                                       name=f"rpjk{s}_{ft}_{sc}", tag="y")
                        for ic in range(8):
                            nc.tensor.matmul(
                                out=pj[:, 0:w],
                                lhsT=wqkv(wk_all, ic, ft),
                                rhs=tiles[("k", ic)][:, sc * 512: sc * 512 + w],
                                start=(ic == 0), stop=(ic == 7))
                        nc.vector.tensor_copy(
                            kTp[s][ft][:, sc * 512: sc * 512 + w], pj[:, 0:w])
                    return emit

                def v_unit(st):
                    def emit():
                        pj = pool.tile([128, 512], F32,
                                       name=f"rpjv{s}_{st}", tag="y")
                        for ic in range(8):
                            nc.tensor.matmul(
                                out=pj[:, 0:256],
                                lhsT=tiles[("v", ic)][:, st * 128:(st + 1) * 128],
                                rhs=wv_all[:, ic * 256: ic * 256 + 256],
                                start=(ic == 0), stop=(ic == 7))
                        nc.vector.tensor_copy(vp[s][st][:], pj[:, 0:256])
                    return emit

                vk = []
                if "v" in kinds:
                    for st in range(NK[s]):
                        vk.append(v_unit(st))
                if "k" in kinds:
                    for ft in range(2):
                        for sc in range(NSCK[s]):
                            vk.append(k_unit(ft, sc))
                if "q" in kinds:
                    for ft in range(2):
                        for sc in range(NQ[s]):
                            units.append((max(0, (sc * 2 + ft) * nkit - nkit // 2),
                                          q_unit(ft, sc)))
                units.sort(key=lambda u: u[0])
                return vk, units

            def emit_attention(s, psc, pat, pz, pwo, units=None, carry_wo=None,
                               lead=4):
                units = list(units) if units else []
                wo_pending = list(carry_wo) if carry_wo else []
                nkit = NK[s]

                def drain_units(cur_idx, pre_group=False):
                    # hard deadlines: emit everything overdue; then up to two
                    # early ones to smooth PE load
                    n = 0
                    while units and (units[0][0] <= cur_idx
                                     or (n < 2 and units[0][0] <= cur_idx + lead)):
                        units.pop(0)[1]()
                        n += 1
                    m = 0
                    while wo_pending and wo_pending[0][0] <= cur_idx and m < 2:
                        wo_pending.pop(0)[1](None)
                        m += 1

                def make_wo_unit(qc, ot, ab_pair, W):
                    def emit(tag):
                        if tag is None:
                            yps = pwo.tile([128, 512], F32,
                                           name=f"yp{s}_{qc}_{ot}", tag="y")
                            ypv = yps[:, 0:W]
                        else:
                            yps = psc.tile([128, 1024], F32,
                                           name=f"yp{s}_{qc}_{ot}", tag="sc")
                            ypv = yps[:, 0:W]
                        for j in range(2):
                            nc.tensor.matmul(
                                out=ypv,
                                lhsT=wo_all[:, j * 1024 + ot * 128: j * 1024 + (ot + 1) * 128],
                                rhs=ab_pair[j][:, 0:W],
                                start=(j == 0), stop=(j == 1))
                        ysb = ystp.tile([128, 512], BF16,
                                        name=f"ysb{s}_{qc}_{ot}", tag="ysb")
                        nc.vector.tensor_copy(ysb[:, 0:W], ypv)
                        (nc.gpsimd if ot % 2 else nc.sync).dma_start(
                            out=d_out[s].ap()[ot * 128:(ot + 1) * 128,
                                              qc * 512: qc * 512 + W],
                            in_=ysb[:, 0:W])
                    return emit

                for qc in range(NQ[s]):
                    W = qcw(s, qc)
                    kqr = small.tile([128, 512], F32, name=f"kqr{s}_{qc}",
                                     tag="kqr")
                    nc.gpsimd.dma_start(
                        out=kqr[:, 0:W],
                        in_=bass.AP(tensor=d_in[f"kq{s}"], offset=qc * 512,
                                    ap=[[0, 128], [1, W]]))
                    att_sb = []
                    for p in range(2):
                        base_idx = (qc * 2 + p) * nkit
                        # hard-deadline units must precede this group's
                        # first scores emission (they share the PE queue)
                        drain_units(base_idx)
                        attn = pat.tile([128, 512], F32,
                                        name=f"at{s}_{qc}_{p}", tag="at")
                        zps = pz.tile([128, 512], F32,
                                      name=f"z{s}_{qc}_{p}", tag="z")

                        # software-pipelined: scores(kc+1) emitted before
                        # PV(kc) so the in-order PE queue never stalls on exp
                        def emit_scores(kc):
                            sc_ps = psc.tile([128, 1024], F32,
                                             name=f"s{s}_{qc}_{p}_{kc}",
                                             tag="sc")
                            pr = probsp.tile([128, 1024], BF16,
                                             name=f"pr{s}_{qc}_{p}_{kc}",
                                             tag="pr")
                            for hh in range(2):
                                hsl = slice(hh * 64, hh * 64 + 64)
                                # 512-strided blocks: the two row-group-
                                # concurrent matmuls must drain to
                                # different PSUM banks
                                nc.tensor.matmul(
                                    out=sc_ps[:, hh * 512: hh * 512 + W],
                                    lhsT=kTp[s][p][hsl, kc * 128:(kc + 1) * 128],
                                    rhs=qTp[s][p][hsl, qc * 512: qc * 512 + W],
                                    start=True, stop=True)
                            if W == 512:
                                exp_in = sc_ps[:, 0:1024]
                                exp_out = pr[:, 0:1024]
                            else:
                                exp_in = sc_ps[:].rearrange(
                                    "p (b w) -> p b w", b=2)[:, :, 0:W]
                                exp_out = pr[:, 0:2 * W].rearrange(
                                    "p (b w) -> p b w", b=2)
                            nc.scalar.activation(
                                out=exp_out, in_=exp_in,
                                func=mybir.ActivationFunctionType.Exp,
                                bias=mb[s][:, kc:kc + 1],
                                scale=1.0 / math.sqrt(DK))
                            return pr

                        pr_next = emit_scores(0)
                        for kc in range(nkit):
                            drain_units(base_idx + kc)
                            first, last = kc == 0, kc == nkit - 1
                            pr = pr_next
                            if not last:
                                pr_next = emit_scores(kc + 1)
                            for hh in range(2):
                                hsl = slice(hh * 64, hh * 64 + 64)
                                nc.tensor.matmul(
                                    out=attn[hsl, 0:W],
                                    lhsT=vp[s][kc][:, p * 128 + hh * 64:p * 128 + (hh + 1) * 64],
                                    rhs=pr[:, hh * W:(hh + 1) * W],
                                    start=first, stop=last)
                                nc.tensor.matmul(
                                    out=zps[hsl, 0:W],
                                    lhsT=ones[:, :], rhs=pr[:, hh * W:(hh + 1) * W],
                                    start=first, stop=last)
                        # normalize: attn * keepq / Z
                        rz = small.tile([128, 512], F32,
                                        name=f"rz{s}_{qc}_{p}", tag="rz")
                        nc.vector.reciprocal_approx_fast(out=rz[:, 0:W],
                                                         in_=zps[:, 0:W])
                        nc.vector.tensor_mul(rz[:, 0:W], rz[:, 0:W], kqr[:, 0:W])
                        ab = attp.tile([128, 512], BF16,
                                       name=f"ab{s}_{qc}_{p}", tag=f"ab{p}")
                        nc.vector.tensor_mul(ab[:, 0:W], attn[:, 0:W], rz[:, 0:W])
                        att_sb.append(ab)
                    # Wo is deferred: one unit per iteration of the NEXT
                    # group, so the output chain never gates the next
                    # group's scores/exp pipeline
                    for ot in range(8):
                        wo_pending.append(((qc + 1) * 2 * nkit + 2 * ot,
                                           make_wo_unit(qc, ot, att_sb, W)))
                # flush stragglers; alternate between the 'y' bank and the
                # now-idle scores slots so the tail chain pipelines
                while units:
                    units.pop(0)[1]()
                return wo_pending

            # slot 0 projections use the full PSUM (released afterwards)
            with tc.tile_pool(name="pproj", bufs=1, space="PSUM") as pproj:
                v0tiles = emit_v_prefetch0()
                emit_proj_streamed(0, pproj)
                emit_v_resident(0, pproj, v0tiles)
            emit_wo_mb()
            tiles1 = emit_prefetch(1)
            with tc.tile_pool(name="psc", bufs=2, space="PSUM") as psc, \
                 tc.tile_pool(name="pat", bufs=2, space="PSUM") as pat, \
                 tc.tile_pool(name="pz", bufs=1, space="PSUM") as pz, \
                 tc.tile_pool(name="pwo", bufs=1, space="PSUM") as pwo:
                vk1, units1 = proj_units(1, pwo, tiles1, NK[1])
                n0 = NQ[0] * 2 * NK[0]
                sp = max(1, (n0 - 6) // max(1, len(vk1)))
                units0 = [(2 + sp * i, fn) for i, fn in enumerate(vk1)]
                left0 = emit_attention(0, psc, pat, pz, pwo, units=units0)
                carry = [(i, fn) for i, (_, fn) in enumerate(left0)]
                left1 = emit_attention(1, psc, pat, pz, pwo, units=units1,
                                       carry_wo=carry)
                for i, (_, fn) in enumerate(left1):
                    fn("sc" if i % 2 == 0 else None)
    nc.compile()
    return nc


def _get_program(NQ, NK, VQ):
    key = (tuple(NQ), tuple(NK), tuple(VQ))
    if key not in _prog_cache:
        _prog_cache[key] = _build_program(list(NQ), list(NK), list(VQ))
    return _prog_cache[key]


def kernel(value, key, query, padding_mask, Wq, Wk, Wv, Wo):
    value = np.asarray(value)
    key = np.asarray(key)
    query = np.asarray(query)
    padding_mask = np.asarray(padding_mask)
    Wq, Wk, Wv, Wo = (np.asarray(a) for a in (Wq, Wk, Wv, Wo))

    lengths = (~padding_mask).sum(axis=0).astype(int)  # (B,)

    # --- batch pairing: assign batches to (group, slot) minimizing baked work ---
    def slot_counts(assign):
        nq = [max((int(lengths[assign[g][sl]]) + 511) // 512 for g in range(2))
              for sl in range(2)]
        nk = [max((int(lengths[assign[g][sl]]) + 127) // 128 for g in range(2))
              for sl in range(2)]
        return nq, nk

    best = None
    for perm in permutations(range(B)):
        a = ((perm[0], perm[1]), (perm[2], perm[3]))
        nq, nk = slot_counts(a)
        c = nq[0] * nk[0] + nq[1] * nk[1]
        if best is None or c < best[0]:
            best = (c, a)
    assign = best[1]
    nq, nk = slot_counts(assign)
    # slot 0 should be the smaller workload (its projections can't overlap)
    if nq[0] * nk[0] > nq[1] * nk[1]:
        assign = tuple((g[1], g[0]) for g in assign)
        nq, nk = slot_counts(assign)
    NQ, NK = nq, nk
    # trimmed width of the last q chunk per slot (multiple of 32)
    VQ = []
    for sl in range(2):
        maxlen = max(int(lengths[assign[g][sl]]) for g in range(2))
        v = maxlen - (NQ[sl] - 1) * 512
        VQ.append(min(512, (v + 31) // 32 * 32))

    nc = _get_program(NQ, NK, VQ)

    # --- per-core inputs ---
    WqT = np.ascontiguousarray(Wq.T).astype(NPBF16)
    WkT = np.ascontiguousarray(Wk.T).astype(NPBF16)
    WvT = np.ascontiguousarray(Wv.T).astype(NPBF16)
    WoT = np.ascontiguousarray(Wo.T).astype(NPBF16)

    batch_qT, batch_kT, batch_vT, batch_mb, batch_kq = {}, {}, {}, {}, {}
    for b in range(B):
        batch_qT[b] = np.ascontiguousarray(query[:, b, :].T).astype(NPBF16)
        batch_kT[b] = np.ascontiguousarray(key[:, b, :].T).astype(NPBF16)
        batch_vT[b] = np.ascontiguousarray(value[:, b, :].T).astype(NPBF16)
        kpos = np.arange(S).reshape(16, 128)  # [kchunk, kpos]
        mbv = np.where(kpos >= lengths[b], np.float32(MASK_BIAS), np.float32(0.0))
        batch_mb[b] = np.ascontiguousarray(mbv.T).astype(np.float32)  # [128, 16]
        batch_kq[b] = (np.arange(S).reshape(4, 512) < lengths[b]).astype(np.float32)

    in_maps = []
    for c in range(N_CORES):
        g, hq = c // 4, c % 4
        f0 = hq * 256
        m = {
            "wqT": np.ascontiguousarray(WqT[:, f0:f0 + 256]),
            "wkT": np.ascontiguousarray(WkT[:, f0:f0 + 256]),
            "wvT": np.ascontiguousarray(WvT[:, f0:f0 + 256]),
            "woT": np.ascontiguousarray(WoT[f0:f0 + 256, :]),
        }
        for sl in range(2):
            b = assign[g][sl]
            m[f"qT{sl}"] = batch_qT[b]
            m[f"kT{sl}"] = batch_kT[b]
            m[f"vT{sl}"] = batch_vT[b]
            m[f"mb{sl}"] = batch_mb[b]
            m[f"kq{sl}"] = batch_kq[b]
        in_maps.append(m)

    res = run_bass_kernel_spmd(nc, in_maps, list(range(N_CORES)))

    # --- gather: sum 4 head-quad partials per batch, transpose ---
    out = np.zeros((S, B, H), dtype=np.float32)
    for g in range(2):
        for sl in range(2):
            b = assign[g][sl]
            acc = np.zeros((H, S), dtype=np.float32)
            for hq in range(4):
                c = g * 4 + hq
                acc += res.results[c][f"y{sl}"].astype(np.float32)
            out[:, b, :] = acc.T
    return out


# revision 29
# speedup vs baseline: 1.1736x; 1.0599x over previous
"""Multi-head attention (S=2048, B=4, H=1024, NH=16) on 8 Trainium2 NeuronCores.

Sharding: each core handles 2 batches x 4 heads (batch pairs balanced by
valid length; tensor-parallel over heads). Within a core everything is bf16
matmul / fp32 accumulate:
  1. q,k projected d-major (qT/kT: [dims, seq]), v seq-major ([seq, dims])
  2. scoresT[k,q] per head-pair via row-tiled matmuls (row_grp concurrency)
  3. mask+scale+exp fused on ScalarE (per-partition bias; PAD keys -> exp 0)
  4. PV col-tiled accumulates attnT; Z row-sums via all-ones stationary
     matmul (col_grp-concurrent with PV)
  5. attnT normalized by 1/Z; keepq applied before the Wo output multiply
  6. Wo projection -> yT partial [H, S]; host sums 4 partials/batch

Schedule: slot0 projections stream from HBM across 4 DMA queues; slot0
attention is ScalarE(exp)-paced; slot1's projections are emitted as
deadline-scheduled units interleaved into slot1's attention kc-loop so they
fill TensorE idle slots, with a dedicated 1-bank PSUM pool. The last q-chunk
of each slot is width-trimmed to the valid length (rounded to 32), shrinking
exp/matmul/DVE work on padded queries. Wo accumulates in the scores PSUM
pool (bufs=2) so the output chain pipelines at the kernel tail.
"""
import sys

if "/opt/trn_rl_repo" not in sys.path:
    sys.path.insert(0, "/opt/trn_rl_repo")

import math
from itertools import permutations

import ml_dtypes
import numpy as np

import concourse.bass as bass
import concourse.mybir as mybir
import concourse.tile as tile
from concourse import bacc
from concourse.bass_utils import run_bass_kernel_spmd

S, B, H, NH, DK = 2048, 4, 1024, 16, 64
N_CORES = 8
BF16 = mybir.dt.bfloat16
F32 = mybir.dt.float32
NPBF16 = ml_dtypes.bfloat16
MASK_BIAS = -30000.0

_prog_cache: dict = {}


def _build_program(NQ, NK, VQ):
    """One SPMD program. Per batch-slot s: NQ[s] 512-wide q chunks (last one
    VQ[s] wide), NK[s] 128-wide k chunks. Slot 0 is the smaller workload."""
    NSCK = [(nk * 128 + 511) // 512 for nk in NK]
    KW = [nk * 128 for nk in NK]                    # k/v valid width
    QW = [(NQ[s] - 1) * 512 + VQ[s] for s in range(2)]  # q valid width
    nc = bacc.Bacc("TRN2", target_bir_lowering=False, debug=False,
                   num_devices=N_CORES)

    d_in = {}
    for s in range(2):
        d_in[f"qT{s}"] = nc.dram_tensor(f"qT{s}", [H, S], BF16, kind="ExternalInput")
        d_in[f"kT{s}"] = nc.dram_tensor(f"kT{s}", [H, S], BF16, kind="ExternalInput")
        d_in[f"vT{s}"] = nc.dram_tensor(f"vT{s}", [H, S], BF16, kind="ExternalInput")
        d_in[f"mb{s}"] = nc.dram_tensor(f"mb{s}", [128, 16], F32, kind="ExternalInput")
        d_in[f"kq{s}"] = nc.dram_tensor(f"kq{s}", [4, 512], F32, kind="ExternalInput")
    d_in["wqT"] = nc.dram_tensor("wqT", [H, 256], BF16, kind="ExternalInput")
    d_in["wkT"] = nc.dram_tensor("wkT", [H, 256], BF16, kind="ExternalInput")
    d_in["wvT"] = nc.dram_tensor("wvT", [H, 256], BF16, kind="ExternalInput")
    d_in["woT"] = nc.dram_tensor("woT", [256, H], BF16, kind="ExternalInput")
    d_out = [nc.dram_tensor(f"y{s}", [H, S], BF16, kind="ExternalOutput")
             for s in range(2)]

    def qcw(s, sc):
        return 512 if sc < NQ[s] - 1 else VQ[s]

    def kcw(s, sc):
        return min(512, KW[s] - sc * 512)

    with tile.TileContext(nc) as tc:
        with tc.tile_pool(name="wpool", bufs=1) as wpool, \
             tc.tile_pool(name="inp", bufs=6) as inp, \
             tc.tile_pool(name="in8", bufs=1) as in8, \
             tc.tile_pool(name="persist", bufs=1) as persist, \
             tc.tile_pool(name="probs", bufs=3) as probsp, \
             tc.tile_pool(name="small", bufs=2) as small, \
             tc.tile_pool(name="att", bufs=3) as attp, \
             tc.tile_pool(name="yst", bufs=3) as ystp:

            # --- weights: consolidated DMAs on the sync queue ---
            # w*_all[p, ic*256 + j] = w*T[ic*128 + p, j]
            wq_all = wpool.tile([128, 2048], BF16, name="wq_all", tag="wq")
            wk_all = wpool.tile([128, 2048], BF16, name="wk_all", tag="wk")
            wv_all = wpool.tile([128, 2048], BF16, name="wv_all", tag="wv")
            # wo_all[p, j*1024 + c] = woT[j*128 + p, c]
            wo_all = wpool.tile([128, 2048], BF16, name="wo_all", tag="wo")
            _wseen = set()

            def wqkv(t, ic, ft):
                return t[:, ic * 256 + ft * 128: ic * 256 + (ft + 1) * 128]

            mb = [wpool.tile([128, 16], F32, name=f"mbt{s}", tag=f"mbt{s}")
                  for s in range(2)]

            def emit_wo_mb():
                for j in range(2):
                    nc.sync.dma_start(
                        out=wo_all[:, j * 1024:(j + 1) * 1024],
                        in_=d_in["woT"].ap()[j * 128:(j + 1) * 128, :])
                for s in range(2):
                    nc.sync.dma_start(out=mb[s][:], in_=d_in[f"mb{s}"].ap())
            ones = wpool.tile([128, 64], BF16, name="ones", tag="ones")
            nc.vector.memset(ones[:], 1.0)

            # --- persistent projection outputs ---
            qTp = [[persist.tile([128, NQ[s] * 512], BF16, name=f"qTp{s}_{p}",
                                 tag=f"qTp{s}_{p}")
                    for p in range(2)] for s in range(2)]
            kTp = [[persist.tile([128, NSCK[s] * 512], BF16, name=f"kTp{s}_{p}",
                                 tag=f"kTp{s}_{p}")
                    for p in range(2)] for s in range(2)]
            vp = [[persist.tile([128, 256], BF16, name=f"vp{s}_{st}", tag=f"vp{s}_{st}")
                   for st in range(NK[s])] for s in range(2)]

            # DMA queue rotation for input streams (keep Scalar clean once
            # attention starts; Sync carries the weights up front)
            s0_queues = [nc.scalar, nc.gpsimd, nc.sync]
            s1_queues = [nc.sync, nc.gpsimd]
            _qi = [0]

            def dma_rot(queues, out, in_):
                q = queues[_qi[0] % len(queues)]
                _qi[0] += 1
                q.dma_start(out=out, in_=in_)

            def emit_w(t, dname, ic):
                # interleave weight-chunk DMAs with the input stream so the
                # first matmuls aren't gated on the full weight load
                if (dname, ic) in _wseen:
                    return
                _wseen.add((dname, ic))
                dma_rot(s0_queues, t[:, ic * 256:(ic + 1) * 256],
                        d_in[dname].ap()[ic * 128:(ic + 1) * 128, :])

            def emit_proj_streamed(s, pool):
                """ic-outer projections with streamed inputs (slot 0)."""
                for kind, wts, dname, nsc, outtiles, cw in (
                        ("q", wq_all, f"qT{s}", NQ[s], qTp[s], qcw),
                        ("k", wk_all, f"kT{s}", NSCK[s], kTp[s], kcw)):
                    wname = "wqT" if kind == "q" else "wkT"
                    ps = [[pool.tile([128, 512], F32,
                                     name=f"pj{kind}{s}_{ft}_{sc}",
                                     tag=f"pj_{ft}_{sc}")
                           for sc in range(nsc)] for ft in range(2)]
                    tw = sum(cw(s, sc) for sc in range(nsc))
                    for ic in range(8):
                        emit_w(wts, wname, ic)
                        it = inp.tile([128, 2048], BF16,
                                      name=f"in{kind}{s}_{ic}", tag="inp")
                        dma_rot(s0_queues,
                                it[:, 0:tw],
                                d_in[dname].ap()[ic * 128:(ic + 1) * 128, 0:tw])
                        for ft in range(2):
                            for sc in range(nsc):
                                w = cw(s, sc)
                                nc.tensor.matmul(
                                    out=ps[ft][sc][:, 0:w],
                                    lhsT=wqkv(wts, ic, ft),
                                    rhs=it[:, sc * 512: sc * 512 + w],
                                    start=(ic == 0), stop=(ic == 7))
                    for ft in range(2):
                        for sc in range(nsc):
                            w = cw(s, sc)
                            if kind == "q":
                                nc.vector.tensor_copy(
                                    outtiles[ft][:, sc * 512: sc * 512 + w],
                                    ps[ft][sc][:, 0:w])
                            else:
                                nc.scalar.copy(
                                    outtiles[ft][:, sc * 512: sc * 512 + w],
                                    ps[ft][sc][:, 0:w])

            def emit_v_prefetch0():
                tiles = []
                for ic in range(8):
                    emit_w(wv_all, "wvT", ic)
                    it = in8.tile([128, KW[0]], BF16, name=f"v0in_{ic}",
                                  tag=f"v0in{ic}")
                    dma_rot(s0_queues, it[:],
                            d_in["vT0"].ap()[ic * 128:(ic + 1) * 128, 0:KW[0]])
                    tiles.append(it)
                return tiles

            def emit_v_resident(s, pool, tiles):
                for st0 in range(0, NK[s], 8):
                    sts = range(st0, min(st0 + 8, NK[s]))
                    psv = {st: pool.tile([128, 256], F32, name=f"pjv{s}_{st}",
                                         tag=f"pj_{(st - st0) // 4}_{(st - st0) % 4}")
                           for st in sts}
                    for ic in range(8):
                        for st in sts:
                            nc.tensor.matmul(
                                out=psv[st][:],
                                lhsT=tiles[ic][:, st * 128:(st + 1) * 128],
                                rhs=wv_all[:, ic * 256: ic * 256 + 256],
                                start=(ic == 0), stop=(ic == 7))
                    for st in sts:
                        if st % 2:
                            nc.scalar.copy(vp[s][st][:], psv[st][:])
                        else:
                            nc.vector.tensor_copy(vp[s][st][:], psv[st][:])

            def emit_prefetch(s):
                """Issue all of slot s's input DMAs into dedicated tiles."""
                tiles = {}
                for kind, dname, w in (("v", f"vT{s}", KW[s]),
                                       ("k", f"kT{s}", KW[s]),
                                       ("q", f"qT{s}", QW[s])):
                    for ic in range(8):
                        it = in8.tile([128, w], BF16, name=f"pf{kind}{s}_{ic}",
                                      tag=f"pf{kind}{ic}")
                        dma_rot(s1_queues,
                                it[:],
                                d_in[dname].ap()[ic * 128:(ic + 1) * 128, 0:w])
                        tiles[(kind, ic)] = it
                return tiles

            def proj_units(s, pool, tiles, nkit, kinds=("v", "k", "q")):
                """Deadline-tagged projection units for slot s, consumed by
                interleaving into the attention kc-loop. Deadline = global
                iteration index ((qc*2)+p)*NK + kc of first use."""
                units = []

                def q_unit(ft, sc):
                    def emit():
                        w = qcw(s, sc)
                        pj = pool.tile([128, 512], F32,
                                       name=f"rpjq{s}_{ft}_{sc}", tag="y")
                        for ic in range(8):
                            nc.tensor.matmul(
                                out=pj[:, 0:w],
                                lhsT=wqkv(wq_all, ic, ft),
                                rhs=tiles[("q", ic)][:, sc * 512: sc * 512 + w],
                                start=(ic == 0), stop=(ic == 7))
                        nc.vector.tensor_copy(
                            qTp[s][ft][:, sc * 512: sc * 512 + w], pj[:, 0:w])
                    return emit

                def k_unit(ft, sc):
                    def emit():
                        w = kcw(s, sc)
                        pj = pool.tile([128, 512], F32,
                                       name=f"rpjk{s}_{ft}_{sc}", tag="y")
                        for ic in range(8):
                            nc.tensor.matmul(
                                out=pj[:, 0:w],
                                lhsT=wqkv(wk_all, ic, ft),
                                rhs=tiles[("k", ic)][:, sc * 512: sc * 512 + w],
                                start=(ic == 0), stop=(ic == 7))
                        nc.vector.tensor_copy(
                            kTp[s][ft][:, sc * 512: sc * 512 + w], pj[:, 0:w])
                    return emit

                def v_unit(st):
                    def emit():
                        pj = pool.tile([128, 512], F32,
                                       name=f"rpjv{s}_{st}", tag="y")
                        for ic in range(8):
                            nc.tensor.matmul(
                                out=pj[:, 0:256],
                                lhsT=tiles[("v", ic)][:, st * 128:(st + 1) * 128],
                                rhs=wv_all[:, ic * 256: ic * 256 + 256],
                                start=(ic == 0), stop=(ic == 7))
                        nc.vector.tensor_copy(vp[s][st][:], pj[:, 0:256])
                    return emit

                vk = []
                if "v" in kinds:
                    for st in range(NK[s]):
                        vk.append(v_unit(st))
                if "k" in kinds:
                    for ft in range(2):
                        for sc in range(NSCK[s]):
                            vk.append(k_unit(ft, sc))
                if "q" in kinds:
                    for ft in range(2):
                        for sc in range(NQ[s]):
                            units.append((max(0, (sc * 2 + ft) * nkit - nkit // 2),
                                          q_unit(ft, sc)))
                units.sort(key=lambda u: u[0])
                return vk, units

            def emit_attention(s, psc, pat, pz, pwo, units=None, carry_wo=None,
                               lead=4):
                units = list(units) if units else []
                wo_pending = list(carry_wo) if carry_wo else []
                nkit = NK[s]

                def drain_units(cur_idx, pre_group=False):
                    # hard deadlines: emit everything overdue; then up to two
                    # early ones to smooth PE load
                    n = 0
                    while units and (units[0][0] <= cur_idx
                                     or (n < 2 and units[0][0] <= cur_idx + lead)):
                        units.pop(0)[1]()
                        n += 1
                    m = 0
                    while wo_pending and wo_pending[0][0] <= cur_idx and m < 2:
                        wo_pending.pop(0)[1](None)
                        m += 1

                def make_wo_unit(qc, ot, ab_pair, W):
                    def emit(tag):
                        if tag is None:
                            yps = pwo.tile([128, 512], F32,
                                           name=f"yp{s}_{qc}_{ot}", tag="y")
                            ypv = yps[:, 0:W]
                        else:
                            yps = psc.tile([128, 1024], F32,
                                           name=f"yp{s}_{qc}_{ot}", tag="sc")
                            ypv = yps[:, 0:W]
                        for j in range(2):
                            nc.tensor.matmul(
                                out=ypv,
                                lhsT=wo_all[:, j * 1024 + ot * 128: j * 1024 + (ot + 1) * 128],
                                rhs=ab_pair[j][:, 0:W],
                                start=(j == 0), stop=(j == 1))
                        ysb = ystp.tile([128, 512], BF16,
                                        name=f"ysb{s}_{qc}_{ot}", tag="ysb")
                        nc.vector.tensor_copy(ysb[:, 0:W], ypv)
                        (nc.gpsimd if ot % 2 else nc.sync).dma_start(
                            out=d_out[s].ap()[ot * 128:(ot + 1) * 128,
                                              qc * 512: qc * 512 + W],
                            in_=ysb[:, 0:W])
                    return emit

                for qc in range(NQ[s]):
                    W = qcw(s, qc)
                    kqr = small.tile([128, 512], F32, name=f"kqr{s}_{qc}",
                                     tag="kqr")
                    nc.gpsimd.dma_start(
                        out=kqr[:, 0:W],
                        in_=bass.AP(tensor=d_in[f"kq{s}"], offset=qc * 512,
                                    ap=[[0, 128], [1, W]]))
                    att_sb = []
                    for p in range(2):
                        base_idx = (qc * 2 + p) * nkit
                        # hard-deadline units must precede this group's
                        # first scores emission (they share the PE queue)
                        drain_units(base_idx)
                        attn = pat.tile([128, 512], F32,
                                        name=f"at{s}_{qc}_{p}", tag="at")
                        zps = pz.tile([128, 512], F32,
                                      name=f"z{s}_{qc}_{p}", tag="z")

                        # software-pipelined: scores(kc+1) emitted before
                        # PV(kc) so the in-order PE queue never stalls on exp
                        def emit_scores(kc):
                            sc_ps = psc.tile([128, 1024], F32,
                                             name=f"s{s}_{qc}_{p}_{kc}",
                                             tag="sc")
                            pr = probsp.tile([128, 1024], BF16,
                                             name=f"pr{s}_{qc}_{p}_{kc}",
                                             tag="pr")
                            for hh in range(2):
                                hsl = slice(hh * 64, hh * 64 + 64)
                                # 512-strided blocks: the two row-group-
                                # concurrent matmuls must drain to
                                # different PSUM banks
                                nc.tensor.matmul(
                                    out=sc_ps[:, hh * 512: hh * 512 + W],
                                    lhsT=kTp[s][p][hsl, kc * 128:(kc + 1) * 128],
                                    rhs=qTp[s][p][hsl, qc * 512: qc * 512 + W],
                                    start=True, stop=True)
                            if W == 512:
                                exp_in = sc_ps[:, 0:1024]
                                exp_out = pr[:, 0:1024]
                            else:
                                exp_in = sc_ps[:].rearrange(
                                    "p (b w) -> p b w", b=2)[:, :, 0:W]
                                exp_out = pr[:, 0:2 * W].rearrange(
                                    "p (b w) -> p b w", b=2)
                            nc.scalar.activation(
                                out=exp_out, in_=exp_in,
                                func=mybir.ActivationFunctionType.Exp,
                                bias=mb[s][:, kc:kc + 1],
                                scale=1.0 / math.sqrt(DK))
                            return pr

                        pr_next = emit_scores(0)
                        for kc in range(nkit):
                            drain_units(base_idx + kc)
                            first, last = kc == 0, kc == nkit - 1
                            pr = pr_next
                            if not last:
                                pr_next = emit_scores(kc + 1)
                            for hh in range(2):
                                hsl = slice(hh * 64, hh * 64 + 64)
                                nc.tensor.matmul(
                                    out=attn[hsl, 0:W],
                                    lhsT=vp[s][kc][:, p * 128 + hh * 64:p * 128 + (hh + 1) * 64],
                                    rhs=pr[:, hh * W:(hh + 1) * W],
                                    start=first, stop=last)
                                nc.tensor.matmul(
                                    out=zps[hsl, 0:W],
                                    lhsT=ones[:, :], rhs=pr[:, hh * W:(hh + 1) * W],
                                    start=first, stop=last)
                        # normalize: attn * keepq / Z
                        rz = small.tile([128, 512], F32,
                                        name=f"rz{s}_{qc}_{p}", tag="rz")
                        nc.vector.reciprocal_approx_fast(out=rz[:, 0:W],
                                                         in_=zps[:, 0:W])
                        nc.vector.tensor_mul(rz[:, 0:W], rz[:, 0:W], kqr[:, 0:W])
                        ab = attp.tile([128, 512], BF16,
                                       name=f"ab{s}_{qc}_{p}", tag=f"ab{p}")
                        nc.vector.tensor_mul(ab[:, 0:W], attn[:, 0:W], rz[:, 0:W])
                        att_sb.append(ab)
                    # Wo is deferred: one unit per iteration of the NEXT
                    # group, so the output chain never gates the next
                    # group's scores/exp pipeline
                    for ot in range(8):
                        wo_pending.append(((qc + 1) * 2 * nkit + 2 * ot,
                                           make_wo_unit(qc, ot, att_sb, W)))
                # flush stragglers; alternate between the 'y' bank and the
                # now-idle scores slots so the tail chain pipelines
                while units:
                    units.pop(0)[1]()
                return wo_pending

            # slot 0 projections use the full PSUM (released afterwards)
            with tc.tile_pool(name="pproj", bufs=1, space="PSUM") as pproj:
                emit_proj_streamed(0, pproj)
                v0tiles = emit_v_prefetch0()
                emit_v_resident(0, pproj, v0tiles)
            emit_wo_mb()
            tiles1 = emit_prefetch(1)
            with tc.tile_pool(name="psc", bufs=2, space="PSUM") as psc, \
                 tc.tile_pool(name="pat", bufs=2, space="PSUM") as pat, \
                 tc.tile_pool(name="pz", bufs=1, space="PSUM") as pz, \
                 tc.tile_pool(name="pwo", bufs=1, space="PSUM") as pwo:
                vk1, units1 = proj_units(1, pwo, tiles1, NK[1])
                n0 = NQ[0] * 2 * NK[0]
                sp = max(1, (n0 - 6) // max(1, len(vk1)))
                units0 = [(2 + sp * i, fn) for i, fn in enumerate(vk1)]
                left0 = emit_attention(0, psc, pat, pz, pwo, units=units0)
                carry = [(i, fn) for i, (_, fn) in enumerate(left0)]
                left1 = emit_attention(1, psc, pat, pz, pwo, units=units1,
                                       carry_wo=carry)
                for i, (_, fn) in enumerate(left1):
                    fn("sc" if i % 2 == 0 else None)
    nc.compile()
    return nc


def _get_program(NQ, NK, VQ):
    key = (tuple(NQ), tuple(NK), tuple(VQ))
    if key not in _prog_cache:
        _prog_cache[key] = _build_program(list(NQ), list(NK), list(VQ))
    return _prog_cache[key]


def kernel(value, key, query, padding_mask, Wq, Wk, Wv, Wo):
    value = np.asarray(value)
    key = np.asarray(key)
    query = np.asarray(query)
    padding_mask = np.asarray(padding_mask)
    Wq, Wk, Wv, Wo = (np.asarray(a) for a in (Wq, Wk, Wv, Wo))

    lengths = (~padding_mask).sum(axis=0).astype(int)  # (B,)

    # --- batch pairing: assign batches to (group, slot) minimizing baked work ---
    def slot_counts(assign):
        nq = [max((int(lengths[assign[g][sl]]) + 511) // 512 for g in range(2))
              for sl in range(2)]
        nk = [max((int(lengths[assign[g][sl]]) + 127) // 128 for g in range(2))
              for sl in range(2)]
        return nq, nk

    best = None
    for perm in permutations(range(B)):
        a = ((perm[0], perm[1]), (perm[2], perm[3]))
        nq, nk = slot_counts(a)
        c = nq[0] * nk[0] + nq[1] * nk[1]
        if best is None or c < best[0]:
            best = (c, a)
    assign = best[1]
    nq, nk = slot_counts(assign)
    # slot 0 should be the smaller workload (its projections can't overlap)
    if nq[0] * nk[0] > nq[1] * nk[1]:
        assign = tuple((g[1], g[0]) for g in assign)
        nq, nk = slot_counts(assign)
    NQ, NK = nq, nk
    # trimmed width of the last q chunk per slot (multiple of 32)
    VQ = []
    for sl in range(2):
        maxlen = max(int(lengths[assign[g][sl]]) for g in range(2))
        v = maxlen - (NQ[sl] - 1) * 512
        VQ.append(min(512, (v + 31) // 32 * 32))

    nc = _get_program(NQ, NK, VQ)

    # --- per-core inputs ---
    WqT = np.ascontiguousarray(Wq.T).astype(NPBF16)
    WkT = np.ascontiguousarray(Wk.T).astype(NPBF16)
    WvT = np.ascontiguousarray(Wv.T).astype(NPBF16)
    WoT = np.ascontiguousarray(Wo.T).astype(NPBF16)

    batch_qT, batch_kT, batch_vT, batch_mb, batch_kq = {}, {}, {}, {}, {}
    for b in range(B):
        batch_qT[b] = np.ascontiguousarray(query[:, b, :].T).astype(NPBF16)
        batch_kT[b] = np.ascontiguousarray(key[:, b, :].T).astype(NPBF16)
        batch_vT[b] = np.ascontiguousarray(value[:, b, :].T).astype(NPBF16)
        kpos = np.arange(S).reshape(16, 128)  # [kchunk, kpos]
        mbv = np.where(kpos >= lengths[b], np.float32(MASK_BIAS), np.float32(0.0))
        batch_mb[b] = np.ascontiguousarray(mbv.T).astype(np.float32)  # [128, 16]
        batch_kq[b] = (np.arange(S).reshape(4, 512) < lengths[b]).astype(np.float32)

    in_maps = []
    for c in range(N_CORES):
        g, hq = c // 4, c % 4
        f0 = hq * 256
        m = {
            "wqT": np.ascontiguousarray(WqT[:, f0:f0 + 256]),
            "wkT": np.ascontiguousarray(WkT[:, f0:f0 + 256]),
            "wvT": np.ascontiguousarray(WvT[:, f0:f0 + 256]),
            "woT": np.ascontiguousarray(WoT[f0:f0 + 256, :]),
        }
        for sl in range(2):
            b = assign[g][sl]
            m[f"qT{sl}"] = batch_qT[b]
            m[f"kT{sl}"] = batch_kT[b]
            m[f"vT{sl}"] = batch_vT[b]
            m[f"mb{sl}"] = batch_mb[b]
            m[f"kq{sl}"] = batch_kq[b]
        in_maps.append(m)

    res = run_bass_kernel_spmd(nc, in_maps, list(range(N_CORES)))

    # --- gather: sum 4 head-quad partials per batch, transpose ---
    out = np.zeros((S, B, H), dtype=np.float32)
    for g in range(2):
        for sl in range(2):
            b = assign[g][sl]
            acc = np.zeros((H, S), dtype=np.float32)
            for hq in range(4):
                c = g * 4 + hq
                acc += res.results[c][f"y{sl}"].astype(np.float32)
            out[:, b, :] = acc.T
    return out


# revision 32
# speedup vs baseline: 1.1745x; 1.0008x over previous
"""Multi-head attention (S=2048, B=4, H=1024, NH=16) on 8 Trainium2 NeuronCores.

Sharding: each core handles 2 batches x 4 heads (batch pairs balanced by
valid length; tensor-parallel over heads). Within a core everything is bf16
matmul / fp32 accumulate:
  1. q,k projected d-major (qT/kT: [dims, seq]), v seq-major ([seq, dims])
  2. scoresT[k,q] per head-pair via row-tiled matmuls (row_grp concurrency)
  3. mask+scale+exp fused on ScalarE (per-partition bias; PAD keys -> exp 0)
  4. PV col-tiled accumulates attnT; Z row-sums via all-ones stationary
     matmul (col_grp-concurrent with PV)
  5. attnT normalized by 1/Z; keepq applied before the Wo output multiply
  6. Wo projection -> yT partial [H, S]; host sums 4 partials/batch

Schedule: weight-chunk DMAs interleave with slot0's q/k input streams over
the three DMA-capable queues (sync/scalar/gpsimd), so the first projection
matmul issues ~2 transfers in; slot0 v inputs prefetch into dedicated tiles
(issue never blocks on tile recycling). Attention is emitted as a
software-pipelined kc-loop (scores(kc+1) before PV(kc)); the two scores
matmuls use 512-strided PSUM blocks so their row-group-concurrent drains hit
different banks (same-bank PE+PE drain is a fatal collision when trimmed),
with a strided-AP exp over both blocks. PV+Z pairs run col-group-concurrent.
Wo is deferred into per-ot units drained one-per-iteration of the NEXT
query-group (1-bank PSUM tag shared with projection units), so the output
chain never gates the scores/exp pipeline; the final group's units flush
alternating into the idle scores banks. Slot1's q/k/v projections become
deadline-scheduled units: v/k spread through attention0, q half-a-group
early in attention1. The last q-chunk of each slot is width-trimmed to the
valid length (multiple of 32), shrinking exp/matmul/DVE/DMA work on padded
queries.
"""
import sys

if "/opt/trn_rl_repo" not in sys.path:
    sys.path.insert(0, "/opt/trn_rl_repo")

import math
from itertools import permutations

import ml_dtypes
import numpy as np

import concourse.bass as bass
import concourse.mybir as mybir
import concourse.tile as tile
from concourse import bacc
from concourse.bass_utils import run_bass_kernel_spmd

S, B, H, NH, DK = 2048, 4, 1024, 16, 64
N_CORES = 8
BF16 = mybir.dt.bfloat16
F32 = mybir.dt.float32
NPBF16 = ml_dtypes.bfloat16
MASK_BIAS = -30000.0

_prog_cache: dict = {}


def _build_program(NQ, NK, VQ):
    """One SPMD program. Per batch-slot s: NQ[s] 512-wide q chunks (last one
    VQ[s] wide), NK[s] 128-wide k chunks. Slot 0 is the smaller workload."""
    NSCK = [(nk * 128 + 511) // 512 for nk in NK]
    KW = [nk * 128 for nk in NK]                    # k/v valid width
    QW = [(NQ[s] - 1) * 512 + VQ[s] for s in range(2)]  # q valid width
    nc = bacc.Bacc("TRN2", target_bir_lowering=False, debug=False,
                   num_devices=N_CORES)

    d_in = {}
    for s in range(2):
        d_in[f"qT{s}"] = nc.dram_tensor(f"qT{s}", [H, S], BF16, kind="ExternalInput")
        d_in[f"kT{s}"] = nc.dram_tensor(f"kT{s}", [H, S], BF16, kind="ExternalInput")
        d_in[f"vT{s}"] = nc.dram_tensor(f"vT{s}", [H, S], BF16, kind="ExternalInput")
        d_in[f"mb{s}"] = nc.dram_tensor(f"mb{s}", [128, 16], F32, kind="ExternalInput")
        d_in[f"kq{s}"] = nc.dram_tensor(f"kq{s}", [4, 512], F32, kind="ExternalInput")
    d_in["wqT"] = nc.dram_tensor("wqT", [H, 256], BF16, kind="ExternalInput")
    d_in["wkT"] = nc.dram_tensor("wkT", [H, 256], BF16, kind="ExternalInput")
    d_in["wvT"] = nc.dram_tensor("wvT", [H, 256], BF16, kind="ExternalInput")
    d_in["woT"] = nc.dram_tensor("woT", [256, H], BF16, kind="ExternalInput")
    d_out = [nc.dram_tensor(f"y{s}", [H, S], BF16, kind="ExternalOutput")
             for s in range(2)]

    def qcw(s, sc):
        return 512 if sc < NQ[s] - 1 else VQ[s]

    def kcw(s, sc):
        return min(512, KW[s] - sc * 512)

    with tile.TileContext(nc) as tc:
        with tc.tile_pool(name="wpool", bufs=1) as wpool, \
             tc.tile_pool(name="inp", bufs=6) as inp, \
             tc.tile_pool(name="in8", bufs=1) as in8, \
             tc.tile_pool(name="persist", bufs=1) as persist, \
             tc.tile_pool(name="probs", bufs=3) as probsp, \
             tc.tile_pool(name="small", bufs=2) as small, \
             tc.tile_pool(name="att", bufs=3) as attp, \
             tc.tile_pool(name="yst", bufs=3) as ystp:

            # --- weights: consolidated DMAs on the sync queue ---
            # w*_all[p, ic*256 + j] = w*T[ic*128 + p, j]
            wq_all = wpool.tile([128, 2048], BF16, name="wq_all", tag="wq")
            wk_all = wpool.tile([128, 2048], BF16, name="wk_all", tag="wk")
            wv_all = wpool.tile([128, 2048], BF16, name="wv_all", tag="wv")
            # wo_all[p, j*1024 + c] = woT[j*128 + p, c]
            wo_all = wpool.tile([128, 2048], BF16, name="wo_all", tag="wo")
            _wseen = set()

            def wqkv(t, ic, ft):
                return t[:, ic * 256 + ft * 128: ic * 256 + (ft + 1) * 128]

            mb = [wpool.tile([128, 16], F32, name=f"mbt{s}", tag=f"mbt{s}")
                  for s in range(2)]

            def emit_wo_mb():
                for j in range(2):
                    nc.sync.dma_start(
                        out=wo_all[:, j * 1024:(j + 1) * 1024],
                        in_=d_in["woT"].ap()[j * 128:(j + 1) * 128, :])
                for s in range(2):
                    nc.sync.dma_start(out=mb[s][:], in_=d_in[f"mb{s}"].ap())
            ones = wpool.tile([128, 64], BF16, name="ones", tag="ones")
            nc.vector.memset(ones[:], 1.0)

            # --- persistent projection outputs ---
            qTp = [[persist.tile([128, NQ[s] * 512], BF16, name=f"qTp{s}_{p}",
                                 tag=f"qTp{s}_{p}")
                    for p in range(2)] for s in range(2)]
            kTp = [[persist.tile([128, NSCK[s] * 512], BF16, name=f"kTp{s}_{p}",
                                 tag=f"kTp{s}_{p}")
                    for p in range(2)] for s in range(2)]
            vp = [[persist.tile([128, 256], BF16, name=f"vp{s}_{st}", tag=f"vp{s}_{st}")
                   for st in range(NK[s])] for s in range(2)]

            # DMA queue rotation for input streams (keep Scalar clean once
            # attention starts; Sync carries the weights up front)
            s0_queues = [nc.scalar, nc.gpsimd, nc.sync]
            s1_queues = [nc.sync, nc.gpsimd]
            _qi = [0]

            def dma_rot(queues, out, in_):
                q = queues[_qi[0] % len(queues)]
                _qi[0] += 1
                q.dma_start(out=out, in_=in_)

            def emit_w(t, dname, ic):
                # interleave weight-chunk DMAs with the input stream so the
                # first matmuls aren't gated on the full weight load
                if (dname, ic) in _wseen:
                    return
                _wseen.add((dname, ic))
                dma_rot(s0_queues, t[:, ic * 256:(ic + 1) * 256],
                        d_in[dname].ap()[ic * 128:(ic + 1) * 128, :])

            def emit_proj_streamed(s, pool):
                """ic-outer projections with streamed inputs (slot 0)."""
                for kind, wts, dname, nsc, outtiles, cw in (
                        ("q", wq_all, f"qT{s}", NQ[s], qTp[s], qcw),
                        ("k", wk_all, f"kT{s}", NSCK[s], kTp[s], kcw)):
                    wname = "wqT" if kind == "q" else "wkT"
                    ps = [[pool.tile([128, 512], F32,
                                     name=f"pj{kind}{s}_{ft}_{sc}",
                                     tag=f"pj_{ft}_{sc}")
                           for sc in range(nsc)] for ft in range(2)]
                    tw = sum(cw(s, sc) for sc in range(nsc))
                    for ic in range(8):
                        emit_w(wts, wname, ic)
                        it = inp.tile([128, 2048], BF16,
                                      name=f"in{kind}{s}_{ic}", tag="inp")
                        dma_rot(s0_queues,
                                it[:, 0:tw],
                                d_in[dname].ap()[ic * 128:(ic + 1) * 128, 0:tw])
                        for ft in range(2):
                            for sc in range(nsc):
                                w = cw(s, sc)
                                nc.tensor.matmul(
                                    out=ps[ft][sc][:, 0:w],
                                    lhsT=wqkv(wts, ic, ft),
                                    rhs=it[:, sc * 512: sc * 512 + w],
                                    start=(ic == 0), stop=(ic == 7))
                    for ft in range(2):
                        for sc in range(nsc):
                            w = cw(s, sc)
                            if kind == "q":
                                nc.vector.tensor_copy(
                                    outtiles[ft][:, sc * 512: sc * 512 + w],
                                    ps[ft][sc][:, 0:w])
                            else:
                                nc.scalar.copy(
                                    outtiles[ft][:, sc * 512: sc * 512 + w],
                                    ps[ft][sc][:, 0:w])

            def emit_v_prefetch0():
                tiles = []
                for ic in range(8):
                    emit_w(wv_all, "wvT", ic)
                    it = in8.tile([128, KW[0]], BF16, name=f"v0in_{ic}",
                                  tag=f"v0in{ic}")
                    dma_rot(s0_queues, it[:],
                            d_in["vT0"].ap()[ic * 128:(ic + 1) * 128, 0:KW[0]])
                    tiles.append(it)
                return tiles

            def emit_v_resident(s, pool, tiles):
                for st0 in range(0, NK[s], 8):
                    sts = range(st0, min(st0 + 8, NK[s]))
                    psv = {st: pool.tile([128, 256], F32, name=f"pjv{s}_{st}",
                                         tag=f"pj_{(st - st0) // 4}_{(st - st0) % 4}")
                           for st in sts}
                    for ic in range(8):
                        for st in sts:
                            nc.tensor.matmul(
                                out=psv[st][:],
                                lhsT=tiles[ic][:, st * 128:(st + 1) * 128],
                                rhs=wv_all[:, ic * 256: ic * 256 + 256],
                                start=(ic == 0), stop=(ic == 7))
                    for st in sts:
                        if st % 2:
                            nc.scalar.copy(vp[s][st][:], psv[st][:])
                        else:
                            nc.vector.tensor_copy(vp[s][st][:], psv[st][:])

            def emit_prefetch(s):
                """Issue all of slot s's input DMAs into dedicated tiles."""
                tiles = {}
                for kind, dname, w in (("v", f"vT{s}", KW[s]),
                                       ("k", f"kT{s}", KW[s]),
                                       ("q", f"qT{s}", QW[s])):
                    for ic in range(8):
                        it = in8.tile([128, w], BF16, name=f"pf{kind}{s}_{ic}",
                                      tag=f"pf{kind}{ic}")
                        dma_rot(s1_queues,
                                it[:],
                                d_in[dname].ap()[ic * 128:(ic + 1) * 128, 0:w])
                        tiles[(kind, ic)] = it
                return tiles

            def proj_units(s, pool, tiles, nkit, kinds=("v", "k", "q")):
                """Deadline-tagged projection units for slot s, consumed by
                interleaving into the attention kc-loop. Deadline = global
                iteration index ((qc*2)+p)*NK + kc of first use."""
                units = []

                def q_unit(ft, sc):
                    def emit():
                        w = qcw(s, sc)
                        pj = pool.tile([128, 512], F32,
                                       name=f"rpjq{s}_{ft}_{sc}", tag="y")
                        for ic in range(8):
                            nc.tensor.matmul(
                                out=pj[:, 0:w],
                                lhsT=wqkv(wq_all, ic, ft),
                                rhs=tiles[("q", ic)][:, sc * 512: sc * 512 + w],
                                start=(ic == 0), stop=(ic == 7))
                        nc.vector.tensor_copy(
                            qTp[s][ft][:, sc * 512: sc * 512 + w], pj[:, 0:w])
                    return emit

                def k_unit(ft, sc):
                    def emit():
                        w = kcw(s, sc)
                        pj = pool.tile([128, 512], F32,
                                       name=f"rpjk{s}_{ft}_{sc}", tag="y")
                        for ic in range(8):
                            nc.tensor.matmul(
                                out=pj[:, 0:w],
                                lhsT=wqkv(wk_all, ic, ft),
                                rhs=tiles[("k", ic)][:, sc * 512: sc * 512 + w],
                                start=(ic == 0), stop=(ic == 7))
                        nc.vector.tensor_copy(
                            kTp[s][ft][:, sc * 512: sc * 512 + w], pj[:, 0:w])
                    return emit

                def v_unit(st):
                    def emit():
                        pj = pool.tile([128, 512], F32,
                                       name=f"rpjv{s}_{st}", tag="y")
                        for ic in range(8):
                            nc.tensor.matmul(
                                out=pj[:, 0:256],
                                lhsT=tiles[("v", ic)][:, st * 128:(st + 1) * 128],
                                rhs=wv_all[:, ic * 256: ic * 256 + 256],
                                start=(ic == 0), stop=(ic == 7))
                        nc.vector.tensor_copy(vp[s][st][:], pj[:, 0:256])
                    return emit

                vk = []
                if "v" in kinds:
                    for st in range(NK[s]):
                        vk.append(v_unit(st))
                if "k" in kinds:
                    for ft in range(2):
                        for sc in range(NSCK[s]):
                            vk.append(k_unit(ft, sc))
                if "q" in kinds:
                    for ft in range(2):
                        for sc in range(NQ[s]):
                            units.append((max(0, (sc * 2 + ft) * nkit - nkit // 2),
                                          q_unit(ft, sc)))
                units.sort(key=lambda u: u[0])
                return vk, units

            def emit_attention(s, psc, pat, pz, pwo, units=None, carry_wo=None,
                               lead=4):
                units = list(units) if units else []
                wo_pending = list(carry_wo) if carry_wo else []
                nkit = NK[s]

                def drain_units(cur_idx, pre_group=False):
                    # hard deadlines: emit everything overdue; then up to two
                    # early ones to smooth PE load
                    n = 0
                    while units and (units[0][0] <= cur_idx
                                     or (n < 2 and units[0][0] <= cur_idx + lead)):
                        units.pop(0)[1]()
                        n += 1
                    m = 0
                    while wo_pending and wo_pending[0][0] <= cur_idx and m < 2:
                        wo_pending.pop(0)[1](None)
                        m += 1

                def make_wo_unit(qc, ot, ab_pair, W):
                    def emit(tag):
                        if tag is None:
                            yps = pwo.tile([128, 512], F32,
                                           name=f"yp{s}_{qc}_{ot}", tag="y")
                            ypv = yps[:, 0:W]
                        else:
                            yps = psc.tile([128, 1024], F32,
                                           name=f"yp{s}_{qc}_{ot}", tag="sc")
                            ypv = yps[:, 0:W]
                        for j in range(2):
                            nc.tensor.matmul(
                                out=ypv,
                                lhsT=wo_all[:, j * 1024 + ot * 128: j * 1024 + (ot + 1) * 128],
                                rhs=ab_pair[j][:, 0:W],
                                start=(j == 0), stop=(j == 1))
                        ysb = ystp.tile([128, 512], BF16,
                                        name=f"ysb{s}_{qc}_{ot}", tag="ysb")
                        nc.vector.tensor_copy(ysb[:, 0:W], ypv)
                        (nc.gpsimd if ot % 2 else nc.sync).dma_start(
                            out=d_out[s].ap()[ot * 128:(ot + 1) * 128,
                                              qc * 512: qc * 512 + W],
                            in_=ysb[:, 0:W])
                    return emit

                for qc in range(NQ[s]):
                    W = qcw(s, qc)
                    kqr = small.tile([128, 512], F32, name=f"kqr{s}_{qc}",
                                     tag="kqr")
                    nc.gpsimd.dma_start(
                        out=kqr[:, 0:W],
                        in_=bass.AP(tensor=d_in[f"kq{s}"], offset=qc * 512,
                                    ap=[[0, 128], [1, W]]))
                    att_sb = []
                    for p in range(2):
                        base_idx = (qc * 2 + p) * nkit
                        # hard-deadline units must precede this group's
                        # first scores emission (they share the PE queue)
                        drain_units(base_idx)
                        attn = pat.tile([128, 512], F32,
                                        name=f"at{s}_{qc}_{p}", tag="at")
                        zps = pz.tile([128, 512], F32,
                                      name=f"z{s}_{qc}_{p}", tag="z")

                        # software-pipelined: scores(kc+1) emitted before
                        # PV(kc) so the in-order PE queue never stalls on exp
                        def emit_scores(kc):
                            sc_ps = psc.tile([128, 1024], F32,
                                             name=f"s{s}_{qc}_{p}_{kc}",
                                             tag="sc")
                            pr = probsp.tile([128, 1024], BF16,
                                             name=f"pr{s}_{qc}_{p}_{kc}",
                                             tag="pr")
                            for hh in range(2):
                                hsl = slice(hh * 64, hh * 64 + 64)
                                # 512-strided blocks: the two row-group-
                                # concurrent matmuls must drain to
                                # different PSUM banks
                                nc.tensor.matmul(
                                    out=sc_ps[:, hh * 512: hh * 512 + W],
                                    lhsT=kTp[s][p][hsl, kc * 128:(kc + 1) * 128],
                                    rhs=qTp[s][p][hsl, qc * 512: qc * 512 + W],
                                    start=True, stop=True)
                            if W == 512:
                                exp_in = sc_ps[:, 0:1024]
                                exp_out = pr[:, 0:1024]
                            else:
                                exp_in = sc_ps[:].rearrange(
                                    "p (b w) -> p b w", b=2)[:, :, 0:W]
                                exp_out = pr[:, 0:2 * W].rearrange(
                                    "p (b w) -> p b w", b=2)
                            nc.scalar.activation(
                                out=exp_out, in_=exp_in,
                                func=mybir.ActivationFunctionType.Exp,
                                bias=mb[s][:, kc:kc + 1],
                                scale=1.0 / math.sqrt(DK))
                            return pr

                        pr_next = emit_scores(0)
                        for kc in range(nkit):
                            drain_units(base_idx + kc)
                            first, last = kc == 0, kc == nkit - 1
                            pr = pr_next
                            if not last:
                                pr_next = emit_scores(kc + 1)
                            for hh in range(2):
                                hsl = slice(hh * 64, hh * 64 + 64)
                                nc.tensor.matmul(
                                    out=attn[hsl, 0:W],
                                    lhsT=vp[s][kc][:, p * 128 + hh * 64:p * 128 + (hh + 1) * 64],
                                    rhs=pr[:, hh * W:(hh + 1) * W],
                                    start=first, stop=last)
                                nc.tensor.matmul(
                                    out=zps[hsl, 0:W],
                                    lhsT=ones[:, :], rhs=pr[:, hh * W:(hh + 1) * W],
                                    start=first, stop=last)
                        # normalize: attn * keepq / Z
                        rz = small.tile([128, 512], F32,
                                        name=f"rz{s}_{qc}_{p}", tag="rz")
                        nc.vector.reciprocal_approx_fast(out=rz[:, 0:W],
                                                         in_=zps[:, 0:W])
                        nc.vector.tensor_mul(rz[:, 0:W], rz[:, 0:W], kqr[:, 0:W])
                        ab = attp.tile([128, 512], BF16,
                                       name=f"ab{s}_{qc}_{p}", tag=f"ab{p}")
                        nc.vector.tensor_mul(ab[:, 0:W], attn[:, 0:W], rz[:, 0:W])
                        att_sb.append(ab)
                    # Wo is deferred: one unit per iteration of the NEXT
                    # group, so the output chain never gates the next
                    # group's scores/exp pipeline
                    for ot in range(8):
                        wo_pending.append(((qc + 1) * 2 * nkit + 2 * ot,
                                           make_wo_unit(qc, ot, att_sb, W)))
                # flush stragglers; alternate between the 'y' bank and the
                # now-idle scores slots so the tail chain pipelines
                while units:
                    units.pop(0)[1]()
                return wo_pending

            # slot 0 projections use the full PSUM (released afterwards)
            with tc.tile_pool(name="pproj", bufs=1, space="PSUM") as pproj:
                emit_proj_streamed(0, pproj)
                v0tiles = emit_v_prefetch0()
                emit_v_resident(0, pproj, v0tiles)
            emit_wo_mb()
            tiles1 = emit_prefetch(1)
            with tc.tile_pool(name="psc", bufs=2, space="PSUM") as psc, \
                 tc.tile_pool(name="pat", bufs=2, space="PSUM") as pat, \
                 tc.tile_pool(name="pz", bufs=1, space="PSUM") as pz, \
                 tc.tile_pool(name="pwo", bufs=1, space="PSUM") as pwo:
                vk1, units1 = proj_units(1, pwo, tiles1, NK[1])
                n0 = NQ[0] * 2 * NK[0]
                sp = max(1, (n0 - 6) // max(1, len(vk1)))
                units0 = [(2 + sp * i, fn) for i, fn in enumerate(vk1)]
                left0 = emit_attention(0, psc, pat, pz, pwo, units=units0)
                carry = [(i, fn) for i, (_, fn) in enumerate(left0)]
                left1 = emit_attention(1, psc, pat, pz, pwo, units=units1,
                                       carry_wo=carry)
                for i, (_, fn) in enumerate(left1):
                    fn("sc" if i % 2 == 0 else None)
    nc.compile()
    return nc


def _get_program(NQ, NK, VQ):
    key = (tuple(NQ), tuple(NK), tuple(VQ))
    if key not in _prog_cache:
        _prog_cache[key] = _build_program(list(NQ), list(NK), list(VQ))
    return _prog_cache[key]


def kernel(value, key, query, padding_mask, Wq, Wk, Wv, Wo):
    value = np.asarray(value)
    key = np.asarray(key)
    query = np.asarray(query)
    padding_mask = np.asarray(padding_mask)
    Wq, Wk, Wv, Wo = (np.asarray(a) for a in (Wq, Wk, Wv, Wo))

    lengths = (~padding_mask).sum(axis=0).astype(int)  # (B,)

    # --- batch pairing: assign batches to (group, slot) minimizing baked work ---
    def slot_counts(assign):
        nq = [max((int(lengths[assign[g][sl]]) + 511) // 512 for g in range(2))
              for sl in range(2)]
        nk = [max((int(lengths[assign[g][sl]]) + 127) // 128 for g in range(2))
              for sl in range(2)]
        return nq, nk

    best = None
    for perm in permutations(range(B)):
        a = ((perm[0], perm[1]), (perm[2], perm[3]))
        nq, nk = slot_counts(a)
        c = nq[0] * nk[0] + nq[1] * nk[1]
        if best is None or c < best[0]:
            best = (c, a)
    assign = best[1]
    nq, nk = slot_counts(assign)
    # slot 0 should be the smaller workload (its projections can't overlap)
    if nq[0] * nk[0] > nq[1] * nk[1]:
        assign = tuple((g[1], g[0]) for g in assign)
        nq, nk = slot_counts(assign)
    NQ, NK = nq, nk
    # trimmed width of the last q chunk per slot (multiple of 32)
    VQ = []
    for sl in range(2):
        maxlen = max(int(lengths[assign[g][sl]]) for g in range(2))
        v = maxlen - (NQ[sl] - 1) * 512
        VQ.append(min(512, (v + 31) // 32 * 32))

    nc = _get_program(NQ, NK, VQ)

    # --- per-core inputs ---
    WqT = np.ascontiguousarray(Wq.T).astype(NPBF16)
    WkT = np.ascontiguousarray(Wk.T).astype(NPBF16)
    WvT = np.ascontiguousarray(Wv.T).astype(NPBF16)
    WoT = np.ascontiguousarray(Wo.T).astype(NPBF16)

    batch_qT, batch_kT, batch_vT, batch_mb, batch_kq = {}, {}, {}, {}, {}
    for b in range(B):
        batch_qT[b] = np.ascontiguousarray(query[:, b, :].T).astype(NPBF16)
        batch_kT[b] = np.ascontiguousarray(key[:, b, :].T).astype(NPBF16)
        batch_vT[b] = np.ascontiguousarray(value[:, b, :].T).astype(NPBF16)
        kpos = np.arange(S).reshape(16, 128)  # [kchunk, kpos]
        mbv = np.where(kpos >= lengths[b], np.float32(MASK_BIAS), np.float32(0.0))
        batch_mb[b] = np.ascontiguousarray(mbv.T).astype(np.float32)  # [128, 16]
        batch_kq[b] = (np.arange(S).reshape(4, 512) < lengths[b]).astype(np.float32)

    in_maps = []
    for c in range(N_CORES):
        g, hq = c // 4, c % 4
        f0 = hq * 256
        m = {
            "wqT": np.ascontiguousarray(WqT[:, f0:f0 + 256]),
            "wkT": np.ascontiguousarray(WkT[:, f0:f0 + 256]),
            "wvT": np.ascontiguousarray(WvT[:, f0:f0 + 256]),
            "woT": np.ascontiguousarray(WoT[f0:f0 + 256, :]),
        }
        for sl in range(2):
            b = assign[g][sl]
            m[f"qT{sl}"] = batch_qT[b]
            m[f"kT{sl}"] = batch_kT[b]
            m[f"vT{sl}"] = batch_vT[b]
            m[f"mb{sl}"] = batch_mb[b]
            m[f"kq{sl}"] = batch_kq[b]
        in_maps.append(m)

    res = run_bass_kernel_spmd(nc, in_maps, list(range(N_CORES)))

    # --- gather: sum 4 head-quad partials per batch, transpose ---
    out = np.zeros((S, B, H), dtype=np.float32)
    for g in range(2):
        for sl in range(2):
            b = assign[g][sl]
            acc = np.zeros((H, S), dtype=np.float32)
            for hq in range(4):
                c = g * 4 + hq
                acc += res.results[c][f"y{sl}"].astype(np.float32)
            out[:, b, :] = acc.T
    return out


# revision 34
# speedup vs baseline: 1.1915x; 1.0145x over previous
"""Multi-head attention (S=2048, B=4, H=1024, NH=16) on 8 Trainium2 NeuronCores.

Sharding: each core handles 2 batches x 4 heads (batch pairs balanced by
valid length; tensor-parallel over heads). Within a core everything is bf16
matmul / fp32 accumulate:
  1. q,k projected d-major (qT/kT: [dims, seq]), v seq-major ([seq, dims])
  2. scoresT[k,q] per head-pair via row-tiled matmuls (row_grp concurrency)
  3. mask+scale+exp fused on ScalarE (per-partition bias; PAD keys -> exp 0)
  4. PV col-tiled accumulates attnT; Z row-sums via all-ones stationary
     matmul (col_grp-concurrent with PV)
  5. attnT normalized by 1/Z; keepq applied before the Wo output multiply
  6. Wo projection -> yT partial [H, S]; host sums 4 partials/batch

Schedule: weight-chunk DMAs interleave with slot0's q/k input streams over
the three DMA-capable queues (sync/scalar/gpsimd), so the first projection
matmul issues ~2 transfers in; slot0 v inputs prefetch into dedicated tiles
(issue never blocks on tile recycling). Attention is emitted as a
software-pipelined kc-loop (scores(kc+1) before PV(kc)); the two scores
matmuls use 512-strided PSUM blocks so their row-group-concurrent drains hit
different banks (same-bank PE+PE drain is a fatal collision when trimmed),
with a strided-AP exp over both blocks. PV+Z pairs run col-group-concurrent.
Wo is deferred into per-ot units drained one-per-iteration of the NEXT
query-group (1-bank PSUM tag shared with projection units), so the output
chain never gates the scores/exp pipeline; the final group's units flush
alternating into the idle scores banks. Slot1's q/k/v projections become
deadline-scheduled units: v/k spread through attention0, q half-a-group
early in attention1. The last q-chunk of each slot is width-trimmed to the
valid length (multiple of 32), shrinking exp/matmul/DVE/DMA work on padded
queries.
"""
import sys

if "/opt/trn_rl_repo" not in sys.path:
    sys.path.insert(0, "/opt/trn_rl_repo")

import math
from itertools import permutations

import ml_dtypes
import numpy as np

import concourse.bass as bass
import concourse.mybir as mybir
import concourse.tile as tile
from concourse import bacc
from concourse.bass_utils import run_bass_kernel_spmd

S, B, H, NH, DK = 2048, 4, 1024, 16, 64
N_CORES = 8
BF16 = mybir.dt.bfloat16
F32 = mybir.dt.float32
NPBF16 = ml_dtypes.bfloat16
MASK_BIAS = -30000.0

_prog_cache: dict = {}


def _build_program(NQ, NK, VQ):
    """One SPMD program. Per batch-slot s: NQ[s] 512-wide q chunks (last one
    VQ[s] wide), NK[s] 128-wide k chunks. Slot 0 is the smaller workload."""
    NSCK = [(nk * 128 + 511) // 512 for nk in NK]
    KW = [nk * 128 for nk in NK]                    # k/v valid width
    QW = [(NQ[s] - 1) * 512 + VQ[s] for s in range(2)]  # q valid width
    nc = bacc.Bacc("TRN2", target_bir_lowering=False, debug=False,
                   num_devices=N_CORES)

    d_in = {}
    for s in range(2):
        d_in[f"qT{s}"] = nc.dram_tensor(f"qT{s}", [H, S], BF16, kind="ExternalInput")
        d_in[f"kT{s}"] = nc.dram_tensor(f"kT{s}", [H, S], BF16, kind="ExternalInput")
        d_in[f"vT{s}"] = nc.dram_tensor(f"vT{s}", [H, S], BF16, kind="ExternalInput")
        d_in[f"mb{s}"] = nc.dram_tensor(f"mb{s}", [128, 16], F32, kind="ExternalInput")
        d_in[f"kq{s}"] = nc.dram_tensor(f"kq{s}", [4, 512], F32, kind="ExternalInput")
    d_in["wqT"] = nc.dram_tensor("wqT", [H, 256], BF16, kind="ExternalInput")
    d_in["wkT"] = nc.dram_tensor("wkT", [H, 256], BF16, kind="ExternalInput")
    d_in["wvT"] = nc.dram_tensor("wvT", [H, 256], BF16, kind="ExternalInput")
    d_in["woT"] = nc.dram_tensor("woT", [256, H], BF16, kind="ExternalInput")
    d_out = [nc.dram_tensor(f"y{s}", [H, S], BF16, kind="ExternalOutput")
             for s in range(2)]

    def qcw(s, sc):
        return 512 if sc < NQ[s] - 1 else VQ[s]

    def kcw(s, sc):
        return min(512, KW[s] - sc * 512)

    with tile.TileContext(nc) as tc:
        with tc.tile_pool(name="wpool", bufs=1) as wpool, \
             tc.tile_pool(name="inp", bufs=6) as inp, \
             tc.tile_pool(name="in8", bufs=1) as in8, \
             tc.tile_pool(name="persist", bufs=1) as persist, \
             tc.tile_pool(name="probs", bufs=3) as probsp, \
             tc.tile_pool(name="small", bufs=2) as small, \
             tc.tile_pool(name="att", bufs=3) as attp, \
             tc.tile_pool(name="yst", bufs=3) as ystp:

            # --- weights: consolidated DMAs on the sync queue ---
            # w*_all[p, ic*256 + j] = w*T[ic*128 + p, j]
            wq_all = wpool.tile([128, 2048], BF16, name="wq_all", tag="wq")
            wk_all = wpool.tile([128, 2048], BF16, name="wk_all", tag="wk")
            wv_all = wpool.tile([128, 2048], BF16, name="wv_all", tag="wv")
            # wo_all[p, j*1024 + c] = woT[j*128 + p, c]
            wo_all = wpool.tile([128, 2048], BF16, name="wo_all", tag="wo")
            _wseen = set()

            def wqkv(t, ic, ft):
                return t[:, ic * 256 + ft * 128: ic * 256 + (ft + 1) * 128]

            mb = [wpool.tile([128, 16], F32, name=f"mbt{s}", tag=f"mbt{s}")
                  for s in range(2)]

            def emit_wo_mb():
                for j in range(2):
                    nc.sync.dma_start(
                        out=wo_all[:, j * 1024:(j + 1) * 1024],
                        in_=d_in["woT"].ap()[j * 128:(j + 1) * 128, :])
                for s in range(2):
                    nc.sync.dma_start(out=mb[s][:], in_=d_in[f"mb{s}"].ap())
            ones = wpool.tile([128, 64], BF16, name="ones", tag="ones")
            nc.vector.memset(ones[:], 1.0)

            # --- persistent projection outputs ---
            qTp = [[persist.tile([128, NQ[s] * 512], BF16, name=f"qTp{s}_{p}",
                                 tag=f"qTp{s}_{p}")
                    for p in range(2)] for s in range(2)]
            kTp = [[persist.tile([128, NSCK[s] * 512], BF16, name=f"kTp{s}_{p}",
                                 tag=f"kTp{s}_{p}")
                    for p in range(2)] for s in range(2)]
            vp = [[persist.tile([128, 256], BF16, name=f"vp{s}_{st}", tag=f"vp{s}_{st}")
                   for st in range(NK[s])] for s in range(2)]

            # DMA queue rotation for input streams (keep Scalar clean once
            # attention starts; Sync carries the weights up front)
            s0_queues = [nc.scalar, nc.gpsimd, nc.sync]
            s1_queues = [nc.sync, nc.gpsimd]
            _qi = [0]

            def dma_rot(queues, out, in_):
                q = queues[_qi[0] % len(queues)]
                _qi[0] += 1
                q.dma_start(out=out, in_=in_)

            def emit_w(t, dname, ic):
                # interleave weight-chunk DMAs with the input stream so the
                # first matmuls aren't gated on the full weight load
                if (dname, ic) in _wseen:
                    return
                _wseen.add((dname, ic))
                dma_rot(s0_queues, t[:, ic * 256:(ic + 1) * 256],
                        d_in[dname].ap()[ic * 128:(ic + 1) * 128, :])

            def emit_proj_streamed(s, pool):
                """ic-outer projections with streamed inputs (slot 0)."""
                for kind, wts, dname, nsc, outtiles, cw in (
                        ("q", wq_all, f"qT{s}", NQ[s], qTp[s], qcw),
                        ("k", wk_all, f"kT{s}", NSCK[s], kTp[s], kcw)):
                    wname = "wqT" if kind == "q" else "wkT"
                    ps = [[pool.tile([128, 512], F32,
                                     name=f"pj{kind}{s}_{ft}_{sc}",
                                     tag=f"pj_{ft}_{sc}")
                           for sc in range(nsc)] for ft in range(2)]
                    tw = sum(cw(s, sc) for sc in range(nsc))
                    for ic in range(8):
                        emit_w(wts, wname, ic)
                        it = inp.tile([128, 2048], BF16,
                                      name=f"in{kind}{s}_{ic}", tag="inp")
                        dma_rot(s0_queues,
                                it[:, 0:tw],
                                d_in[dname].ap()[ic * 128:(ic + 1) * 128, 0:tw])
                        for ft in range(2):
                            for sc in range(nsc):
                                w = cw(s, sc)
                                nc.tensor.matmul(
                                    out=ps[ft][sc][:, 0:w],
                                    lhsT=wqkv(wts, ic, ft),
                                    rhs=it[:, sc * 512: sc * 512 + w],
                                    start=(ic == 0), stop=(ic == 7))
                    for ft in range(2):
                        for sc in range(nsc):
                            w = cw(s, sc)
                            if kind == "q":
                                nc.vector.tensor_copy(
                                    outtiles[ft][:, sc * 512: sc * 512 + w],
                                    ps[ft][sc][:, 0:w])
                            else:
                                nc.scalar.copy(
                                    outtiles[ft][:, sc * 512: sc * 512 + w],
                                    ps[ft][sc][:, 0:w])

            def emit_v_prefetch0():
                tiles = []
                for ic in range(8):
                    emit_w(wv_all, "wvT", ic)
                    it = in8.tile([128, KW[0]], BF16, name=f"v0in_{ic}",
                                  tag=f"v0in{ic}")
                    dma_rot(s0_queues, it[:],
                            d_in["vT0"].ap()[ic * 128:(ic + 1) * 128, 0:KW[0]])
                    tiles.append(it)
                return tiles

            def emit_v_resident(s, pool, tiles):
                for st0 in range(0, NK[s], 8):
                    sts = range(st0, min(st0 + 8, NK[s]))
                    psv = {st: pool.tile([128, 256], F32, name=f"pjv{s}_{st}",
                                         tag=f"pj_{(st - st0) // 4}_{(st - st0) % 4}")
                           for st in sts}
                    for ic in range(8):
                        for st in sts:
                            nc.tensor.matmul(
                                out=psv[st][:],
                                lhsT=tiles[ic][:, st * 128:(st + 1) * 128],
                                rhs=wv_all[:, ic * 256: ic * 256 + 256],
                                start=(ic == 0), stop=(ic == 7))
                    for st in sts:
                        if st % 2:
                            nc.scalar.copy(vp[s][st][:], psv[st][:])
                        else:
                            nc.vector.tensor_copy(vp[s][st][:], psv[st][:])

            def emit_prefetch(s):
                """Issue all of slot s's input DMAs into dedicated tiles."""
                tiles = {}
                for kind, dname, w in (("v", f"vT{s}", KW[s]),
                                       ("k", f"kT{s}", KW[s]),
                                       ("q", f"qT{s}", QW[s])):
                    for ic in range(8):
                        it = in8.tile([128, w], BF16, name=f"pf{kind}{s}_{ic}",
                                      tag=f"pf{kind}{ic}")
                        dma_rot(s1_queues,
                                it[:],
                                d_in[dname].ap()[ic * 128:(ic + 1) * 128, 0:w])
                        tiles[(kind, ic)] = it
                return tiles

            def proj_units(s, pool, tiles, nkit, kinds=("v", "k", "q")):
                """Deadline-tagged projection units for slot s, consumed by
                interleaving into the attention kc-loop. Deadline = global
                iteration index ((qc*2)+p)*NK + kc of first use."""
                units = []

                def q_unit(ft, sc):
                    def emit():
                        w = qcw(s, sc)
                        pj = pool.tile([128, 512], F32,
                                       name=f"rpjq{s}_{ft}_{sc}", tag="y")
                        for ic in range(8):
                            nc.tensor.matmul(
                                out=pj[:, 0:w],
                                lhsT=wqkv(wq_all, ic, ft),
                                rhs=tiles[("q", ic)][:, sc * 512: sc * 512 + w],
                                start=(ic == 0), stop=(ic == 7))
                        nc.vector.tensor_copy(
                            qTp[s][ft][:, sc * 512: sc * 512 + w], pj[:, 0:w])
                    return emit

                def k_unit(ft, sc):
                    def emit():
                        w = kcw(s, sc)
                        pj = pool.tile([128, 512], F32,
                                       name=f"rpjk{s}_{ft}_{sc}", tag="y")
                        for ic in range(8):
                            nc.tensor.matmul(
                                out=pj[:, 0:w],
                                lhsT=wqkv(wk_all, ic, ft),
                                rhs=tiles[("k", ic)][:, sc * 512: sc * 512 + w],
                                start=(ic == 0), stop=(ic == 7))
                        nc.vector.tensor_copy(
                            kTp[s][ft][:, sc * 512: sc * 512 + w], pj[:, 0:w])
                    return emit

                def v_unit(st):
                    def emit():
                        pj = pool.tile([128, 512], F32,
                                       name=f"rpjv{s}_{st}", tag="y")
                        for ic in range(8):
                            nc.tensor.matmul(
                                out=pj[:, 0:256],
                                lhsT=tiles[("v", ic)][:, st * 128:(st + 1) * 128],
                                rhs=wv_all[:, ic * 256: ic * 256 + 256],
                                start=(ic == 0), stop=(ic == 7))
                        nc.vector.tensor_copy(vp[s][st][:], pj[:, 0:256])
                    return emit

                vk = []
                if "v" in kinds:
                    for st in range(NK[s]):
                        vk.append(v_unit(st))
                if "k" in kinds:
                    for ft in range(2):
                        for sc in range(NSCK[s]):
                            vk.append(k_unit(ft, sc))
                if "q" in kinds:
                    for ft in range(2):
                        for sc in range(NQ[s]):
                            units.append((max(0, (sc * 2 + ft) * nkit - nkit // 2),
                                          q_unit(ft, sc)))
                units.sort(key=lambda u: u[0])
                return vk, units

            def emit_attention(s, psc, pat, pz, pwo, units=None, carry_wo=None,
                               lead=4):
                units = list(units) if units else []
                wo_pending = list(carry_wo) if carry_wo else []
                nkit = NK[s]

                def drain_units(cur_idx, pre_group=False):
                    # hard deadlines only: consumers of these units follow
                    # in the in-order PE stream
                    while units and units[0][0] <= cur_idx:
                        units.pop(0)[1]()

                def drain_early(cur_idx):
                    # opportunistic emission after the iteration's PV/Z, so
                    # units never delay the scores->exp pipeline
                    n = 0
                    while units and n < 2 and units[0][0] <= cur_idx + lead:
                        units.pop(0)[1]()
                        n += 1
                    m = 0
                    while wo_pending and wo_pending[0][0] <= cur_idx and m < 2:
                        wo_pending.pop(0)[1](None)
                        m += 1

                def make_wo_unit(qc, ot, ab_pair, W):
                    def emit(tag):
                        if tag is None:
                            yps = pwo.tile([128, 512], F32,
                                           name=f"yp{s}_{qc}_{ot}", tag="y")
                            ypv = yps[:, 0:W]
                        else:
                            yps = psc.tile([128, 1024], F32,
                                           name=f"yp{s}_{qc}_{ot}", tag="sc")
                            ypv = yps[:, 0:W]
                        for j in range(2):
                            nc.tensor.matmul(
                                out=ypv,
                                lhsT=wo_all[:, j * 1024 + ot * 128: j * 1024 + (ot + 1) * 128],
                                rhs=ab_pair[j][:, 0:W],
                                start=(j == 0), stop=(j == 1))
                        ysb = ystp.tile([128, 512], BF16,
                                        name=f"ysb{s}_{qc}_{ot}", tag="ysb")
                        nc.vector.tensor_copy(ysb[:, 0:W], ypv)
                        (nc.gpsimd if ot % 2 else nc.sync).dma_start(
                            out=d_out[s].ap()[ot * 128:(ot + 1) * 128,
                                              qc * 512: qc * 512 + W],
                            in_=ysb[:, 0:W])
                    return emit

                for qc in range(NQ[s]):
                    W = qcw(s, qc)
                    kqr = small.tile([128, 512], F32, name=f"kqr{s}_{qc}",
                                     tag="kqr")
                    nc.gpsimd.dma_start(
                        out=kqr[:, 0:W],
                        in_=bass.AP(tensor=d_in[f"kq{s}"], offset=qc * 512,
                                    ap=[[0, 128], [1, W]]))
                    att_sb = []
                    for p in range(2):
                        base_idx = (qc * 2 + p) * nkit
                        # hard-deadline units must precede this group's
                        # first scores emission (they share the PE queue)
                        drain_units(base_idx)
                        attn = pat.tile([128, 512], F32,
                                        name=f"at{s}_{qc}_{p}", tag="at")
                        zps = pz.tile([128, 512], F32,
                                      name=f"z{s}_{qc}_{p}", tag="z")

                        # software-pipelined: scores(kc+1) emitted before
                        # PV(kc) so the in-order PE queue never stalls on exp
                        def emit_scores(kc):
                            sc_ps = psc.tile([128, 1024], F32,
                                             name=f"s{s}_{qc}_{p}_{kc}",
                                             tag="sc")
                            pr = probsp.tile([128, 1024], BF16,
                                             name=f"pr{s}_{qc}_{p}_{kc}",
                                             tag="pr")
                            for hh in range(2):
                                hsl = slice(hh * 64, hh * 64 + 64)
                                # 512-strided blocks: the two row-group-
                                # concurrent matmuls must drain to
                                # different PSUM banks
                                nc.tensor.matmul(
                                    out=sc_ps[:, hh * 512: hh * 512 + W],
                                    lhsT=kTp[s][p][hsl, kc * 128:(kc + 1) * 128],
                                    rhs=qTp[s][p][hsl, qc * 512: qc * 512 + W],
                                    start=True, stop=True)
                            if W == 512:
                                exp_in = sc_ps[:, 0:1024]
                                exp_out = pr[:, 0:1024]
                            else:
                                exp_in = sc_ps[:].rearrange(
                                    "p (b w) -> p b w", b=2)[:, :, 0:W]
                                exp_out = pr[:, 0:2 * W].rearrange(
                                    "p (b w) -> p b w", b=2)
                            nc.scalar.activation(
                                out=exp_out, in_=exp_in,
                                func=mybir.ActivationFunctionType.Exp,
                                bias=mb[s][:, kc:kc + 1],
                                scale=1.0 / math.sqrt(DK))
                            return pr

                        pr_next = emit_scores(0)
                        for kc in range(nkit):
                            drain_units(base_idx + kc)
                            first, last = kc == 0, kc == nkit - 1
                            pr = pr_next
                            if not last:
                                pr_next = emit_scores(kc + 1)
                            for hh in range(2):
                                hsl = slice(hh * 64, hh * 64 + 64)
                                nc.tensor.matmul(
                                    out=attn[hsl, 0:W],
                                    lhsT=vp[s][kc][:, p * 128 + hh * 64:p * 128 + (hh + 1) * 64],
                                    rhs=pr[:, hh * W:(hh + 1) * W],
                                    start=first, stop=last)
                                nc.tensor.matmul(
                                    out=zps[hsl, 0:W],
                                    lhsT=ones[:, :], rhs=pr[:, hh * W:(hh + 1) * W],
                                    start=first, stop=last)
                            drain_early(base_idx + kc)
                        # normalize: attn * keepq / Z
                        rz = small.tile([128, 512], F32,
                                        name=f"rz{s}_{qc}_{p}", tag="rz")
                        nc.vector.reciprocal_approx_fast(out=rz[:, 0:W],
                                                         in_=zps[:, 0:W])
                        nc.vector.tensor_mul(rz[:, 0:W], rz[:, 0:W], kqr[:, 0:W])
                        ab = attp.tile([128, 512], BF16,
                                       name=f"ab{s}_{qc}_{p}", tag=f"ab{p}")
                        nc.vector.tensor_mul(ab[:, 0:W], attn[:, 0:W], rz[:, 0:W])
                        att_sb.append(ab)
                    # Wo is deferred: one unit per iteration of the NEXT
                    # group, so the output chain never gates the next
                    # group's scores/exp pipeline
                    for ot in range(8):
                        wo_pending.append(((qc + 1) * 2 * nkit + 1 + 2 * ot,
                                           make_wo_unit(qc, ot, att_sb, W)))
                # flush stragglers; alternate between the 'y' bank and the
                # now-idle scores slots so the tail chain pipelines
                while units:
                    units.pop(0)[1]()
                return wo_pending

            # slot 0 projections use the full PSUM (released afterwards)
            with tc.tile_pool(name="pproj", bufs=1, space="PSUM") as pproj:
                emit_proj_streamed(0, pproj)
                v0tiles = emit_v_prefetch0()
                emit_v_resident(0, pproj, v0tiles)
            emit_wo_mb()
            tiles1 = emit_prefetch(1)
            with tc.tile_pool(name="psc", bufs=2, space="PSUM") as psc, \
                 tc.tile_pool(name="pat", bufs=2, space="PSUM") as pat, \
                 tc.tile_pool(name="pz", bufs=1, space="PSUM") as pz, \
                 tc.tile_pool(name="pwo", bufs=1, space="PSUM") as pwo:
                vk1, units1 = proj_units(1, pwo, tiles1, NK[1])
                n0 = NQ[0] * 2 * NK[0]
                sp = max(1, (n0 - 6) // max(1, len(vk1)))
                units0 = [(2 + sp * i, fn) for i, fn in enumerate(vk1)]
                left0 = emit_attention(0, psc, pat, pz, pwo, units=units0)
                carry = [(i + 1, fn) for i, (_, fn) in enumerate(left0)]
                left1 = emit_attention(1, psc, pat, pz, pwo, units=units1,
                                       carry_wo=carry)
                for i, (_, fn) in enumerate(left1):
                    fn("sc" if i % 2 == 0 else None)
    nc.compile()
    return nc


def _get_program(NQ, NK, VQ):
    key = (tuple(NQ), tuple(NK), tuple(VQ))
    if key not in _prog_cache:
        _prog_cache[key] = _build_program(list(NQ), list(NK), list(VQ))
    return _prog_cache[key]


def kernel(value, key, query, padding_mask, Wq, Wk, Wv, Wo):
    value = np.asarray(value)
    key = np.asarray(key)
    query = np.asarray(query)
    padding_mask = np.asarray(padding_mask)
    Wq, Wk, Wv, Wo = (np.asarray(a) for a in (Wq, Wk, Wv, Wo))

    lengths = (~padding_mask).sum(axis=0).astype(int)  # (B,)

    # --- batch pairing: assign batches to (group, slot) minimizing baked work ---
    def slot_counts(assign):
        nq = [max((int(lengths[assign[g][sl]]) + 511) // 512 for g in range(2))
              for sl in range(2)]
        nk = [max((int(lengths[assign[g][sl]]) + 127) // 128 for g in range(2))
              for sl in range(2)]
        return nq, nk

    best = None
    for perm in permutations(range(B)):
        a = ((perm[0], perm[1]), (perm[2], perm[3]))
        nq, nk = slot_counts(a)
        c = nq[0] * nk[0] + nq[1] * nk[1]
        if best is None or c < best[0]:
            best = (c, a)
    assign = best[1]
    nq, nk = slot_counts(assign)
    # slot 0 should be the smaller workload (its projections can't overlap)
    if nq[0] * nk[0] > nq[1] * nk[1]:
        assign = tuple((g[1], g[0]) for g in assign)
        nq, nk = slot_counts(assign)
    NQ, NK = nq, nk
    # trimmed width of the last q chunk per slot (multiple of 32)
    VQ = []
    for sl in range(2):
        maxlen = max(int(lengths[assign[g][sl]]) for g in range(2))
        v = maxlen - (NQ[sl] - 1) * 512
        VQ.append(min(512, (v + 31) // 32 * 32))

    nc = _get_program(NQ, NK, VQ)

    # --- per-core inputs ---
    WqT = np.ascontiguousarray(Wq.T).astype(NPBF16)
    WkT = np.ascontiguousarray(Wk.T).astype(NPBF16)
    WvT = np.ascontiguousarray(Wv.T).astype(NPBF16)
    WoT = np.ascontiguousarray(Wo.T).astype(NPBF16)

    batch_qT, batch_kT, batch_vT, batch_mb, batch_kq = {}, {}, {}, {}, {}
    for b in range(B):
        batch_qT[b] = np.ascontiguousarray(query[:, b, :].T).astype(NPBF16)
        batch_kT[b] = np.ascontiguousarray(key[:, b, :].T).astype(NPBF16)
        batch_vT[b] = np.ascontiguousarray(value[:, b, :].T).astype(NPBF16)
        kpos = np.arange(S).reshape(16, 128)  # [kchunk, kpos]
        mbv = np.where(kpos >= lengths[b], np.float32(MASK_BIAS), np.float32(0.0))
        batch_mb[b] = np.ascontiguousarray(mbv.T).astype(np.float32)  # [128, 16]
        batch_kq[b] = (np.arange(S).reshape(4, 512) < lengths[b]).astype(np.float32)

    in_maps = []
    for c in range(N_CORES):
        g, hq = c // 4, c % 4
        f0 = hq * 256
        m = {
            "wqT": np.ascontiguousarray(WqT[:, f0:f0 + 256]),
            "wkT": np.ascontiguousarray(WkT[:, f0:f0 + 256]),
            "wvT": np.ascontiguousarray(WvT[:, f0:f0 + 256]),
            "woT": np.ascontiguousarray(WoT[f0:f0 + 256, :]),
        }
        for sl in range(2):
            b = assign[g][sl]
            m[f"qT{sl}"] = batch_qT[b]
            m[f"kT{sl}"] = batch_kT[b]
            m[f"vT{sl}"] = batch_vT[b]
            m[f"mb{sl}"] = batch_mb[b]
            m[f"kq{sl}"] = batch_kq[b]
        in_maps.append(m)

    res = run_bass_kernel_spmd(nc, in_maps, list(range(N_CORES)))

    # --- gather: sum 4 head-quad partials per batch, transpose ---
    out = np.zeros((S, B, H), dtype=np.float32)
    for g in range(2):
        for sl in range(2):
            b = assign[g][sl]
            acc = np.zeros((H, S), dtype=np.float32)
            for hq in range(4):
                c = g * 4 + hq
                acc += res.results[c][f"y{sl}"].astype(np.float32)
            out[:, b, :] = acc.T
    return out


# revision 35
# speedup vs baseline: 1.1949x; 1.0029x over previous
"""Multi-head attention (S=2048, B=4, H=1024, NH=16) on 8 Trainium2 NeuronCores.

Sharding: each core handles 2 batches x 4 heads (batch pairs balanced by
valid length; tensor-parallel over heads). Within a core everything is bf16
matmul / fp32 accumulate:
  1. q,k projected d-major (qT/kT: [dims, seq]), v seq-major ([seq, dims])
  2. scoresT[k,q] per head-pair via row-tiled matmuls (row_grp concurrency)
  3. mask+scale+exp fused on ScalarE (per-partition bias; PAD keys -> exp 0)
  4. PV col-tiled accumulates attnT; Z row-sums via all-ones stationary
     matmul (col_grp-concurrent with PV)
  5. attnT normalized by 1/Z; keepq applied before the Wo output multiply
  6. Wo projection -> yT partial [H, S]; host sums 4 partials/batch

Schedule: weight-chunk DMAs interleave with slot0's q/k input streams over
the three DMA-capable queues (sync/scalar/gpsimd), so the first projection
matmul issues ~2 transfers in; slot0 v inputs prefetch into dedicated tiles
(issue never blocks on tile recycling). Attention is emitted as a
software-pipelined kc-loop (scores(kc+1) before PV(kc)); the two scores
matmuls use 512-strided PSUM blocks so their row-group-concurrent drains hit
different banks (same-bank PE+PE drain is a fatal collision when trimmed),
with a strided-AP exp over both blocks. PV+Z pairs run col-group-concurrent.
Wo is deferred into per-ot units drained one-per-iteration of the NEXT
query-group (1-bank PSUM tag shared with projection units), so the output
chain never gates the scores/exp pipeline; the final group's units flush
alternating into the idle scores banks. Slot1's q/k/v projections become
deadline-scheduled units: v/k spread through attention0, q half-a-group
early in attention1. The last q-chunk of each slot is width-trimmed to the
valid length (multiple of 32), shrinking exp/matmul/DVE/DMA work on padded
queries.
"""
import sys

if "/opt/trn_rl_repo" not in sys.path:
    sys.path.insert(0, "/opt/trn_rl_repo")

import math
from itertools import permutations

import ml_dtypes
import numpy as np

import concourse.bass as bass
import concourse.mybir as mybir
import concourse.tile as tile
from concourse import bacc
from concourse.bass_utils import run_bass_kernel_spmd

S, B, H, NH, DK = 2048, 4, 1024, 16, 64
N_CORES = 8
BF16 = mybir.dt.bfloat16
F32 = mybir.dt.float32
NPBF16 = ml_dtypes.bfloat16
MASK_BIAS = -30000.0

_prog_cache: dict = {}


def _build_program(NQ, NK, VQ):
    """One SPMD program. Per batch-slot s: NQ[s] 512-wide q chunks (last one
    VQ[s] wide), NK[s] 128-wide k chunks. Slot 0 is the smaller workload."""
    NSCK = [(nk * 128 + 511) // 512 for nk in NK]
    KW = [nk * 128 for nk in NK]                    # k/v valid width
    QW = [(NQ[s] - 1) * 512 + VQ[s] for s in range(2)]  # q valid width
    nc = bacc.Bacc("TRN2", target_bir_lowering=False, debug=False,
                   num_devices=N_CORES)

    d_in = {}
    for s in range(2):
        d_in[f"qT{s}"] = nc.dram_tensor(f"qT{s}", [H, S], BF16, kind="ExternalInput")
        d_in[f"kT{s}"] = nc.dram_tensor(f"kT{s}", [H, S], BF16, kind="ExternalInput")
        d_in[f"vT{s}"] = nc.dram_tensor(f"vT{s}", [H, S], BF16, kind="ExternalInput")
        d_in[f"mb{s}"] = nc.dram_tensor(f"mb{s}", [128, 16], F32, kind="ExternalInput")
        d_in[f"kq{s}"] = nc.dram_tensor(f"kq{s}", [4, 512], F32, kind="ExternalInput")
    d_in["wqT"] = nc.dram_tensor("wqT", [H, 256], BF16, kind="ExternalInput")
    d_in["wkT"] = nc.dram_tensor("wkT", [H, 256], BF16, kind="ExternalInput")
    d_in["wvT"] = nc.dram_tensor("wvT", [H, 256], BF16, kind="ExternalInput")
    d_in["woT"] = nc.dram_tensor("woT", [256, H], BF16, kind="ExternalInput")
    d_out = [nc.dram_tensor(f"y{s}", [H, S], BF16, kind="ExternalOutput")
             for s in range(2)]

    def qcw(s, sc):
        return 512 if sc < NQ[s] - 1 else VQ[s]

    def kcw(s, sc):
        return min(512, KW[s] - sc * 512)

    with tile.TileContext(nc) as tc:
        with tc.tile_pool(name="wpool", bufs=1) as wpool, \
             tc.tile_pool(name="inp", bufs=6) as inp, \
             tc.tile_pool(name="in8", bufs=1) as in8, \
             tc.tile_pool(name="persist", bufs=1) as persist, \
             tc.tile_pool(name="probs", bufs=3) as probsp, \
             tc.tile_pool(name="small", bufs=2) as small, \
             tc.tile_pool(name="att", bufs=3) as attp, \
             tc.tile_pool(name="yst", bufs=3) as ystp:

            # --- weights: consolidated DMAs on the sync queue ---
            # w*_all[p, ic*256 + j] = w*T[ic*128 + p, j]
            wq_all = wpool.tile([128, 2048], BF16, name="wq_all", tag="wq")
            wk_all = wpool.tile([128, 2048], BF16, name="wk_all", tag="wk")
            wv_all = wpool.tile([128, 2048], BF16, name="wv_all", tag="wv")
            # wo_all[p, j*1024 + c] = woT[j*128 + p, c]
            wo_all = wpool.tile([128, 2048], BF16, name="wo_all", tag="wo")
            _wseen = set()

            def wqkv(t, ic, ft):
                return t[:, ic * 256 + ft * 128: ic * 256 + (ft + 1) * 128]

            mb = [wpool.tile([128, 16], F32, name=f"mbt{s}", tag=f"mbt{s}")
                  for s in range(2)]

            def emit_wo_mb():
                for j in range(2):
                    nc.sync.dma_start(
                        out=wo_all[:, j * 1024:(j + 1) * 1024],
                        in_=d_in["woT"].ap()[j * 128:(j + 1) * 128, :])
                for s in range(2):
                    nc.sync.dma_start(out=mb[s][:], in_=d_in[f"mb{s}"].ap())
            ones = wpool.tile([128, 64], BF16, name="ones", tag="ones")
            nc.vector.memset(ones[:], 1.0)

            # --- persistent projection outputs ---
            qTp = [[persist.tile([128, NQ[s] * 512], BF16, name=f"qTp{s}_{p}",
                                 tag=f"qTp{s}_{p}")
                    for p in range(2)] for s in range(2)]
            kTp = [[persist.tile([128, NSCK[s] * 512], BF16, name=f"kTp{s}_{p}",
                                 tag=f"kTp{s}_{p}")
                    for p in range(2)] for s in range(2)]
            vp = [[persist.tile([128, 256], BF16, name=f"vp{s}_{st}", tag=f"vp{s}_{st}")
                   for st in range(NK[s])] for s in range(2)]

            # DMA queue rotation for input streams (keep Scalar clean once
            # attention starts; Sync carries the weights up front)
            s0_queues = [nc.scalar, nc.gpsimd, nc.sync]
            s1_queues = [nc.sync, nc.gpsimd]
            _qi = [0]

            def dma_rot(queues, out, in_):
                q = queues[_qi[0] % len(queues)]
                _qi[0] += 1
                q.dma_start(out=out, in_=in_)

            def emit_w(t, dname, ic):
                # interleave weight-chunk DMAs with the input stream so the
                # first matmuls aren't gated on the full weight load
                if (dname, ic) in _wseen:
                    return
                _wseen.add((dname, ic))
                dma_rot(s0_queues, t[:, ic * 256:(ic + 1) * 256],
                        d_in[dname].ap()[ic * 128:(ic + 1) * 128, :])

            def emit_proj_streamed(s, pool):
                """ic-outer projections with streamed inputs (slot 0)."""
                for kind, wts, dname, nsc, outtiles, cw in (
                        ("q", wq_all, f"qT{s}", NQ[s], qTp[s], qcw),
                        ("k", wk_all, f"kT{s}", NSCK[s], kTp[s], kcw)):
                    wname = "wqT" if kind == "q" else "wkT"
                    ps = [[pool.tile([128, 512], F32,
                                     name=f"pj{kind}{s}_{ft}_{sc}",
                                     tag=f"pj_{ft}_{sc}")
                           for sc in range(nsc)] for ft in range(2)]
                    tw = sum(cw(s, sc) for sc in range(nsc))
                    for ic in range(8):
                        emit_w(wts, wname, ic)
                        it = inp.tile([128, 2048], BF16,
                                      name=f"in{kind}{s}_{ic}", tag="inp")
                        dma_rot(s0_queues,
                                it[:, 0:tw],
                                d_in[dname].ap()[ic * 128:(ic + 1) * 128, 0:tw])
                        for ft in range(2):
                            for sc in range(nsc):
                                w = cw(s, sc)
                                nc.tensor.matmul(
                                    out=ps[ft][sc][:, 0:w],
                                    lhsT=wqkv(wts, ic, ft),
                                    rhs=it[:, sc * 512: sc * 512 + w],
                                    start=(ic == 0), stop=(ic == 7))
                    for ft in range(2):
                        for sc in range(nsc):
                            w = cw(s, sc)
                            if kind == "q":
                                nc.vector.tensor_copy(
                                    outtiles[ft][:, sc * 512: sc * 512 + w],
                                    ps[ft][sc][:, 0:w])
                            else:
                                nc.scalar.copy(
                                    outtiles[ft][:, sc * 512: sc * 512 + w],
                                    ps[ft][sc][:, 0:w])

            def emit_v_prefetch0():
                tiles = []
                for ic in range(8):
                    emit_w(wv_all, "wvT", ic)
                    it = in8.tile([128, KW[0]], BF16, name=f"v0in_{ic}",
                                  tag=f"v0in{ic}")
                    dma_rot(s0_queues, it[:],
                            d_in["vT0"].ap()[ic * 128:(ic + 1) * 128, 0:KW[0]])
                    tiles.append(it)
                return tiles

            def emit_v_resident(s, pool, tiles):
                for st0 in range(0, NK[s], 8):
                    sts = range(st0, min(st0 + 8, NK[s]))
                    psv = {st: pool.tile([128, 256], F32, name=f"pjv{s}_{st}",
                                         tag=f"pj_{(st - st0) // 4}_{(st - st0) % 4}")
                           for st in sts}
                    for ic in range(8):
                        for st in sts:
                            nc.tensor.matmul(
                                out=psv[st][:],
                                lhsT=tiles[ic][:, st * 128:(st + 1) * 128],
                                rhs=wv_all[:, ic * 256: ic * 256 + 256],
                                start=(ic == 0), stop=(ic == 7))
                    for st in sts:
                        if st % 2:
                            nc.scalar.copy(vp[s][st][:], psv[st][:])
                        else:
                            nc.vector.tensor_copy(vp[s][st][:], psv[st][:])

            def emit_prefetch(s):
                """Issue all of slot s's input DMAs into dedicated tiles."""
                tiles = {}
                for kind, dname, w in (("v", f"vT{s}", KW[s]),
                                       ("k", f"kT{s}", KW[s]),
                                       ("q", f"qT{s}", QW[s])):
                    for ic in range(8):
                        it = in8.tile([128, w], BF16, name=f"pf{kind}{s}_{ic}",
                                      tag=f"pf{kind}{ic}")
                        dma_rot(s1_queues,
                                it[:],
                                d_in[dname].ap()[ic * 128:(ic + 1) * 128, 0:w])
                        tiles[(kind, ic)] = it
                return tiles

            def proj_units(s, pool, tiles, nkit, kinds=("v", "k", "q")):
                """Deadline-tagged projection units for slot s, consumed by
                interleaving into the attention kc-loop. Deadline = global
                iteration index ((qc*2)+p)*NK + kc of first use."""
                units = []

                def q_unit(ft, sc):
                    def emit():
                        w = qcw(s, sc)
                        pj = pool.tile([128, 512], F32,
                                       name=f"rpjq{s}_{ft}_{sc}", tag="y")
                        for ic in range(8):
                            nc.tensor.matmul(
                                out=pj[:, 0:w],
                                lhsT=wqkv(wq_all, ic, ft),
                                rhs=tiles[("q", ic)][:, sc * 512: sc * 512 + w],
                                start=(ic == 0), stop=(ic == 7))
                        nc.vector.tensor_copy(
                            qTp[s][ft][:, sc * 512: sc * 512 + w], pj[:, 0:w])
                    return emit

                def k_unit(ft, sc):
                    def emit():
                        w = kcw(s, sc)
                        pj = pool.tile([128, 512], F32,
                                       name=f"rpjk{s}_{ft}_{sc}", tag="y")
                        for ic in range(8):
                            nc.tensor.matmul(
                                out=pj[:, 0:w],
                                lhsT=wqkv(wk_all, ic, ft),
                                rhs=tiles[("k", ic)][:, sc * 512: sc * 512 + w],
                                start=(ic == 0), stop=(ic == 7))
                        nc.vector.tensor_copy(
                            kTp[s][ft][:, sc * 512: sc * 512 + w], pj[:, 0:w])
                    return emit

                def v_unit(st):
                    def emit():
                        pj = pool.tile([128, 512], F32,
                                       name=f"rpjv{s}_{st}", tag="y")
                        for ic in range(8):
                            nc.tensor.matmul(
                                out=pj[:, 0:256],
                                lhsT=tiles[("v", ic)][:, st * 128:(st + 1) * 128],
                                rhs=wv_all[:, ic * 256: ic * 256 + 256],
                                start=(ic == 0), stop=(ic == 7))
                        nc.vector.tensor_copy(vp[s][st][:], pj[:, 0:256])
                    return emit

                vk = []
                if "v" in kinds:
                    for st in range(NK[s]):
                        vk.append(v_unit(st))
                if "k" in kinds:
                    for ft in range(2):
                        for sc in range(NSCK[s]):
                            vk.append(k_unit(ft, sc))
                if "q" in kinds:
                    for ft in range(2):
                        for sc in range(NQ[s]):
                            units.append((max(0, (sc * 2 + ft) * nkit - nkit // 2),
                                          q_unit(ft, sc)))
                units.sort(key=lambda u: u[0])
                return vk, units

            def emit_attention(s, psc, pat, pz, pwo, units=None, carry_wo=None,
                               lead=4):
                units = list(units) if units else []
                wo_pending = list(carry_wo) if carry_wo else []
                nkit = NK[s]

                def drain_units(cur_idx, pre_group=False):
                    # hard deadlines only: consumers of these units follow
                    # in the in-order PE stream
                    while units and units[0][0] <= cur_idx:
                        units.pop(0)[1]()

                def drain_early(cur_idx):
                    # opportunistic emission after the iteration's PV/Z, so
                    # units never delay the scores->exp pipeline
                    n = 0
                    while units and n < 2 and units[0][0] <= cur_idx + lead:
                        units.pop(0)[1]()
                        n += 1
                    m = 0
                    while wo_pending and wo_pending[0][0] <= cur_idx and m < 2:
                        wo_pending.pop(0)[1](None)
                        m += 1

                def make_wo_unit(qc, ot, ab_pair, W):
                    def emit(tag):
                        if tag is None:
                            yps = pwo.tile([128, 512], F32,
                                           name=f"yp{s}_{qc}_{ot}", tag="y")
                            ypv = yps[:, 0:W]
                        else:
                            yps = psc.tile([128, 1024], F32,
                                           name=f"yp{s}_{qc}_{ot}", tag="sc")
                            ypv = yps[:, 0:W]
                        for j in range(2):
                            nc.tensor.matmul(
                                out=ypv,
                                lhsT=wo_all[:, j * 1024 + ot * 128: j * 1024 + (ot + 1) * 128],
                                rhs=ab_pair[j][:, 0:W],
                                start=(j == 0), stop=(j == 1))
                        ysb = ystp.tile([128, 512], BF16,
                                        name=f"ysb{s}_{qc}_{ot}", tag="ysb")
                        if tag is not None:
                            nc.scalar.copy(ysb[:, 0:W], ypv)
                        else:
                            nc.vector.tensor_copy(ysb[:, 0:W], ypv)
                        (nc.gpsimd if ot % 2 else nc.sync).dma_start(
                            out=d_out[s].ap()[ot * 128:(ot + 1) * 128,
                                              qc * 512: qc * 512 + W],
                            in_=ysb[:, 0:W])
                    return emit

                kqr_t = {}

                def emit_kqr(qc):
                    w = qcw(s, qc)
                    t = small.tile([128, 512], F32, name=f"kqr{s}_{qc}",
                                   tag="kqr")
                    nc.gpsimd.dma_start(
                        out=t[:, 0:w],
                        in_=bass.AP(tensor=d_in[f"kq{s}"], offset=qc * 512,
                                    ap=[[0, 128], [1, w]]))
                    kqr_t[qc] = t

                emit_kqr(0)
                for qc in range(NQ[s]):
                    W = qcw(s, qc)
                    if qc + 1 < NQ[s]:
                        emit_kqr(qc + 1)
                    kqr = kqr_t.pop(qc)
                    att_sb = []
                    for p in range(2):
                        base_idx = (qc * 2 + p) * nkit
                        # hard-deadline units must precede this group's
                        # first scores emission (they share the PE queue)
                        drain_units(base_idx)
                        attn = pat.tile([128, 512], F32,
                                        name=f"at{s}_{qc}_{p}", tag="at")
                        zps = pz.tile([128, 512], F32,
                                      name=f"z{s}_{qc}_{p}", tag="z")

                        # software-pipelined: scores(kc+1) emitted before
                        # PV(kc) so the in-order PE queue never stalls on exp
                        def emit_scores(kc):
                            sc_ps = psc.tile([128, 1024], F32,
                                             name=f"s{s}_{qc}_{p}_{kc}",
                                             tag="sc")
                            pr = probsp.tile([128, 1024], BF16,
                                             name=f"pr{s}_{qc}_{p}_{kc}",
                                             tag="pr")
                            for hh in range(2):
                                hsl = slice(hh * 64, hh * 64 + 64)
                                # 512-strided blocks: the two row-group-
                                # concurrent matmuls must drain to
                                # different PSUM banks
                                nc.tensor.matmul(
                                    out=sc_ps[:, hh * 512: hh * 512 + W],
                                    lhsT=kTp[s][p][hsl, kc * 128:(kc + 1) * 128],
                                    rhs=qTp[s][p][hsl, qc * 512: qc * 512 + W],
                                    start=True, stop=True)
                            if W == 512:
                                exp_in = sc_ps[:, 0:1024]
                                exp_out = pr[:, 0:1024]
                            else:
                                exp_in = sc_ps[:].rearrange(
                                    "p (b w) -> p b w", b=2)[:, :, 0:W]
                                exp_out = pr[:, 0:2 * W].rearrange(
                                    "p (b w) -> p b w", b=2)
                            nc.scalar.activation(
                                out=exp_out, in_=exp_in,
                                func=mybir.ActivationFunctionType.Exp,
                                bias=mb[s][:, kc:kc + 1],
                                scale=1.0 / math.sqrt(DK))
                            return pr

                        pr_next = emit_scores(0)
                        for kc in range(nkit):
                            drain_units(base_idx + kc)
                            first, last = kc == 0, kc == nkit - 1
                            pr = pr_next
                            if not last:
                                pr_next = emit_scores(kc + 1)
                            for hh in range(2):
                                hsl = slice(hh * 64, hh * 64 + 64)
                                nc.tensor.matmul(
                                    out=attn[hsl, 0:W],
                                    lhsT=vp[s][kc][:, p * 128 + hh * 64:p * 128 + (hh + 1) * 64],
                                    rhs=pr[:, hh * W:(hh + 1) * W],
                                    start=first, stop=last)
                                nc.tensor.matmul(
                                    out=zps[hsl, 0:W],
                                    lhsT=ones[:, :], rhs=pr[:, hh * W:(hh + 1) * W],
                                    start=first, stop=last)
                            drain_early(base_idx + kc)
                        # normalize: attn * keepq / Z
                        rz = small.tile([128, 512], F32,
                                        name=f"rz{s}_{qc}_{p}", tag="rz")
                        nc.vector.reciprocal_approx_fast(out=rz[:, 0:W],
                                                         in_=zps[:, 0:W])
                        nc.vector.tensor_mul(rz[:, 0:W], rz[:, 0:W], kqr[:, 0:W])
                        ab = attp.tile([128, 512], BF16,
                                       name=f"ab{s}_{qc}_{p}", tag=f"ab{p}")
                        nc.vector.tensor_mul(ab[:, 0:W], attn[:, 0:W], rz[:, 0:W])
                        att_sb.append(ab)
                    # Wo is deferred: one unit per iteration of the NEXT
                    # group, so the output chain never gates the next
                    # group's scores/exp pipeline
                    for ot in range(8):
                        wo_pending.append(((qc + 1) * 2 * nkit + 1 + 2 * ot,
                                           make_wo_unit(qc, ot, att_sb, W)))
                # flush stragglers; alternate between the 'y' bank and the
                # now-idle scores slots so the tail chain pipelines
                while units:
                    units.pop(0)[1]()
                return wo_pending

            # slot 0 projections use the full PSUM (released afterwards)
            with tc.tile_pool(name="pproj", bufs=1, space="PSUM") as pproj:
                emit_proj_streamed(0, pproj)
                v0tiles = emit_v_prefetch0()
                emit_v_resident(0, pproj, v0tiles)
            emit_wo_mb()
            tiles1 = emit_prefetch(1)
            with tc.tile_pool(name="psc", bufs=2, space="PSUM") as psc, \
                 tc.tile_pool(name="pat", bufs=2, space="PSUM") as pat, \
                 tc.tile_pool(name="pz", bufs=1, space="PSUM") as pz, \
                 tc.tile_pool(name="pwo", bufs=1, space="PSUM") as pwo:
                vk1, units1 = proj_units(1, pwo, tiles1, NK[1])
                n0 = NQ[0] * 2 * NK[0]
                sp = max(1, (n0 - 6) // max(1, len(vk1)))
                units0 = [(2 + sp * i, fn) for i, fn in enumerate(vk1)]
                left0 = emit_attention(0, psc, pat, pz, pwo, units=units0)
                carry = [(i + 1, fn) for i, (_, fn) in enumerate(left0)]
                left1 = emit_attention(1, psc, pat, pz, pwo, units=units1,
                                       carry_wo=carry)
                for i, (_, fn) in enumerate(left1):
                    fn("sc" if i % 2 == 0 else None)
    nc.compile()
    return nc


def _get_program(NQ, NK, VQ):
    key = (tuple(NQ), tuple(NK), tuple(VQ))
    if key not in _prog_cache:
        _prog_cache[key] = _build_program(list(NQ), list(NK), list(VQ))
    return _prog_cache[key]


def kernel(value, key, query, padding_mask, Wq, Wk, Wv, Wo):
    value = np.asarray(value)
    key = np.asarray(key)
    query = np.asarray(query)
    padding_mask = np.asarray(padding_mask)
    Wq, Wk, Wv, Wo = (np.asarray(a) for a in (Wq, Wk, Wv, Wo))

    lengths = (~padding_mask).sum(axis=0).astype(int)  # (B,)

    # --- batch pairing: assign batches to (group, slot) minimizing baked work ---
    def slot_counts(assign):
        nq = [max((int(lengths[assign[g][sl]]) + 511) // 512 for g in range(2))
              for sl in range(2)]
        nk = [max((int(lengths[assign[g][sl]]) + 127) // 128 for g in range(2))
              for sl in range(2)]
        return nq, nk

    best = None
    for perm in permutations(range(B)):
        a = ((perm[0], perm[1]), (perm[2], perm[3]))
        nq, nk = slot_counts(a)
        c = nq[0] * nk[0] + nq[1] * nk[1]
        if best is None or c < best[0]:
            best = (c, a)
    assign = best[1]
    nq, nk = slot_counts(assign)
    # slot 0 should be the smaller workload (its projections can't overlap)
    if nq[0] * nk[0] > nq[1] * nk[1]:
        assign = tuple((g[1], g[0]) for g in assign)
        nq, nk = slot_counts(assign)
    NQ, NK = nq, nk
    # trimmed width of the last q chunk per slot (multiple of 32)
    VQ = []
    for sl in range(2):
        maxlen = max(int(lengths[assign[g][sl]]) for g in range(2))
        v = maxlen - (NQ[sl] - 1) * 512
        VQ.append(min(512, (v + 31) // 32 * 32))

    nc = _get_program(NQ, NK, VQ)

    # --- per-core inputs ---
    WqT = np.ascontiguousarray(Wq.T).astype(NPBF16)
    WkT = np.ascontiguousarray(Wk.T).astype(NPBF16)
    WvT = np.ascontiguousarray(Wv.T).astype(NPBF16)
    WoT = np.ascontiguousarray(Wo.T).astype(NPBF16)

    batch_qT, batch_kT, batch_vT, batch_mb, batch_kq = {}, {}, {}, {}, {}
    for b in range(B):
        batch_qT[b] = np.ascontiguousarray(query[:, b, :].T).astype(NPBF16)
        batch_kT[b] = np.ascontiguousarray(key[:, b, :].T).astype(NPBF16)
        batch_vT[b] = np.ascontiguousarray(value[:, b, :].T).astype(NPBF16)
        kpos = np.arange(S).reshape(16, 128)  # [kchunk, kpos]
        mbv = np.where(kpos >= lengths[b], np.float32(MASK_BIAS), np.float32(0.0))
        batch_mb[b] = np.ascontiguousarray(mbv.T).astype(np.float32)  # [128, 16]
        batch_kq[b] = (np.arange(S).reshape(4, 512) < lengths[b]).astype(np.float32)

    in_maps = []
    for c in range(N_CORES):
        g, hq = c // 4, c % 4
        f0 = hq * 256
        m = {
            "wqT": np.ascontiguousarray(WqT[:, f0:f0 + 256]),
            "wkT": np.ascontiguousarray(WkT[:, f0:f0 + 256]),
            "wvT": np.ascontiguousarray(WvT[:, f0:f0 + 256]),
            "woT": np.ascontiguousarray(WoT[f0:f0 + 256, :]),
        }
        for sl in range(2):
            b = assign[g][sl]
            m[f"qT{sl}"] = batch_qT[b]
            m[f"kT{sl}"] = batch_kT[b]
            m[f"vT{sl}"] = batch_vT[b]
            m[f"mb{sl}"] = batch_mb[b]
            m[f"kq{sl}"] = batch_kq[b]
        in_maps.append(m)

    res = run_bass_kernel_spmd(nc, in_maps, list(range(N_CORES)))

    # --- gather: sum 4 head-quad partials per batch, transpose ---
    out = np.zeros((S, B, H), dtype=np.float32)
    for g in range(2):
        for sl in range(2):
            b = assign[g][sl]
            acc = np.zeros((H, S), dtype=np.float32)
            for hq in range(4):
                c = g * 4 + hq
                acc += res.results[c][f"y{sl}"].astype(np.float32)
            out[:, b, :] = acc.T
    return out


# revision 36
# speedup vs baseline: 1.1971x; 1.0018x over previous
"""Multi-head attention (S=2048, B=4, H=1024, NH=16) on 8 Trainium2 NeuronCores.

Sharding: each core handles 2 batches x 4 heads (batch pairs balanced by
valid length; tensor-parallel over heads). Within a core everything is bf16
matmul / fp32 accumulate:
  1. q,k projected d-major (qT/kT: [dims, seq]), v seq-major ([seq, dims])
  2. scoresT[k,q] per head-pair via row-tiled matmuls (row_grp concurrency)
  3. mask+scale+exp fused on ScalarE (per-partition bias; PAD keys -> exp 0)
  4. PV col-tiled accumulates attnT; Z row-sums via all-ones stationary
     matmul (col_grp-concurrent with PV)
  5. attnT normalized by 1/Z; keepq applied before the Wo output multiply
  6. Wo projection -> yT partial [H, S]; host sums 4 partials/batch

Schedule: weight-chunk DMAs interleave with slot0's q/k input streams over
the three DMA-capable queues (sync/scalar/gpsimd), so the first projection
matmul issues ~2 transfers in; slot0 v inputs prefetch into dedicated tiles
(issue never blocks on tile recycling). Attention is emitted as a
software-pipelined kc-loop (scores(kc+1) before PV(kc)); the two scores
matmuls use 512-strided PSUM blocks so their row-group-concurrent drains hit
different banks (same-bank PE+PE drain is a fatal collision when trimmed),
with a strided-AP exp over both blocks. PV+Z pairs run col-group-concurrent.
Wo is deferred into per-ot units drained one-per-iteration of the NEXT
query-group (1-bank PSUM tag shared with projection units), so the output
chain never gates the scores/exp pipeline; the final group's units flush
alternating into the idle scores banks. Slot1's q/k/v projections become
deadline-scheduled units: v/k spread through attention0, q half-a-group
early in attention1. The last q-chunk of each slot is width-trimmed to the
valid length (multiple of 32), shrinking exp/matmul/DVE/DMA work on padded
queries.
"""
import sys

if "/opt/trn_rl_repo" not in sys.path:
    sys.path.insert(0, "/opt/trn_rl_repo")

import math
from itertools import permutations

import ml_dtypes
import numpy as np

import concourse.bass as bass
import concourse.mybir as mybir
import concourse.tile as tile
from concourse import bacc
from concourse.bass_utils import run_bass_kernel_spmd

S, B, H, NH, DK = 2048, 4, 1024, 16, 64
N_CORES = 8
BF16 = mybir.dt.bfloat16
F32 = mybir.dt.float32
NPBF16 = ml_dtypes.bfloat16
MASK_BIAS = -30000.0

_prog_cache: dict = {}


def _build_program(NQ, NK, VQ):
    """One SPMD program. Per batch-slot s: NQ[s] 512-wide q chunks (last one
    VQ[s] wide), NK[s] 128-wide k chunks. Slot 0 is the smaller workload."""
    NSCK = [(nk * 128 + 511) // 512 for nk in NK]
    KW = [nk * 128 for nk in NK]                    # k/v valid width
    QW = [(NQ[s] - 1) * 512 + VQ[s] for s in range(2)]  # q valid width
    nc = bacc.Bacc("TRN2", target_bir_lowering=False, debug=False,
                   num_devices=N_CORES)

    d_in = {}
    for s in range(2):
        d_in[f"qT{s}"] = nc.dram_tensor(f"qT{s}", [H, S], BF16, kind="ExternalInput")
        d_in[f"kT{s}"] = nc.dram_tensor(f"kT{s}", [H, S], BF16, kind="ExternalInput")
        d_in[f"vT{s}"] = nc.dram_tensor(f"vT{s}", [H, S], BF16, kind="ExternalInput")
        d_in[f"mb{s}"] = nc.dram_tensor(f"mb{s}", [128, 16], F32, kind="ExternalInput")
        d_in[f"kq{s}"] = nc.dram_tensor(f"kq{s}", [4, 512], F32, kind="ExternalInput")
    d_in["wqT"] = nc.dram_tensor("wqT", [H, 256], BF16, kind="ExternalInput")
    d_in["wkT"] = nc.dram_tensor("wkT", [H, 256], BF16, kind="ExternalInput")
    d_in["wvT"] = nc.dram_tensor("wvT", [H, 256], BF16, kind="ExternalInput")
    d_in["woT"] = nc.dram_tensor("woT", [256, H], BF16, kind="ExternalInput")
    d_out = [nc.dram_tensor(f"y{s}", [H, S], BF16, kind="ExternalOutput")
             for s in range(2)]

    def qcw(s, sc):
        return 512 if sc < NQ[s] - 1 else VQ[s]

    def kcw(s, sc):
        return min(512, KW[s] - sc * 512)

    with tile.TileContext(nc) as tc:
        with tc.tile_pool(name="wpool", bufs=1) as wpool, \
             tc.tile_pool(name="inp", bufs=6) as inp, \
             tc.tile_pool(name="in8", bufs=1) as in8, \
             tc.tile_pool(name="persist", bufs=1) as persist, \
             tc.tile_pool(name="probs", bufs=3) as probsp, \
             tc.tile_pool(name="small", bufs=2) as small, \
             tc.tile_pool(name="att", bufs=3) as attp, \
             tc.tile_pool(name="yst", bufs=3) as ystp:

            # --- weights: consolidated DMAs on the sync queue ---
            # w*_all[p, ic*256 + j] = w*T[ic*128 + p, j]
            wq_all = wpool.tile([128, 2048], BF16, name="wq_all", tag="wq")
            wk_all = wpool.tile([128, 2048], BF16, name="wk_all", tag="wk")
            wv_all = wpool.tile([128, 2048], BF16, name="wv_all", tag="wv")
            # wo_all[p, j*1024 + c] = woT[j*128 + p, c]
            wo_all = wpool.tile([128, 2048], BF16, name="wo_all", tag="wo")
            _wseen = set()

            def wqkv(t, ic, ft):
                return t[:, ic * 256 + ft * 128: ic * 256 + (ft + 1) * 128]

            mb = [wpool.tile([128, 16], F32, name=f"mbt{s}", tag=f"mbt{s}")
                  for s in range(2)]

            def emit_wo_mb():
                for j in range(2):
                    nc.sync.dma_start(
                        out=wo_all[:, j * 1024:(j + 1) * 1024],
                        in_=d_in["woT"].ap()[j * 128:(j + 1) * 128, :])
                for s in range(2):
                    nc.sync.dma_start(out=mb[s][:], in_=d_in[f"mb{s}"].ap())
            ones = wpool.tile([128, 64], BF16, name="ones", tag="ones")
            nc.vector.memset(ones[:], 1.0)

            # --- persistent projection outputs ---
            qTp = [[persist.tile([128, NQ[s] * 512], BF16, name=f"qTp{s}_{p}",
                                 tag=f"qTp{s}_{p}")
                    for p in range(2)] for s in range(2)]
            kTp = [[persist.tile([128, NSCK[s] * 512], BF16, name=f"kTp{s}_{p}",
                                 tag=f"kTp{s}_{p}")
                    for p in range(2)] for s in range(2)]
            vp = [[persist.tile([128, 256], BF16, name=f"vp{s}_{st}", tag=f"vp{s}_{st}")
                   for st in range(NK[s])] for s in range(2)]

            # DMA queue rotation for input streams (keep Scalar clean once
            # attention starts; Sync carries the weights up front)
            s0_queues = [nc.scalar, nc.gpsimd, nc.sync]
            s1_queues = [nc.sync, nc.gpsimd]
            _qi = [0]

            def dma_rot(queues, out, in_):
                q = queues[_qi[0] % len(queues)]
                _qi[0] += 1
                q.dma_start(out=out, in_=in_)

            def emit_w(t, dname, ic):
                # interleave weight-chunk DMAs with the input stream so the
                # first matmuls aren't gated on the full weight load
                if (dname, ic) in _wseen:
                    return
                _wseen.add((dname, ic))
                dma_rot(s0_queues, t[:, ic * 256:(ic + 1) * 256],
                        d_in[dname].ap()[ic * 128:(ic + 1) * 128, :])

            def emit_proj_streamed(s, pool):
                """ic-outer projections with streamed inputs (slot 0)."""
                for kind, wts, dname, nsc, outtiles, cw in (
                        ("q", wq_all, f"qT{s}", NQ[s], qTp[s], qcw),
                        ("k", wk_all, f"kT{s}", NSCK[s], kTp[s], kcw)):
                    wname = "wqT" if kind == "q" else "wkT"
                    ps = [[pool.tile([128, 512], F32,
                                     name=f"pj{kind}{s}_{ft}_{sc}",
                                     tag=f"pj_{ft}_{sc}")
                           for sc in range(nsc)] for ft in range(2)]
                    tw = sum(cw(s, sc) for sc in range(nsc))
                    for ic in range(8):
                        emit_w(wts, wname, ic)
                        it = inp.tile([128, 2048], BF16,
                                      name=f"in{kind}{s}_{ic}", tag="inp")
                        dma_rot(s0_queues,
                                it[:, 0:tw],
                                d_in[dname].ap()[ic * 128:(ic + 1) * 128, 0:tw])
                        for ft in range(2):
                            for sc in range(nsc):
                                w = cw(s, sc)
                                nc.tensor.matmul(
                                    out=ps[ft][sc][:, 0:w],
                                    lhsT=wqkv(wts, ic, ft),
                                    rhs=it[:, sc * 512: sc * 512 + w],
                                    start=(ic == 0), stop=(ic == 7))
                    for ft in range(2):
                        for sc in range(nsc):
                            w = cw(s, sc)
                            if kind == "q":
                                nc.vector.tensor_copy(
                                    outtiles[ft][:, sc * 512: sc * 512 + w],
                                    ps[ft][sc][:, 0:w])
                            else:
                                nc.scalar.copy(
                                    outtiles[ft][:, sc * 512: sc * 512 + w],
                                    ps[ft][sc][:, 0:w])

            def emit_v_prefetch0():
                tiles = []
                for ic in range(8):
                    emit_w(wv_all, "wvT", ic)
                    it = in8.tile([128, KW[0]], BF16, name=f"v0in_{ic}",
                                  tag=f"v0in{ic}")
                    dma_rot(s0_queues, it[:],
                            d_in["vT0"].ap()[ic * 128:(ic + 1) * 128, 0:KW[0]])
                    tiles.append(it)
                return tiles

            def emit_v_resident(s, pool, tiles):
                for st0 in range(0, NK[s], 8):
                    sts = range(st0, min(st0 + 8, NK[s]))
                    psv = {st: pool.tile([128, 256], F32, name=f"pjv{s}_{st}",
                                         tag=f"pj_{(st - st0) // 4}_{(st - st0) % 4}")
                           for st in sts}
                    for ic in range(8):
                        for st in sts:
                            nc.tensor.matmul(
                                out=psv[st][:],
                                lhsT=tiles[ic][:, st * 128:(st + 1) * 128],
                                rhs=wv_all[:, ic * 256: ic * 256 + 256],
                                start=(ic == 0), stop=(ic == 7))
                    for st in sts:
                        if st % 2:
                            nc.scalar.copy(vp[s][st][:], psv[st][:])
                        else:
                            nc.vector.tensor_copy(vp[s][st][:], psv[st][:])

            def emit_prefetch(s):
                """Issue all of slot s's input DMAs into dedicated tiles."""
                tiles = {}
                for kind, dname, w in (("v", f"vT{s}", KW[s]),
                                       ("k", f"kT{s}", KW[s]),
                                       ("q", f"qT{s}", QW[s])):
                    for ic in range(8):
                        it = in8.tile([128, w], BF16, name=f"pf{kind}{s}_{ic}",
                                      tag=f"pf{kind}{ic}")
                        dma_rot(s1_queues,
                                it[:],
                                d_in[dname].ap()[ic * 128:(ic + 1) * 128, 0:w])
                        tiles[(kind, ic)] = it
                return tiles

            def proj_units(s, pool, tiles, nkit, kinds=("v", "k", "q")):
                """Deadline-tagged projection units for slot s, consumed by
                interleaving into the attention kc-loop. Deadline = global
                iteration index ((qc*2)+p)*NK + kc of first use."""
                units = []

                def q_unit(ft, sc):
                    def emit():
                        w = qcw(s, sc)
                        pj = pool.tile([128, 512], F32,
                                       name=f"rpjq{s}_{ft}_{sc}", tag="y")
                        for ic in range(8):
                            nc.tensor.matmul(
                                out=pj[:, 0:w],
                                lhsT=wqkv(wq_all, ic, ft),
                                rhs=tiles[("q", ic)][:, sc * 512: sc * 512 + w],
                                start=(ic == 0), stop=(ic == 7))
                        nc.vector.tensor_copy(
                            qTp[s][ft][:, sc * 512: sc * 512 + w], pj[:, 0:w])
                    return emit

                def k_unit(ft, sc):
                    def emit():
                        w = kcw(s, sc)
                        pj = pool.tile([128, 512], F32,
                                       name=f"rpjk{s}_{ft}_{sc}", tag="y")
                        for ic in range(8):
                            nc.tensor.matmul(
                                out=pj[:, 0:w],
                                lhsT=wqkv(wk_all, ic, ft),
                                rhs=tiles[("k", ic)][:, sc * 512: sc * 512 + w],
                                start=(ic == 0), stop=(ic == 7))
                        nc.vector.tensor_copy(
                            kTp[s][ft][:, sc * 512: sc * 512 + w], pj[:, 0:w])
                    return emit

                def v_unit(st):
                    def emit():
                        pj = pool.tile([128, 512], F32,
                                       name=f"rpjv{s}_{st}", tag="y")
                        for ic in range(8):
                            nc.tensor.matmul(
                                out=pj[:, 0:256],
                                lhsT=tiles[("v", ic)][:, st * 128:(st + 1) * 128],
                                rhs=wv_all[:, ic * 256: ic * 256 + 256],
                                start=(ic == 0), stop=(ic == 7))
                        nc.vector.tensor_copy(vp[s][st][:], pj[:, 0:256])
                    return emit

                vk = []
                if "v" in kinds:
                    for st in range(NK[s]):
                        vk.append(v_unit(st))
                if "k" in kinds:
                    for ft in range(2):
                        for sc in range(NSCK[s]):
                            vk.append(k_unit(ft, sc))
                if "q" in kinds:
                    for ft in range(2):
                        for sc in range(NQ[s]):
                            units.append((max(0, (sc * 2 + ft) * nkit - nkit // 2),
                                          q_unit(ft, sc)))
                units.sort(key=lambda u: u[0])
                return vk, units

            def emit_attention(s, psc, pat, pz, pwo, units=None, carry_wo=None,
                               lead=4):
                units = list(units) if units else []
                wo_pending = list(carry_wo) if carry_wo else []
                nkit = NK[s]

                def drain_units(cur_idx, pre_group=False):
                    # hard deadlines only: consumers of these units follow
                    # in the in-order PE stream
                    while units and units[0][0] <= cur_idx:
                        units.pop(0)[1]()

                def drain_early(cur_idx):
                    # opportunistic emission after the iteration's PV/Z, so
                    # units never delay the scores->exp pipeline
                    n = 0
                    while units and n < 2 and units[0][0] <= cur_idx + lead:
                        units.pop(0)[1]()
                        n += 1
                    m = 0
                    while wo_pending and wo_pending[0][0] <= cur_idx and m < 2:
                        wo_pending.pop(0)[1](None)
                        m += 1

                def make_wo_unit(qc, ot, ab_pair, W):
                    def emit(tag):
                        if tag is None:
                            yps = pwo.tile([128, 512], F32,
                                           name=f"yp{s}_{qc}_{ot}", tag="y")
                            ypv = yps[:, 0:W]
                        else:
                            yps = psc.tile([128, 1024], F32,
                                           name=f"yp{s}_{qc}_{ot}", tag="sc")
                            ypv = yps[:, 0:W]
                        for j in range(2):
                            nc.tensor.matmul(
                                out=ypv,
                                lhsT=wo_all[:, j * 1024 + ot * 128: j * 1024 + (ot + 1) * 128],
                                rhs=ab_pair[j][:, 0:W],
                                start=(j == 0), stop=(j == 1))
                        ysb = ystp.tile([128, 512], BF16,
                                        name=f"ysb{s}_{qc}_{ot}", tag="ysb")
                        if tag is not None:
                            nc.scalar.copy(ysb[:, 0:W], ypv)
                        else:
                            nc.vector.tensor_copy(ysb[:, 0:W], ypv)
                        (nc.gpsimd if ot % 2 else nc.sync).dma_start(
                            out=d_out[s].ap()[ot * 128:(ot + 1) * 128,
                                              qc * 512: qc * 512 + W],
                            in_=ysb[:, 0:W])
                    return emit

                kqr_t = {}

                def emit_kqr(qc):
                    w = qcw(s, qc)
                    t = small.tile([128, 512], F32, name=f"kqr{s}_{qc}",
                                   tag="kqr")
                    nc.gpsimd.dma_start(
                        out=t[:, 0:w],
                        in_=bass.AP(tensor=d_in[f"kq{s}"], offset=qc * 512,
                                    ap=[[0, 128], [1, w]]))
                    kqr_t[qc] = t

                emit_kqr(0)

                def emit_scores_g(qc, p, kc, W):
                    sc_ps = psc.tile([128, 1024], F32,
                                     name=f"s{s}_{qc}_{p}_{kc}", tag="sc")
                    pr = probsp.tile([128, 1024], BF16,
                                     name=f"pr{s}_{qc}_{p}_{kc}", tag="pr")
                    for hh in range(2):
                        hsl = slice(hh * 64, hh * 64 + 64)
                        # 512-strided blocks: the two row-group-concurrent
                        # matmuls must drain to different PSUM banks
                        nc.tensor.matmul(
                            out=sc_ps[:, hh * 512: hh * 512 + W],
                            lhsT=kTp[s][p][hsl, kc * 128:(kc + 1) * 128],
                            rhs=qTp[s][p][hsl, qc * 512: qc * 512 + W],
                            start=True, stop=True)
                    if W == 512:
                        exp_in = sc_ps[:, 0:1024]
                        exp_out = pr[:, 0:1024]
                    else:
                        exp_in = sc_ps[:].rearrange(
                            "p (b w) -> p b w", b=2)[:, :, 0:W]
                        exp_out = pr[:, 0:2 * W].rearrange(
                            "p (b w) -> p b w", b=2)
                    nc.scalar.activation(
                        out=exp_out, in_=exp_in,
                        func=mybir.ActivationFunctionType.Exp,
                        bias=mb[s][:, kc:kc + 1],
                        scale=1.0 / math.sqrt(DK))
                    return pr

                # groups software-pipelined ACROSS boundaries: the next
                # group's first scores/exp is emitted during the current
                # group's last iteration, so ScalarE never refills cold
                groups = [(qc, p) for qc in range(NQ[s]) for p in range(2)]
                att_sb = []
                drain_units(0)
                pr_next = emit_scores_g(0, 0, 0, qcw(s, 0))
                for gi, (qc, p) in enumerate(groups):
                    W = qcw(s, qc)
                    base_idx = gi * nkit
                    if p == 0:
                        if qc + 1 < NQ[s]:
                            emit_kqr(qc + 1)
                        att_sb = []
                    kqr = kqr_t[qc]
                    drain_units(base_idx)
                    attn = pat.tile([128, 512], F32,
                                    name=f"at{s}_{qc}_{p}", tag="at")
                    zps = pz.tile([128, 512], F32,
                                  name=f"z{s}_{qc}_{p}", tag="z")
                    for kc in range(nkit):
                        drain_units(base_idx + kc)
                        first, last = kc == 0, kc == nkit - 1
                        pr = pr_next
                        if not last:
                            pr_next = emit_scores_g(qc, p, kc + 1, W)
                        elif gi + 1 < len(groups):
                            nqc, npp = groups[gi + 1]
                            pr_next = emit_scores_g(nqc, npp, 0, qcw(s, nqc))
                        for hh in range(2):
                            hsl = slice(hh * 64, hh * 64 + 64)
                            nc.tensor.matmul(
                                out=attn[hsl, 0:W],
                                lhsT=vp[s][kc][:, p * 128 + hh * 64:p * 128 + (hh + 1) * 64],
                                rhs=pr[:, hh * W:(hh + 1) * W],
                                start=first, stop=last)
                            nc.tensor.matmul(
                                out=zps[hsl, 0:W],
                                lhsT=ones[:, :], rhs=pr[:, hh * W:(hh + 1) * W],
                                start=first, stop=last)
                        drain_early(base_idx + kc)
                    # normalize: attn * keepq / Z
                    rz = small.tile([128, 512], F32,
                                    name=f"rz{s}_{qc}_{p}", tag="rz")
                    nc.vector.reciprocal_approx_fast(out=rz[:, 0:W],
                                                     in_=zps[:, 0:W])
                    nc.vector.tensor_mul(rz[:, 0:W], rz[:, 0:W], kqr[:, 0:W])
                    ab = attp.tile([128, 512], BF16,
                                   name=f"ab{s}_{qc}_{p}", tag=f"ab{p}")
                    nc.vector.tensor_mul(ab[:, 0:W], attn[:, 0:W], rz[:, 0:W])
                    att_sb.append(ab)
                    if p == 1:
                        kqr_t.pop(qc)
                        # Wo deferred: one unit per iteration of the NEXT
                        # group, never gating the scores/exp pipeline
                        for ot in range(8):
                            wo_pending.append(((qc + 1) * 2 * nkit + 1 + 2 * ot,
                                               make_wo_unit(qc, ot, att_sb, W)))
                # flush stragglers; alternate between the 'y' bank and the
                # now-idle scores slots so the tail chain pipelines
                while units:
                    units.pop(0)[1]()
                return wo_pending

            # slot 0 projections use the full PSUM (released afterwards)
            with tc.tile_pool(name="pproj", bufs=1, space="PSUM") as pproj:
                emit_proj_streamed(0, pproj)
                v0tiles = emit_v_prefetch0()
                emit_v_resident(0, pproj, v0tiles)
            emit_wo_mb()
            tiles1 = emit_prefetch(1)
            with tc.tile_pool(name="psc", bufs=2, space="PSUM") as psc, \
                 tc.tile_pool(name="pat", bufs=2, space="PSUM") as pat, \
                 tc.tile_pool(name="pz", bufs=1, space="PSUM") as pz, \
                 tc.tile_pool(name="pwo", bufs=1, space="PSUM") as pwo:
                vk1, units1 = proj_units(1, pwo, tiles1, NK[1])
                n0 = NQ[0] * 2 * NK[0]
                sp = max(1, (n0 - 6) // max(1, len(vk1)))
                units0 = [(2 + sp * i, fn) for i, fn in enumerate(vk1)]
                left0 = emit_attention(0, psc, pat, pz, pwo, units=units0)
                carry = [(i + 1, fn) for i, (_, fn) in enumerate(left0)]
                left1 = emit_attention(1, psc, pat, pz, pwo, units=units1,
                                       carry_wo=carry)
                for i, (_, fn) in enumerate(left1):
                    fn("sc" if i % 2 == 0 else None)
    nc.compile()
    return nc


def _get_program(NQ, NK, VQ):
    key = (tuple(NQ), tuple(NK), tuple(VQ))
    if key not in _prog_cache:
        _prog_cache[key] = _build_program(list(NQ), list(NK), list(VQ))
    return _prog_cache[key]


def kernel(value, key, query, padding_mask, Wq, Wk, Wv, Wo):
    value = np.asarray(value)
    key = np.asarray(key)
    query = np.asarray(query)
    padding_mask = np.asarray(padding_mask)
    Wq, Wk, Wv, Wo = (np.asarray(a) for a in (Wq, Wk, Wv, Wo))

    lengths = (~padding_mask).sum(axis=0).astype(int)  # (B,)

    # --- batch pairing: assign batches to (group, slot) minimizing baked work ---
    def slot_counts(assign):
        nq = [max((int(lengths[assign[g][sl]]) + 511) // 512 for g in range(2))
              for sl in range(2)]
        nk = [max((int(lengths[assign[g][sl]]) + 127) // 128 for g in range(2))
              for sl in range(2)]
        return nq, nk

    best = None
    for perm in permutations(range(B)):
        a = ((perm[0], perm[1]), (perm[2], perm[3]))
        nq, nk = slot_counts(a)
        c = nq[0] * nk[0] + nq[1] * nk[1]
        if best is None or c < best[0]:
            best = (c, a)
    assign = best[1]
    nq, nk = slot_counts(assign)
    # slot 0 should be the smaller workload (its projections can't overlap)
    if nq[0] * nk[0] > nq[1] * nk[1]:
        assign = tuple((g[1], g[0]) for g in assign)
        nq, nk = slot_counts(assign)
    NQ, NK = nq, nk
    # trimmed width of the last q chunk per slot (multiple of 32)
    VQ = []
    for sl in range(2):
        maxlen = max(int(lengths[assign[g][sl]]) for g in range(2))
        v = maxlen - (NQ[sl] - 1) * 512
        VQ.append(min(512, (v + 31) // 32 * 32))

    nc = _get_program(NQ, NK, VQ)

    # --- per-core inputs ---
    WqT = np.ascontiguousarray(Wq.T).astype(NPBF16)
    WkT = np.ascontiguousarray(Wk.T).astype(NPBF16)
    WvT = np.ascontiguousarray(Wv.T).astype(NPBF16)
    WoT = np.ascontiguousarray(Wo.T).astype(NPBF16)

    batch_qT, batch_kT, batch_vT, batch_mb, batch_kq = {}, {}, {}, {}, {}
    for b in range(B):
        batch_qT[b] = np.ascontiguousarray(query[:, b, :].T).astype(NPBF16)
        batch_kT[b] = np.ascontiguousarray(key[:, b, :].T).astype(NPBF16)
        batch_vT[b] = np.ascontiguousarray(value[:, b, :].T).astype(NPBF16)
        kpos = np.arange(S).reshape(16, 128)  # [kchunk, kpos]
        mbv = np.where(kpos >= lengths[b], np.float32(MASK_BIAS), np.float32(0.0))
        batch_mb[b] = np.ascontiguousarray(mbv.T).astype(np.float32)  # [128, 16]
        batch_kq[b] = (np.arange(S).reshape(4, 512) < lengths[b]).astype(np.float32)

    in_maps = []
    for c in range(N_CORES):
        g, hq = c // 4, c % 4
        f0 = hq * 256
        m = {
            "wqT": np.ascontiguousarray(WqT[:, f0:f0 + 256]),
            "wkT": np.ascontiguousarray(WkT[:, f0:f0 + 256]),
            "wvT": np.ascontiguousarray(WvT[:, f0:f0 + 256]),
            "woT": np.ascontiguousarray(WoT[f0:f0 + 256, :]),
        }
        for sl in range(2):
            b = assign[g][sl]
            m[f"qT{sl}"] = batch_qT[b]
            m[f"kT{sl}"] = batch_kT[b]
            m[f"vT{sl}"] = batch_vT[b]
            m[f"mb{sl}"] = batch_mb[b]
            m[f"kq{sl}"] = batch_kq[b]
        in_maps.append(m)

    res = run_bass_kernel_spmd(nc, in_maps, list(range(N_CORES)))

    # --- gather: sum 4 head-quad partials per batch, transpose ---
    out = np.zeros((S, B, H), dtype=np.float32)
    for g in range(2):
        for sl in range(2):
            b = assign[g][sl]
            acc = np.zeros((H, S), dtype=np.float32)
            for hq in range(4):
                c = g * 4 + hq
                acc += res.results[c][f"y{sl}"].astype(np.float32)
            out[:, b, :] = acc.T
    return out


# revision 37
# speedup vs baseline: 1.2733x; 1.0636x over previous
"""Multi-head attention (S=2048, B=4, H=1024, NH=16) on 8 Trainium2 NeuronCores.

Sharding: each core handles 2 batches x 4 heads (batch pairs balanced by
valid length; tensor-parallel over heads). Within a core everything is bf16
matmul / fp32 accumulate:
  1. q,k projected d-major (qT/kT: [dims, seq]), v seq-major ([seq, dims])
  2. scoresT[k,q] per head-pair via row-tiled matmuls (row_grp concurrency)
  3. mask+scale+exp fused on ScalarE (per-partition bias; PAD keys -> exp 0)
  4. PV col-tiled accumulates attnT; Z row-sums via all-ones stationary
     matmul (col_grp-concurrent with PV)
  5. attnT normalized by 1/Z; keepq applied before the Wo output multiply
  6. Wo projection -> yT partial [H, S]; host sums 4 partials/batch

Schedule: weight-chunk DMAs interleave with slot0's q/k input streams over
the three DMA-capable queues (sync/scalar/gpsimd), so the first projection
matmul issues ~2 transfers in; slot0 v inputs prefetch into dedicated tiles
(issue never blocks on tile recycling). Attention is emitted as a
software-pipelined kc-loop (scores(kc+1) before PV(kc)); the two scores
matmuls use 512-strided PSUM blocks so their row-group-concurrent drains hit
different banks (same-bank PE+PE drain is a fatal collision when trimmed),
with a strided-AP exp over both blocks. PV+Z pairs run col-group-concurrent.
Wo is deferred into per-ot units drained one-per-iteration of the NEXT
query-group (1-bank PSUM tag shared with projection units), so the output
chain never gates the scores/exp pipeline; the final group's units flush
alternating into the idle scores banks. Slot1's q/k/v projections become
deadline-scheduled units: v/k spread through attention0, q half-a-group
early in attention1. The last q-chunk of each slot is width-trimmed to the
valid length (multiple of 32), shrinking exp/matmul/DVE/DMA work on padded
queries.
"""
import sys

if "/opt/trn_rl_repo" not in sys.path:
    sys.path.insert(0, "/opt/trn_rl_repo")

import math
from itertools import permutations

import ml_dtypes
import numpy as np

import concourse.bass as bass
import concourse.mybir as mybir
import concourse.tile as tile
from concourse import bacc
from concourse.bass_utils import run_bass_kernel_spmd

S, B, H, NH, DK = 2048, 4, 1024, 16, 64
N_CORES = 8
BF16 = mybir.dt.bfloat16
F32 = mybir.dt.float32
NPBF16 = ml_dtypes.bfloat16
MASK_BIAS = -30000.0

_prog_cache: dict = {}


def _build_program(NQ, NK, VQ):
    """One SPMD program. Per batch-slot s: NQ[s] 512-wide q chunks (last one
    VQ[s] wide), NK[s] 128-wide k chunks. Slot 0 is the smaller workload."""
    NSCK = [(nk * 128 + 511) // 512 for nk in NK]
    KW = [nk * 128 for nk in NK]                    # k/v valid width
    QW = [(NQ[s] - 1) * 512 + VQ[s] for s in range(2)]  # q valid width
    nc = bacc.Bacc("TRN2", target_bir_lowering=False, debug=False,
                   num_devices=N_CORES)

    d_in = {}
    for s in range(2):
        d_in[f"qT{s}"] = nc.dram_tensor(f"qT{s}", [H, S], BF16, kind="ExternalInput")
        d_in[f"kT{s}"] = nc.dram_tensor(f"kT{s}", [H, S], BF16, kind="ExternalInput")
        d_in[f"vT{s}"] = nc.dram_tensor(f"vT{s}", [H, S], BF16, kind="ExternalInput")
        d_in[f"mb{s}"] = nc.dram_tensor(f"mb{s}", [128, 16], F32, kind="ExternalInput")
        d_in[f"kq{s}"] = nc.dram_tensor(f"kq{s}", [4, 512], F32, kind="ExternalInput")
    d_in["wqT"] = nc.dram_tensor("wqT", [H, 256], BF16, kind="ExternalInput")
    d_in["wkT"] = nc.dram_tensor("wkT", [H, 256], BF16, kind="ExternalInput")
    d_in["wvT"] = nc.dram_tensor("wvT", [H, 256], BF16, kind="ExternalInput")
    d_in["woT"] = nc.dram_tensor("woT", [256, H], BF16, kind="ExternalInput")
    d_out = [nc.dram_tensor(f"y{s}", [H, S], BF16, kind="ExternalOutput")
             for s in range(2)]

    def qcw(s, sc):
        return 512 if sc < NQ[s] - 1 else VQ[s]

    def kcw(s, sc):
        return min(512, KW[s] - sc * 512)

    with tile.TileContext(nc) as tc:
        with tc.tile_pool(name="wpool", bufs=1) as wpool, \
             tc.tile_pool(name="inp", bufs=6) as inp, \
             tc.tile_pool(name="in8", bufs=1) as in8, \
             tc.tile_pool(name="persist", bufs=1) as persist, \
             tc.tile_pool(name="probs", bufs=3) as probsp, \
             tc.tile_pool(name="small", bufs=2) as small, \
             tc.tile_pool(name="att", bufs=3) as attp, \
             tc.tile_pool(name="yst", bufs=3) as ystp:

            # --- weights: consolidated DMAs on the sync queue ---
            # w*_all[p, ic*256 + j] = w*T[ic*128 + p, j]
            wq_all = wpool.tile([128, 2048], BF16, name="wq_all", tag="wq")
            wk_all = wpool.tile([128, 2048], BF16, name="wk_all", tag="wk")
            wv_all = wpool.tile([128, 2048], BF16, name="wv_all", tag="wv")
            # wo_all[p, j*1024 + c] = woT[j*128 + p, c]
            wo_all = wpool.tile([128, 2048], BF16, name="wo_all", tag="wo")
            _wseen = set()

            def wqkv(t, ic, ft):
                return t[:, ic * 256 + ft * 128: ic * 256 + (ft + 1) * 128]

            mb = [wpool.tile([128, 16], F32, name=f"mbt{s}", tag=f"mbt{s}")
                  for s in range(2)]

            def emit_wo_mb():
                for j in range(2):
                    nc.sync.dma_start(
                        out=wo_all[:, j * 1024:(j + 1) * 1024],
                        in_=d_in["woT"].ap()[j * 128:(j + 1) * 128, :])
                for s in range(2):
                    nc.sync.dma_start(out=mb[s][:], in_=d_in[f"mb{s}"].ap())
            ones = wpool.tile([128, 64], BF16, name="ones", tag="ones")
            nc.vector.memset(ones[:], 1.0)

            # --- persistent projection outputs ---
            qTp = [[persist.tile([128, NQ[s] * 512], BF16, name=f"qTp{s}_{p}",
                                 tag=f"qTp{s}_{p}")
                    for p in range(2)] for s in range(2)]
            kTp = [[persist.tile([128, NSCK[s] * 512], BF16, name=f"kTp{s}_{p}",
                                 tag=f"kTp{s}_{p}")
                    for p in range(2)] for s in range(2)]
            vp = [[persist.tile([128, 256], BF16, name=f"vp{s}_{st}", tag=f"vp{s}_{st}")
                   for st in range(NK[s])] for s in range(2)]

            # DMA queue rotation for input streams (keep Scalar clean once
            # attention starts; Sync carries the weights up front)
            s0_queues = [nc.scalar, nc.gpsimd, nc.sync]
            s1_queues = [nc.sync, nc.gpsimd]
            _qi = [0]

            def dma_rot(queues, out, in_):
                q = queues[_qi[0] % len(queues)]
                _qi[0] += 1
                q.dma_start(out=out, in_=in_)

            def emit_w(t, dname, ic):
                # interleave weight-chunk DMAs with the input stream so the
                # first matmuls aren't gated on the full weight load
                if (dname, ic) in _wseen:
                    return
                _wseen.add((dname, ic))
                dma_rot(s0_queues, t[:, ic * 256:(ic + 1) * 256],
                        d_in[dname].ap()[ic * 128:(ic + 1) * 128, :])

            def emit_proj_streamed(s, pool):
                """ic-outer projections with streamed inputs (slot 0)."""
                for kind, wts, dname, nsc, outtiles, cw in (
                        ("q", wq_all, f"qT{s}", NQ[s], qTp[s], qcw),
                        ("k", wk_all, f"kT{s}", NSCK[s], kTp[s], kcw)):
                    wname = "wqT" if kind == "q" else "wkT"
                    ps = [[pool.tile([128, 512], F32,
                                     name=f"pj{kind}{s}_{ft}_{sc}",
                                     tag=f"pj_{ft}_{sc}")
                           for sc in range(nsc)] for ft in range(2)]
                    tw = sum(cw(s, sc) for sc in range(nsc))
                    for ic in range(8):
                        emit_w(wts, wname, ic)
                        it = inp.tile([128, 2048], BF16,
                                      name=f"in{kind}{s}_{ic}", tag="inp")
                        dma_rot(s0_queues,
                                it[:, 0:tw],
                                d_in[dname].ap()[ic * 128:(ic + 1) * 128, 0:tw])
                        for ft in range(2):
                            for sc in range(nsc):
                                w = cw(s, sc)
                                nc.tensor.matmul(
                                    out=ps[ft][sc][:, 0:w],
                                    lhsT=wqkv(wts, ic, ft),
                                    rhs=it[:, sc * 512: sc * 512 + w],
                                    start=(ic == 0), stop=(ic == 7))
                    for ft in range(2):
                        for sc in range(nsc):
                            w = cw(s, sc)
                            if kind == "q":
                                nc.vector.tensor_copy(
                                    outtiles[ft][:, sc * 512: sc * 512 + w],
                                    ps[ft][sc][:, 0:w])
                            else:
                                nc.scalar.copy(
                                    outtiles[ft][:, sc * 512: sc * 512 + w],
                                    ps[ft][sc][:, 0:w])

            def emit_v_prefetch0():
                tiles = []
                for ic in range(8):
                    emit_w(wv_all, "wvT", ic)
                    it = in8.tile([128, KW[0]], BF16, name=f"v0in_{ic}",
                                  tag=f"v0in{ic}")
                    dma_rot(s0_queues, it[:],
                            d_in["vT0"].ap()[ic * 128:(ic + 1) * 128, 0:KW[0]])
                    tiles.append(it)
                return tiles

            def emit_v_resident(s, pool, tiles):
                for st0 in range(0, NK[s], 8):
                    sts = range(st0, min(st0 + 8, NK[s]))
                    psv = {st: pool.tile([128, 256], F32, name=f"pjv{s}_{st}",
                                         tag=f"pj_{(st - st0) // 4}_{(st - st0) % 4}")
                           for st in sts}
                    for ic in range(8):
                        for st in sts:
                            nc.tensor.matmul(
                                out=psv[st][:],
                                lhsT=tiles[ic][:, st * 128:(st + 1) * 128],
                                rhs=wv_all[:, ic * 256: ic * 256 + 256],
                                start=(ic == 0), stop=(ic == 7))
                    for st in sts:
                        if st % 2:
                            nc.scalar.copy(vp[s][st][:], psv[st][:])
                        else:
                            nc.vector.tensor_copy(vp[s][st][:], psv[st][:])

            def emit_prefetch(s):
                """Issue all of slot s's input DMAs into dedicated tiles."""
                tiles = {}
                for kind, dname, w in (("v", f"vT{s}", KW[s]),
                                       ("k", f"kT{s}", KW[s]),
                                       ("q", f"qT{s}", QW[s])):
                    for ic in range(8):
                        it = in8.tile([128, w], BF16, name=f"pf{kind}{s}_{ic}",
                                      tag=f"pf{kind}{ic}")
                        dma_rot(s1_queues,
                                it[:],
                                d_in[dname].ap()[ic * 128:(ic + 1) * 128, 0:w])
                        tiles[(kind, ic)] = it
                return tiles

            def proj_units(s, pool, tiles, nkit, kinds=("v", "k", "q")):
                """Deadline-tagged projection units for slot s, consumed by
                interleaving into the attention kc-loop. Deadline = global
                iteration index ((qc*2)+p)*NK + kc of first use."""
                units = []

                def q_unit(ft, sc):
                    def emit():
                        w = qcw(s, sc)
                        pj = pool.tile([128, 512], F32,
                                       name=f"rpjq{s}_{ft}_{sc}", tag="y")
                        for ic in range(8):
                            nc.tensor.matmul(
                                out=pj[:, 0:w],
                                lhsT=wqkv(wq_all, ic, ft),
                                rhs=tiles[("q", ic)][:, sc * 512: sc * 512 + w],
                                start=(ic == 0), stop=(ic == 7))
                        nc.vector.tensor_copy(
                            qTp[s][ft][:, sc * 512: sc * 512 + w], pj[:, 0:w])
                    return emit

                def k_unit(ft, sc):
                    def emit():
                        w = kcw(s, sc)
                        pj = pool.tile([128, 512], F32,
                                       name=f"rpjk{s}_{ft}_{sc}", tag="y")
                        for ic in range(8):
                            nc.tensor.matmul(
                                out=pj[:, 0:w],
                                lhsT=wqkv(wk_all, ic, ft),
                                rhs=tiles[("k", ic)][:, sc * 512: sc * 512 + w],
                                start=(ic == 0), stop=(ic == 7))
                        nc.vector.tensor_copy(
                            kTp[s][ft][:, sc * 512: sc * 512 + w], pj[:, 0:w])
                    return emit

                def v_unit(st):
                    def emit():
                        pj = pool.tile([128, 512], F32,
                                       name=f"rpjv{s}_{st}", tag="y")
                        for ic in range(8):
                            nc.tensor.matmul(
                                out=pj[:, 0:256],
                                lhsT=tiles[("v", ic)][:, st * 128:(st + 1) * 128],
                                rhs=wv_all[:, ic * 256: ic * 256 + 256],
                                start=(ic == 0), stop=(ic == 7))
                        nc.vector.tensor_copy(vp[s][st][:], pj[:, 0:256])
                    return emit

                vk = []
                if "v" in kinds:
                    for st in range(NK[s]):
                        vk.append(v_unit(st))
                if "k" in kinds:
                    for ft in range(2):
                        for sc in range(NSCK[s]):
                            vk.append(k_unit(ft, sc))
                if "q" in kinds:
                    for ft in range(2):
                        for sc in range(NQ[s]):
                            units.append((max(0, (sc * 2 + ft) * nkit - nkit // 2),
                                          q_unit(ft, sc)))
                units.sort(key=lambda u: u[0])
                return vk, units

            def emit_attention(s, psc, pat, pz, pwo, units=None, carry_wo=None,
                               lead=4):
                units = list(units) if units else []
                wo_pending = list(carry_wo) if carry_wo else []
                nkit = NK[s]

                def drain_units(cur_idx, pre_group=False):
                    # hard deadlines only: consumers of these units follow
                    # in the in-order PE stream
                    while units and units[0][0] <= cur_idx:
                        units.pop(0)[1]()

                def drain_early(cur_idx):
                    # opportunistic emission after the iteration's PV/Z, so
                    # units never delay the scores->exp pipeline
                    n = 0
                    while units and n < 2 and units[0][0] <= cur_idx + lead:
                        units.pop(0)[1]()
                        n += 1
                    m = 0
                    while wo_pending and wo_pending[0][0] <= cur_idx and m < 2:
                        wo_pending.pop(0)[1](None)
                        m += 1

                def make_wo_unit(qc, ot, ab_pair, W):
                    def emit(tag):
                        if tag is None:
                            yps = pwo.tile([128, 512], F32,
                                           name=f"yp{s}_{qc}_{ot}", tag="y")
                            ypv = yps[:, 0:W]
                        else:
                            yps = psc.tile([128, 1024], F32,
                                           name=f"yp{s}_{qc}_{ot}", tag="sc")
                            ypv = yps[:, 0:W]
                        for j in range(2):
                            nc.tensor.matmul(
                                out=ypv,
                                lhsT=wo_all[:, j * 1024 + ot * 128: j * 1024 + (ot + 1) * 128],
                                rhs=ab_pair[j][:, 0:W],
                                start=(j == 0), stop=(j == 1))
                        ysb = ystp.tile([128, 512], BF16,
                                        name=f"ysb{s}_{qc}_{ot}", tag="ysb")
                        if tag is not None:
                            nc.scalar.copy(ysb[:, 0:W], ypv)
                        else:
                            nc.vector.tensor_copy(ysb[:, 0:W], ypv)
                        (nc.gpsimd if ot % 2 else nc.sync).dma_start(
                            out=d_out[s].ap()[ot * 128:(ot + 1) * 128,
                                              qc * 512: qc * 512 + W],
                            in_=ysb[:, 0:W])
                    return emit

                kqr_t = {}

                def emit_kqr(qc):
                    w = qcw(s, qc)
                    t = small.tile([128, 512], F32, name=f"kqr{s}_{qc}",
                                   tag="kqr")
                    nc.gpsimd.dma_start(
                        out=t[:, 0:w],
                        in_=bass.AP(tensor=d_in[f"kq{s}"], offset=qc * 512,
                                    ap=[[0, 128], [1, w]]))
                    kqr_t[qc] = t

                emit_kqr(0)

                def emit_scores_g(qc, p, kc, W):
                    sc_ps = psc.tile([128, 1024], F32,
                                     name=f"s{s}_{qc}_{p}_{kc}", tag="sc")
                    pr = probsp.tile([128, 1024], BF16,
                                     name=f"pr{s}_{qc}_{p}_{kc}", tag="pr")
                    for hh in range(2):
                        hsl = slice(hh * 64, hh * 64 + 64)
                        # 512-strided blocks: the two row-group-concurrent
                        # matmuls must drain to different PSUM banks
                        nc.tensor.matmul(
                            out=sc_ps[:, hh * 512: hh * 512 + W],
                            lhsT=kTp[s][p][hsl, kc * 128:(kc + 1) * 128],
                            rhs=qTp[s][p][hsl, qc * 512: qc * 512 + W],
                            start=True, stop=True)
                    if W == 512:
                        exp_in = sc_ps[:, 0:1024]
                        exp_out = pr[:, 0:1024]
                    else:
                        exp_in = sc_ps[:].rearrange(
                            "p (b w) -> p b w", b=2)[:, :, 0:W]
                        exp_out = pr[:, 0:2 * W].rearrange(
                            "p (b w) -> p b w", b=2)
                    nc.scalar.activation(
                        out=exp_out, in_=exp_in,
                        func=mybir.ActivationFunctionType.Exp,
                        bias=mb[s][:, kc:kc + 1],
                        scale=1.0 / math.sqrt(DK))
                    return pr

                # groups software-pipelined ACROSS boundaries: the next
                # group's first scores/exp is emitted during the current
                # group's last iteration, so ScalarE never refills cold
                groups = [(qc, p) for qc in range(NQ[s]) for p in range(2)]
                att_sb = []
                sc_q = []

                def push_scores(abs_it):
                    # emit scores for absolute iteration abs_it (depth-2
                    # lookahead, rolling across group boundaries)
                    g2, kc2 = divmod(abs_it, nkit)
                    if g2 < len(groups):
                        qc2, p2 = groups[g2]
                        sc_q.append(emit_scores_g(qc2, p2, kc2, qcw(s, qc2)))

                drain_units(0)
                push_scores(0)
                push_scores(1)
                for gi, (qc, p) in enumerate(groups):
                    W = qcw(s, qc)
                    base_idx = gi * nkit
                    if p == 0:
                        if qc + 1 < NQ[s]:
                            emit_kqr(qc + 1)
                        att_sb = []
                    kqr = kqr_t[qc]
                    drain_units(base_idx)
                    attn = pat.tile([128, 512], F32,
                                    name=f"at{s}_{qc}_{p}", tag="at")
                    zps = pz.tile([128, 512], F32,
                                  name=f"z{s}_{qc}_{p}", tag="z")
                    for kc in range(nkit):
                        drain_units(base_idx + kc)
                        first, last = kc == 0, kc == nkit - 1
                        pr = sc_q.pop(0)
                        push_scores(base_idx + kc + 2)
                        for hh in range(2):
                            hsl = slice(hh * 64, hh * 64 + 64)
                            nc.tensor.matmul(
                                out=attn[hsl, 0:W],
                                lhsT=vp[s][kc][:, p * 128 + hh * 64:p * 128 + (hh + 1) * 64],
                                rhs=pr[:, hh * W:(hh + 1) * W],
                                start=first, stop=last)
                            nc.tensor.matmul(
                                out=zps[hsl, 0:W],
                                lhsT=ones[:, :], rhs=pr[:, hh * W:(hh + 1) * W],
                                start=first, stop=last)
                        drain_early(base_idx + kc)
                    # normalize: attn * keepq / Z
                    rz = small.tile([128, 512], F32,
                                    name=f"rz{s}_{qc}_{p}", tag="rz")
                    nc.vector.reciprocal_approx_fast(out=rz[:, 0:W],
                                                     in_=zps[:, 0:W])
                    nc.vector.tensor_mul(rz[:, 0:W], rz[:, 0:W], kqr[:, 0:W])
                    ab = attp.tile([128, 512], BF16,
                                   name=f"ab{s}_{qc}_{p}", tag=f"ab{p}")
                    nc.vector.tensor_mul(ab[:, 0:W], attn[:, 0:W], rz[:, 0:W])
                    att_sb.append(ab)
                    if p == 1:
                        kqr_t.pop(qc)
                        # Wo deferred: one unit per iteration of the NEXT
                        # group, never gating the scores/exp pipeline
                        for ot in range(8):
                            wo_pending.append(((qc + 1) * 2 * nkit + 1 + 2 * ot,
                                               make_wo_unit(qc, ot, att_sb, W)))
                # flush stragglers; alternate between the 'y' bank and the
                # now-idle scores slots so the tail chain pipelines
                while units:
                    units.pop(0)[1]()
                return wo_pending

            # slot 0 projections use the full PSUM (released afterwards)
            with tc.tile_pool(name="pproj", bufs=1, space="PSUM") as pproj:
                emit_proj_streamed(0, pproj)
                v0tiles = emit_v_prefetch0()
                emit_v_resident(0, pproj, v0tiles)
            emit_wo_mb()
            tiles1 = emit_prefetch(1)
            with tc.tile_pool(name="psc", bufs=2, space="PSUM") as psc, \
                 tc.tile_pool(name="pat", bufs=2, space="PSUM") as pat, \
                 tc.tile_pool(name="pz", bufs=1, space="PSUM") as pz, \
                 tc.tile_pool(name="pwo", bufs=1, space="PSUM") as pwo:
                vk1, units1 = proj_units(1, pwo, tiles1, NK[1])
                n0 = NQ[0] * 2 * NK[0]
                sp = max(1, (n0 - 6) // max(1, len(vk1)))
                units0 = [(2 + sp * i, fn) for i, fn in enumerate(vk1)]
                left0 = emit_attention(0, psc, pat, pz, pwo, units=units0)
                carry = [(i + 1, fn) for i, (_, fn) in enumerate(left0)]
                left1 = emit_attention(1, psc, pat, pz, pwo, units=units1,
                                       carry_wo=carry)
                for i, (_, fn) in enumerate(left1):
                    fn("sc" if i % 2 == 0 else None)
    nc.compile()
    return nc


def _get_program(NQ, NK, VQ):
    key = (tuple(NQ), tuple(NK), tuple(VQ))
    if key not in _prog_cache:
        _prog_cache[key] = _build_program(list(NQ), list(NK), list(VQ))
    return _prog_cache[key]


def kernel(value, key, query, padding_mask, Wq, Wk, Wv, Wo):
    value = np.asarray(value)
    key = np.asarray(key)
    query = np.asarray(query)
    padding_mask = np.asarray(padding_mask)
    Wq, Wk, Wv, Wo = (np.asarray(a) for a in (Wq, Wk, Wv, Wo))

    lengths = (~padding_mask).sum(axis=0).astype(int)  # (B,)

    # --- batch pairing: assign batches to (group, slot) minimizing baked work ---
    def slot_counts(assign):
        nq = [max((int(lengths[assign[g][sl]]) + 511) // 512 for g in range(2))
              for sl in range(2)]
        nk = [max((int(lengths[assign[g][sl]]) + 127) // 128 for g in range(2))
              for sl in range(2)]
        return nq, nk

    best = None
    for perm in permutations(range(B)):
        a = ((perm[0], perm[1]), (perm[2], perm[3]))
        nq, nk = slot_counts(a)
        c = nq[0] * nk[0] + nq[1] * nk[1]
        if best is None or c < best[0]:
            best = (c, a)
    assign = best[1]
    nq, nk = slot_counts(assign)
    # slot 0 should be the smaller workload (its projections can't overlap)
    if nq[0] * nk[0] > nq[1] * nk[1]:
        assign = tuple((g[1], g[0]) for g in assign)
        nq, nk = slot_counts(assign)
    NQ, NK = nq, nk
    # trimmed width of the last q chunk per slot (multiple of 32)
    VQ = []
    for sl in range(2):
        maxlen = max(int(lengths[assign[g][sl]]) for g in range(2))
        v = maxlen - (NQ[sl] - 1) * 512
        VQ.append(min(512, (v + 31) // 32 * 32))

    nc = _get_program(NQ, NK, VQ)

    # --- per-core inputs ---
    WqT = np.ascontiguousarray(Wq.T).astype(NPBF16)
    WkT = np.ascontiguousarray(Wk.T).astype(NPBF16)
    WvT = np.ascontiguousarray(Wv.T).astype(NPBF16)
    WoT = np.ascontiguousarray(Wo.T).astype(NPBF16)

    batch_qT, batch_kT, batch_vT, batch_mb, batch_kq = {}, {}, {}, {}, {}
    for b in range(B):
        batch_qT[b] = np.ascontiguousarray(query[:, b, :].T).astype(NPBF16)
        batch_kT[b] = np.ascontiguousarray(key[:, b, :].T).astype(NPBF16)
        batch_vT[b] = np.ascontiguousarray(value[:, b, :].T).astype(NPBF16)
        kpos = np.arange(S).reshape(16, 128)  # [kchunk, kpos]
        mbv = np.where(kpos >= lengths[b], np.float32(MASK_BIAS), np.float32(0.0))
        batch_mb[b] = np.ascontiguousarray(mbv.T).astype(np.float32)  # [128, 16]
        batch_kq[b] = (np.arange(S).reshape(4, 512) < lengths[b]).astype(np.float32)

    in_maps = []
    for c in range(N_CORES):
        g, hq = c // 4, c % 4
        f0 = hq * 256
        m = {
            "wqT": np.ascontiguousarray(WqT[:, f0:f0 + 256]),
            "wkT": np.ascontiguousarray(WkT[:, f0:f0 + 256]),
            "wvT": np.ascontiguousarray(WvT[:, f0:f0 + 256]),
            "woT": np.ascontiguousarray(WoT[f0:f0 + 256, :]),
        }
        for sl in range(2):
            b = assign[g][sl]
            m[f"qT{sl}"] = batch_qT[b]
            m[f"kT{sl}"] = batch_kT[b]
            m[f"vT{sl}"] = batch_vT[b]
            m[f"mb{sl}"] = batch_mb[b]
            m[f"kq{sl}"] = batch_kq[b]
        in_maps.append(m)

    res = run_bass_kernel_spmd(nc, in_maps, list(range(N_CORES)))

    # --- gather: sum 4 head-quad partials per batch, transpose ---
    out = np.zeros((S, B, H), dtype=np.float32)
    for g in range(2):
        for sl in range(2):
            b = assign[g][sl]
            acc = np.zeros((H, S), dtype=np.float32)
            for hq in range(4):
                c = g * 4 + hq
                acc += res.results[c][f"y{sl}"].astype(np.float32)
            out[:, b, :] = acc.T
    return out


# revision 38
# speedup vs baseline: 1.2872x; 1.0109x over previous
"""Multi-head attention (S=2048, B=4, H=1024, NH=16) on 8 Trainium2 NeuronCores.

Sharding: each core handles 2 batches x 4 heads (batch pairs balanced by
valid length; tensor-parallel over heads). Within a core everything is bf16
matmul / fp32 accumulate:
  1. q,k projected d-major (qT/kT: [dims, seq]), v seq-major ([seq, dims])
  2. scoresT[k,q] per head-pair via row-tiled matmuls (row_grp concurrency)
  3. mask+scale+exp fused on ScalarE (per-partition bias; PAD keys -> exp 0)
  4. PV col-tiled accumulates attnT; Z row-sums via all-ones stationary
     matmul (col_grp-concurrent with PV)
  5. attnT normalized by 1/Z; keepq applied before the Wo output multiply
  6. Wo projection -> yT partial [H, S]; host sums 4 partials/batch

Schedule: weight-chunk DMAs interleave with slot0's q/k input streams over
the three DMA-capable queues (sync/scalar/gpsimd), so the first projection
matmul issues ~2 transfers in; slot0 v inputs prefetch into dedicated tiles
(issue never blocks on tile recycling). Attention is emitted as a
software-pipelined kc-loop (scores(kc+1) before PV(kc)); the two scores
matmuls use 512-strided PSUM blocks so their row-group-concurrent drains hit
different banks (same-bank PE+PE drain is a fatal collision when trimmed),
with a strided-AP exp over both blocks. PV+Z pairs run col-group-concurrent.
Wo is deferred into per-ot units drained one-per-iteration of the NEXT
query-group (1-bank PSUM tag shared with projection units), so the output
chain never gates the scores/exp pipeline; the final group's units flush
alternating into the idle scores banks. Slot1's q/k/v projections become
deadline-scheduled units: v/k spread through attention0, q half-a-group
early in attention1. The last q-chunk of each slot is width-trimmed to the
valid length (multiple of 32), shrinking exp/matmul/DVE/DMA work on padded
queries.
"""
import sys

if "/opt/trn_rl_repo" not in sys.path:
    sys.path.insert(0, "/opt/trn_rl_repo")

import math
from itertools import permutations

import ml_dtypes
import numpy as np

import concourse.bass as bass
import concourse.mybir as mybir
import concourse.tile as tile
from concourse import bacc
from concourse.bass_utils import run_bass_kernel_spmd

S, B, H, NH, DK = 2048, 4, 1024, 16, 64
N_CORES = 8
BF16 = mybir.dt.bfloat16
F32 = mybir.dt.float32
NPBF16 = ml_dtypes.bfloat16
MASK_BIAS = -30000.0

_prog_cache: dict = {}


def _build_program(NQ, NK, VQ):
    """One SPMD program. Per batch-slot s: NQ[s] 512-wide q chunks (last one
    VQ[s] wide), NK[s] 128-wide k chunks. Slot 0 is the smaller workload."""
    NSCK = [(nk * 128 + 511) // 512 for nk in NK]
    KW = [nk * 128 for nk in NK]                    # k/v valid width
    QW = [(NQ[s] - 1) * 512 + VQ[s] for s in range(2)]  # q valid width
    nc = bacc.Bacc("TRN2", target_bir_lowering=False, debug=False,
                   num_devices=N_CORES)

    d_in = {}
    for s in range(2):
        d_in[f"qT{s}"] = nc.dram_tensor(f"qT{s}", [H, S], BF16, kind="ExternalInput")
        d_in[f"kT{s}"] = nc.dram_tensor(f"kT{s}", [H, S], BF16, kind="ExternalInput")
        d_in[f"vT{s}"] = nc.dram_tensor(f"vT{s}", [H, S], BF16, kind="ExternalInput")
        d_in[f"mb{s}"] = nc.dram_tensor(f"mb{s}", [128, 16], F32, kind="ExternalInput")
        d_in[f"kq{s}"] = nc.dram_tensor(f"kq{s}", [4, 512], F32, kind="ExternalInput")
    d_in["wqT"] = nc.dram_tensor("wqT", [H, 256], BF16, kind="ExternalInput")
    d_in["wkT"] = nc.dram_tensor("wkT", [H, 256], BF16, kind="ExternalInput")
    d_in["wvT"] = nc.dram_tensor("wvT", [H, 256], BF16, kind="ExternalInput")
    d_in["woT"] = nc.dram_tensor("woT", [256, H], BF16, kind="ExternalInput")
    d_out = [nc.dram_tensor(f"y{s}", [H, S], BF16, kind="ExternalOutput")
             for s in range(2)]

    def qcw(s, sc):
        return 512 if sc < NQ[s] - 1 else VQ[s]

    def kcw(s, sc):
        return min(512, KW[s] - sc * 512)

    with tile.TileContext(nc) as tc:
        with tc.tile_pool(name="wpool", bufs=1) as wpool, \
             tc.tile_pool(name="inp", bufs=6) as inp, \
             tc.tile_pool(name="in8", bufs=1) as in8, \
             tc.tile_pool(name="persist", bufs=1) as persist, \
             tc.tile_pool(name="probs", bufs=3) as probsp, \
             tc.tile_pool(name="small", bufs=2) as small, \
             tc.tile_pool(name="att", bufs=3) as attp, \
             tc.tile_pool(name="yst", bufs=3) as ystp:

            # --- weights: consolidated DMAs on the sync queue ---
            # w*_all[p, ic*256 + j] = w*T[ic*128 + p, j]
            wq_all = wpool.tile([128, 2048], BF16, name="wq_all", tag="wq")
            wk_all = wpool.tile([128, 2048], BF16, name="wk_all", tag="wk")
            wv_all = wpool.tile([128, 2048], BF16, name="wv_all", tag="wv")
            # wo_all[p, j*1024 + c] = woT[j*128 + p, c]
            wo_all = wpool.tile([128, 2048], BF16, name="wo_all", tag="wo")
            _wseen = set()

            def wqkv(t, ic, ft):
                return t[:, ic * 256 + ft * 128: ic * 256 + (ft + 1) * 128]

            mb = [wpool.tile([128, 16], F32, name=f"mbt{s}", tag=f"mbt{s}")
                  for s in range(2)]

            def emit_wo_mb():
                for j in range(2):
                    nc.sync.dma_start(
                        out=wo_all[:, j * 1024:(j + 1) * 1024],
                        in_=d_in["woT"].ap()[j * 128:(j + 1) * 128, :])
                for s in range(2):
                    nc.sync.dma_start(out=mb[s][:], in_=d_in[f"mb{s}"].ap())
            ones = wpool.tile([128, 64], BF16, name="ones", tag="ones")
            nc.vector.memset(ones[:], 1.0)

            # --- persistent projection outputs ---
            qTp = [[persist.tile([128, NQ[s] * 512], BF16, name=f"qTp{s}_{p}",
                                 tag=f"qTp{s}_{p}")
                    for p in range(2)] for s in range(2)]
            kTp = [[persist.tile([128, NSCK[s] * 512], BF16, name=f"kTp{s}_{p}",
                                 tag=f"kTp{s}_{p}")
                    for p in range(2)] for s in range(2)]
            vp = [[persist.tile([128, 256], BF16, name=f"vp{s}_{st}", tag=f"vp{s}_{st}")
                   for st in range(NK[s])] for s in range(2)]

            # DMA queue rotation for input streams (keep Scalar clean once
            # attention starts; Sync carries the weights up front)
            s0_queues = [nc.scalar, nc.gpsimd, nc.sync]
            s1_queues = [nc.sync, nc.gpsimd]
            _qi = [0]

            def dma_rot(queues, out, in_):
                q = queues[_qi[0] % len(queues)]
                _qi[0] += 1
                q.dma_start(out=out, in_=in_)

            def emit_w(t, dname, ic):
                # interleave weight-chunk DMAs with the input stream so the
                # first matmuls aren't gated on the full weight load
                if (dname, ic) in _wseen:
                    return
                _wseen.add((dname, ic))
                dma_rot(s0_queues, t[:, ic * 256:(ic + 1) * 256],
                        d_in[dname].ap()[ic * 128:(ic + 1) * 128, :])

            def emit_proj_streamed(s, pool):
                """ic-outer projections with streamed inputs (slot 0)."""
                for kind, wts, dname, nsc, outtiles, cw in (
                        ("q", wq_all, f"qT{s}", NQ[s], qTp[s], qcw),
                        ("k", wk_all, f"kT{s}", NSCK[s], kTp[s], kcw)):
                    wname = "wqT" if kind == "q" else "wkT"
                    ps = [[pool.tile([128, 512], F32,
                                     name=f"pj{kind}{s}_{ft}_{sc}",
                                     tag=f"pj_{ft}_{sc}")
                           for sc in range(nsc)] for ft in range(2)]
                    tw = sum(cw(s, sc) for sc in range(nsc))
                    for ic in range(8):
                        emit_w(wts, wname, ic)
                        it = inp.tile([128, 2048], BF16,
                                      name=f"in{kind}{s}_{ic}", tag="inp")
                        dma_rot(s0_queues,
                                it[:, 0:tw],
                                d_in[dname].ap()[ic * 128:(ic + 1) * 128, 0:tw])
                        for ft in range(2):
                            for sc in range(nsc):
                                w = cw(s, sc)
                                nc.tensor.matmul(
                                    out=ps[ft][sc][:, 0:w],
                                    lhsT=wqkv(wts, ic, ft),
                                    rhs=it[:, sc * 512: sc * 512 + w],
                                    start=(ic == 0), stop=(ic == 7))
                    for ft in range(2):
                        for sc in range(nsc):
                            w = cw(s, sc)
                            if kind == "q":
                                nc.vector.tensor_copy(
                                    outtiles[ft][:, sc * 512: sc * 512 + w],
                                    ps[ft][sc][:, 0:w])
                            else:
                                nc.scalar.copy(
                                    outtiles[ft][:, sc * 512: sc * 512 + w],
                                    ps[ft][sc][:, 0:w])

            def emit_v_prefetch0():
                tiles = []
                for ic in range(8):
                    emit_w(wv_all, "wvT", ic)
                    it = in8.tile([128, KW[0]], BF16, name=f"v0in_{ic}",
                                  tag=f"v0in{ic}")
                    dma_rot(s0_queues, it[:],
                            d_in["vT0"].ap()[ic * 128:(ic + 1) * 128, 0:KW[0]])
                    tiles.append(it)
                return tiles

            def emit_v_resident(s, pool, tiles):
                for st0 in range(0, NK[s], 8):
                    sts = range(st0, min(st0 + 8, NK[s]))
                    psv = {st: pool.tile([128, 256], F32, name=f"pjv{s}_{st}",
                                         tag=f"pj_{(st - st0) // 4}_{(st - st0) % 4}")
                           for st in sts}
                    for ic in range(8):
                        for st in sts:
                            nc.tensor.matmul(
                                out=psv[st][:],
                                lhsT=tiles[ic][:, st * 128:(st + 1) * 128],
                                rhs=wv_all[:, ic * 256: ic * 256 + 256],
                                start=(ic == 0), stop=(ic == 7))
                    for st in sts:
                        if st % 2:
                            nc.scalar.copy(vp[s][st][:], psv[st][:])
                        else:
                            nc.vector.tensor_copy(vp[s][st][:], psv[st][:])

            def emit_prefetch(s):
                """Issue all of slot s's input DMAs into dedicated tiles."""
                tiles = {}
                for kind, dname, w in (("v", f"vT{s}", KW[s]),
                                       ("k", f"kT{s}", KW[s]),
                                       ("q", f"qT{s}", QW[s])):
                    for ic in range(8):
                        it = in8.tile([128, w], BF16, name=f"pf{kind}{s}_{ic}",
                                      tag=f"pf{kind}{ic}")
                        dma_rot(s1_queues,
                                it[:],
                                d_in[dname].ap()[ic * 128:(ic + 1) * 128, 0:w])
                        tiles[(kind, ic)] = it
                return tiles

            def proj_units(s, pool, tiles, nkit, kinds=("v", "k", "q")):
                """Deadline-tagged projection units for slot s, consumed by
                interleaving into the attention kc-loop. Deadline = global
                iteration index ((qc*2)+p)*NK + kc of first use."""
                units = []

                def q_unit(ft, sc):
                    def emit():
                        w = qcw(s, sc)
                        pj = pool.tile([128, 512], F32,
                                       name=f"rpjq{s}_{ft}_{sc}", tag="y")
                        for ic in range(8):
                            nc.tensor.matmul(
                                out=pj[:, 0:w],
                                lhsT=wqkv(wq_all, ic, ft),
                                rhs=tiles[("q", ic)][:, sc * 512: sc * 512 + w],
                                start=(ic == 0), stop=(ic == 7))
                        nc.vector.tensor_copy(
                            qTp[s][ft][:, sc * 512: sc * 512 + w], pj[:, 0:w])
                    return emit

                def k_unit(ft, sc):
                    def emit():
                        w = kcw(s, sc)
                        pj = pool.tile([128, 512], F32,
                                       name=f"rpjk{s}_{ft}_{sc}", tag="y")
                        for ic in range(8):
                            nc.tensor.matmul(
                                out=pj[:, 0:w],
                                lhsT=wqkv(wk_all, ic, ft),
                                rhs=tiles[("k", ic)][:, sc * 512: sc * 512 + w],
                                start=(ic == 0), stop=(ic == 7))
                        nc.vector.tensor_copy(
                            kTp[s][ft][:, sc * 512: sc * 512 + w], pj[:, 0:w])
                    return emit

                def v_unit(st):
                    def emit():
                        pj = pool.tile([128, 512], F32,
                                       name=f"rpjv{s}_{st}", tag="y")
                        for ic in range(8):
                            nc.tensor.matmul(
                                out=pj[:, 0:256],
                                lhsT=tiles[("v", ic)][:, st * 128:(st + 1) * 128],
                                rhs=wv_all[:, ic * 256: ic * 256 + 256],
                                start=(ic == 0), stop=(ic == 7))
                        nc.vector.tensor_copy(vp[s][st][:], pj[:, 0:256])
                    return emit

                vk = []
                if "v" in kinds:
                    for st in range(NK[s]):
                        vk.append(v_unit(st))
                if "k" in kinds:
                    for ft in range(2):
                        for sc in range(NSCK[s]):
                            vk.append(k_unit(ft, sc))
                if "q" in kinds:
                    for ft in range(2):
                        for sc in range(NQ[s]):
                            units.append((max(0, (sc * 2 + ft) * nkit - nkit + 3),
                                          q_unit(ft, sc)))
                units.sort(key=lambda u: u[0])
                return vk, units

            def emit_attention(s, psc, pat, pz, pwo, units=None, carry_wo=None,
                               lead=4):
                units = list(units) if units else []
                wo_pending = list(carry_wo) if carry_wo else []
                nkit = NK[s]

                def drain_units(cur_idx, pre_group=False):
                    # hard deadlines only: consumers of these units follow
                    # in the in-order PE stream
                    while units and units[0][0] <= cur_idx:
                        units.pop(0)[1]()

                def drain_early(cur_idx):
                    # opportunistic emission after the iteration's PV/Z, so
                    # units never delay the scores->exp pipeline
                    n = 0
                    while units and n < 2 and units[0][0] <= cur_idx + lead:
                        units.pop(0)[1]()
                        n += 1
                    m = 0
                    while wo_pending and wo_pending[0][0] <= cur_idx and m < 2:
                        wo_pending.pop(0)[1](None)
                        m += 1

                def make_wo_unit(qc, ot, ab_pair, W):
                    def emit(tag):
                        if tag is None:
                            yps = pwo.tile([128, 512], F32,
                                           name=f"yp{s}_{qc}_{ot}", tag="y")
                            ypv = yps[:, 0:W]
                        else:
                            yps = psc.tile([128, 1024], F32,
                                           name=f"yp{s}_{qc}_{ot}", tag="sc")
                            ypv = yps[:, 0:W]
                        for j in range(2):
                            nc.tensor.matmul(
                                out=ypv,
                                lhsT=wo_all[:, j * 1024 + ot * 128: j * 1024 + (ot + 1) * 128],
                                rhs=ab_pair[j][:, 0:W],
                                start=(j == 0), stop=(j == 1))
                        ysb = ystp.tile([128, 512], BF16,
                                        name=f"ysb{s}_{qc}_{ot}", tag="ysb")
                        if tag is not None:
                            nc.scalar.copy(ysb[:, 0:W], ypv)
                        else:
                            nc.vector.tensor_copy(ysb[:, 0:W], ypv)
                        (nc.gpsimd if ot % 2 else nc.sync).dma_start(
                            out=d_out[s].ap()[ot * 128:(ot + 1) * 128,
                                              qc * 512: qc * 512 + W],
                            in_=ysb[:, 0:W])
                    return emit

                kqr_t = {}

                def emit_kqr(qc):
                    w = qcw(s, qc)
                    t = small.tile([128, 512], F32, name=f"kqr{s}_{qc}",
                                   tag="kqr")
                    nc.gpsimd.dma_start(
                        out=t[:, 0:w],
                        in_=bass.AP(tensor=d_in[f"kq{s}"], offset=qc * 512,
                                    ap=[[0, 128], [1, w]]))
                    kqr_t[qc] = t

                emit_kqr(0)

                def emit_scores_g(qc, p, kc, W):
                    sc_ps = psc.tile([128, 1024], F32,
                                     name=f"s{s}_{qc}_{p}_{kc}", tag="sc")
                    pr = probsp.tile([128, 1024], BF16,
                                     name=f"pr{s}_{qc}_{p}_{kc}", tag="pr")
                    for hh in range(2):
                        hsl = slice(hh * 64, hh * 64 + 64)
                        # 512-strided blocks: the two row-group-concurrent
                        # matmuls must drain to different PSUM banks
                        nc.tensor.matmul(
                            out=sc_ps[:, hh * 512: hh * 512 + W],
                            lhsT=kTp[s][p][hsl, kc * 128:(kc + 1) * 128],
                            rhs=qTp[s][p][hsl, qc * 512: qc * 512 + W],
                            start=True, stop=True)
                    if W == 512:
                        exp_in = sc_ps[:, 0:1024]
                        exp_out = pr[:, 0:1024]
                    else:
                        exp_in = sc_ps[:].rearrange(
                            "p (b w) -> p b w", b=2)[:, :, 0:W]
                        exp_out = pr[:, 0:2 * W].rearrange(
                            "p (b w) -> p b w", b=2)
                    nc.scalar.activation(
                        out=exp_out, in_=exp_in,
                        func=mybir.ActivationFunctionType.Exp,
                        bias=mb[s][:, kc:kc + 1],
                        scale=1.0 / math.sqrt(DK))
                    return pr

                # groups software-pipelined ACROSS boundaries: the next
                # group's first scores/exp is emitted during the current
                # group's last iteration, so ScalarE never refills cold
                groups = [(qc, p) for qc in range(NQ[s]) for p in range(2)]
                att_sb = []
                sc_q = []

                def push_scores(abs_it):
                    # emit scores for absolute iteration abs_it (depth-2
                    # lookahead, rolling across group boundaries)
                    g2, kc2 = divmod(abs_it, nkit)
                    if g2 < len(groups):
                        qc2, p2 = groups[g2]
                        sc_q.append(emit_scores_g(qc2, p2, kc2, qcw(s, qc2)))

                drain_units(0)
                push_scores(0)
                push_scores(1)
                for gi, (qc, p) in enumerate(groups):
                    W = qcw(s, qc)
                    base_idx = gi * nkit
                    if p == 0:
                        if qc + 1 < NQ[s]:
                            emit_kqr(qc + 1)
                        att_sb = []
                    kqr = kqr_t[qc]
                    drain_units(base_idx)
                    attn = pat.tile([128, 512], F32,
                                    name=f"at{s}_{qc}_{p}", tag="at")
                    zps = pz.tile([128, 512], F32,
                                  name=f"z{s}_{qc}_{p}", tag="z")
                    for kc in range(nkit):
                        drain_units(base_idx + kc)
                        first, last = kc == 0, kc == nkit - 1
                        pr = sc_q.pop(0)
                        push_scores(base_idx + kc + 2)
                        for hh in range(2):
                            hsl = slice(hh * 64, hh * 64 + 64)
                            nc.tensor.matmul(
                                out=attn[hsl, 0:W],
                                lhsT=vp[s][kc][:, p * 128 + hh * 64:p * 128 + (hh + 1) * 64],
                                rhs=pr[:, hh * W:(hh + 1) * W],
                                start=first, stop=last)
                            nc.tensor.matmul(
                                out=zps[hsl, 0:W],
                                lhsT=ones[:, :], rhs=pr[:, hh * W:(hh + 1) * W],
                                start=first, stop=last)
                        drain_early(base_idx + kc)
                    # normalize: attn * keepq / Z
                    rz = small.tile([128, 512], F32,
                                    name=f"rz{s}_{qc}_{p}", tag="rz")
                    nc.vector.reciprocal_approx_fast(out=rz[:, 0:W],
                                                     in_=zps[:, 0:W])
                    nc.vector.tensor_mul(rz[:, 0:W], rz[:, 0:W], kqr[:, 0:W])
                    ab = attp.tile([128, 512], BF16,
                                   name=f"ab{s}_{qc}_{p}", tag=f"ab{p}")
                    nc.vector.tensor_mul(ab[:, 0:W], attn[:, 0:W], rz[:, 0:W])
                    att_sb.append(ab)
                    if p == 1:
                        kqr_t.pop(qc)
                        # Wo deferred: one unit per iteration of the NEXT
                        # group, never gating the scores/exp pipeline
                        for ot in range(8):
                            wo_pending.append(((qc + 1) * 2 * nkit + 1 + 2 * ot,
                                               make_wo_unit(qc, ot, att_sb, W)))
                # flush stragglers; alternate between the 'y' bank and the
                # now-idle scores slots so the tail chain pipelines
                while units:
                    units.pop(0)[1]()
                return wo_pending

            # slot 0 projections use the full PSUM (released afterwards)
            with tc.tile_pool(name="pproj", bufs=1, space="PSUM") as pproj:
                emit_proj_streamed(0, pproj)
                v0tiles = emit_v_prefetch0()
                emit_v_resident(0, pproj, v0tiles)
            emit_wo_mb()
            tiles1 = emit_prefetch(1)
            with tc.tile_pool(name="psc", bufs=2, space="PSUM") as psc, \
                 tc.tile_pool(name="pat", bufs=2, space="PSUM") as pat, \
                 tc.tile_pool(name="pz", bufs=1, space="PSUM") as pz, \
                 tc.tile_pool(name="pwo", bufs=1, space="PSUM") as pwo:
                vk1, units1 = proj_units(1, pwo, tiles1, NK[1])
                n0 = NQ[0] * 2 * NK[0]
                sp = max(1, (n0 - 6) // max(1, len(vk1)))
                units0 = [(2 + sp * i, fn) for i, fn in enumerate(vk1)]
                left0 = emit_attention(0, psc, pat, pz, pwo, units=units0)
                carry = [(i + 1, fn) for i, (_, fn) in enumerate(left0)]
                left1 = emit_attention(1, psc, pat, pz, pwo, units=units1,
                                       carry_wo=carry)
                for i, (_, fn) in enumerate(left1):
                    fn("sc" if i % 2 == 0 else None)
    nc.compile()
    return nc


def _get_program(NQ, NK, VQ):
    key = (tuple(NQ), tuple(NK), tuple(VQ))
    if key not in _prog_cache:
        _prog_cache[key] = _build_program(list(NQ), list(NK), list(VQ))
    return _prog_cache[key]


def kernel(value, key, query, padding_mask, Wq, Wk, Wv, Wo):
    value = np.asarray(value)
    key = np.asarray(key)
    query = np.asarray(query)
    padding_mask = np.asarray(padding_mask)
    Wq, Wk, Wv, Wo = (np.asarray(a) for a in (Wq, Wk, Wv, Wo))

    lengths = (~padding_mask).sum(axis=0).astype(int)  # (B,)

    # --- batch pairing: assign batches to (group, slot) minimizing baked work ---
    def slot_counts(assign):
        nq = [max((int(lengths[assign[g][sl]]) + 511) // 512 for g in range(2))
              for sl in range(2)]
        nk = [max((int(lengths[assign[g][sl]]) + 127) // 128 for g in range(2))
              for sl in range(2)]
        return nq, nk

    best = None
    for perm in permutations(range(B)):
        a = ((perm[0], perm[1]), (perm[2], perm[3]))
        nq, nk = slot_counts(a)
        c = nq[0] * nk[0] + nq[1] * nk[1]
        if best is None or c < best[0]:
            best = (c, a)
    assign = best[1]
    nq, nk = slot_counts(assign)
    # slot 0 should be the smaller workload (its projections can't overlap)
    if nq[0] * nk[0] > nq[1] * nk[1]:
        assign = tuple((g[1], g[0]) for g in assign)
        nq, nk = slot_counts(assign)
    NQ, NK = nq, nk
    # trimmed width of the last q chunk per slot (multiple of 32)
    VQ = []
    for sl in range(2):
        maxlen = max(int(lengths[assign[g][sl]]) for g in range(2))
        v = maxlen - (NQ[sl] - 1) * 512
        VQ.append(min(512, (v + 31) // 32 * 32))

    nc = _get_program(NQ, NK, VQ)

    # --- per-core inputs ---
    WqT = np.ascontiguousarray(Wq.T).astype(NPBF16)
    WkT = np.ascontiguousarray(Wk.T).astype(NPBF16)
    WvT = np.ascontiguousarray(Wv.T).astype(NPBF16)
    WoT = np.ascontiguousarray(Wo.T).astype(NPBF16)

    batch_qT, batch_kT, batch_vT, batch_mb, batch_kq = {}, {}, {}, {}, {}
    for b in range(B):
        batch_qT[b] = np.ascontiguousarray(query[:, b, :].T).astype(NPBF16)
        batch_kT[b] = np.ascontiguousarray(key[:, b, :].T).astype(NPBF16)
        batch_vT[b] = np.ascontiguousarray(value[:, b, :].T).astype(NPBF16)
        kpos = np.arange(S).reshape(16, 128)  # [kchunk, kpos]
        mbv = np.where(kpos >= lengths[b], np.float32(MASK_BIAS), np.float32(0.0))
        batch_mb[b] = np.ascontiguousarray(mbv.T).astype(np.float32)  # [128, 16]
        batch_kq[b] = (np.arange(S).reshape(4, 512) < lengths[b]).astype(np.float32)

    in_maps = []
    for c in range(N_CORES):
        g, hq = c // 4, c % 4
        f0 = hq * 256
        m = {
            "wqT": np.ascontiguousarray(WqT[:, f0:f0 + 256]),
            "wkT": np.ascontiguousarray(WkT[:, f0:f0 + 256]),
            "wvT": np.ascontiguousarray(WvT[:, f0:f0 + 256]),
            "woT": np.ascontiguousarray(WoT[f0:f0 + 256, :]),
        }
        for sl in range(2):
            b = assign[g][sl]
            m[f"qT{sl}"] = batch_qT[b]
            m[f"kT{sl}"] = batch_kT[b]
            m[f"vT{sl}"] = batch_vT[b]
            m[f"mb{sl}"] = batch_mb[b]
            m[f"kq{sl}"] = batch_kq[b]
        in_maps.append(m)

    res = run_bass_kernel_spmd(nc, in_maps, list(range(N_CORES)))

    # --- gather: sum 4 head-quad partials per batch, transpose ---
    out = np.zeros((S, B, H), dtype=np.float32)
    for g in range(2):
        for sl in range(2):
            b = assign[g][sl]
            acc = np.zeros((H, S), dtype=np.float32)
            for hq in range(4):
                c = g * 4 + hq
                acc += res.results[c][f"y{sl}"].astype(np.float32)
            out[:, b, :] = acc.T
    return out


# revision 39
# speedup vs baseline: 1.2985x; 1.0088x over previous
"""Multi-head attention (S=2048, B=4, H=1024, NH=16) on 8 Trainium2 NeuronCores.

Sharding: each core handles 2 batches x 4 heads (batch pairs balanced by
valid length; tensor-parallel over heads). Within a core everything is bf16
matmul / fp32 accumulate:
  1. q,k projected d-major (qT/kT: [dims, seq]), v seq-major ([seq, dims])
  2. scoresT[k,q] per head-pair via row-tiled matmuls (row_grp concurrency)
  3. mask+scale+exp fused on ScalarE (per-partition bias; PAD keys -> exp 0)
  4. PV col-tiled accumulates attnT; Z row-sums via all-ones stationary
     matmul (col_grp-concurrent with PV)
  5. attnT normalized by 1/Z; keepq applied before the Wo output multiply
  6. Wo projection -> yT partial [H, S]; host sums 4 partials/batch

Schedule: weight-chunk DMAs interleave with slot0's q/k input streams over
the three DMA-capable queues (sync/scalar/gpsimd), so the first projection
matmul issues ~2 transfers in; slot0 v inputs prefetch into dedicated tiles
(issue never blocks on tile recycling). Attention is emitted as a
software-pipelined kc-loop (scores(kc+1) before PV(kc)); the two scores
matmuls use 512-strided PSUM blocks so their row-group-concurrent drains hit
different banks (same-bank PE+PE drain is a fatal collision when trimmed),
with a strided-AP exp over both blocks. PV+Z pairs run col-group-concurrent.
Wo is deferred into per-ot units drained one-per-iteration of the NEXT
query-group (1-bank PSUM tag shared with projection units), so the output
chain never gates the scores/exp pipeline; the final group's units flush
alternating into the idle scores banks. Slot1's q/k/v projections become
deadline-scheduled units: v/k spread through attention0, q half-a-group
early in attention1. The last q-chunk of each slot is width-trimmed to the
valid length (multiple of 32), shrinking exp/matmul/DVE/DMA work on padded
queries.
"""
import sys

if "/opt/trn_rl_repo" not in sys.path:
    sys.path.insert(0, "/opt/trn_rl_repo")

import math
from itertools import permutations

import ml_dtypes
import numpy as np

import concourse.bass as bass
import concourse.mybir as mybir
import concourse.tile as tile
from concourse import bacc
from concourse.bass_utils import run_bass_kernel_spmd

S, B, H, NH, DK = 2048, 4, 1024, 16, 64
N_CORES = 8
BF16 = mybir.dt.bfloat16
F32 = mybir.dt.float32
NPBF16 = ml_dtypes.bfloat16
MASK_BIAS = -30000.0

_prog_cache: dict = {}


def _build_program(NQ, NK, VQ):
    """One SPMD program. Per batch-slot s: NQ[s] 512-wide q chunks (last one
    VQ[s] wide), NK[s] 128-wide k chunks. Slot 0 is the smaller workload."""
    NSCK = [(nk * 128 + 511) // 512 for nk in NK]
    KW = [nk * 128 for nk in NK]                    # k/v valid width
    QW = [(NQ[s] - 1) * 512 + VQ[s] for s in range(2)]  # q valid width
    nc = bacc.Bacc("TRN2", target_bir_lowering=False, debug=False,
                   num_devices=N_CORES)

    d_in = {}
    for s in range(2):
        d_in[f"qT{s}"] = nc.dram_tensor(f"qT{s}", [H, S], BF16, kind="ExternalInput")
        d_in[f"kT{s}"] = nc.dram_tensor(f"kT{s}", [H, S], BF16, kind="ExternalInput")
        d_in[f"vT{s}"] = nc.dram_tensor(f"vT{s}", [H, S], BF16, kind="ExternalInput")
        d_in[f"mb{s}"] = nc.dram_tensor(f"mb{s}", [128, 16], F32, kind="ExternalInput")
        d_in[f"kq{s}"] = nc.dram_tensor(f"kq{s}", [4, 512], F32, kind="ExternalInput")
    d_in["wqT"] = nc.dram_tensor("wqT", [H, 256], BF16, kind="ExternalInput")
    d_in["wkT"] = nc.dram_tensor("wkT", [H, 256], BF16, kind="ExternalInput")
    d_in["wvT"] = nc.dram_tensor("wvT", [H, 256], BF16, kind="ExternalInput")
    d_in["woT"] = nc.dram_tensor("woT", [256, H], BF16, kind="ExternalInput")
    d_out = [nc.dram_tensor(f"y{s}", [H, S], BF16, kind="ExternalOutput")
             for s in range(2)]

    def qcw(s, sc):
        return 512 if sc < NQ[s] - 1 else VQ[s]

    def kcw(s, sc):
        return min(512, KW[s] - sc * 512)

    with tile.TileContext(nc) as tc:
        with tc.tile_pool(name="wpool", bufs=1) as wpool, \
             tc.tile_pool(name="inp", bufs=6) as inp, \
             tc.tile_pool(name="in8", bufs=1) as in8, \
             tc.tile_pool(name="persist", bufs=1) as persist, \
             tc.tile_pool(name="probs", bufs=3) as probsp, \
             tc.tile_pool(name="small", bufs=2) as small, \
             tc.tile_pool(name="att", bufs=3) as attp, \
             tc.tile_pool(name="yst", bufs=3) as ystp:

            # --- weights: consolidated DMAs on the sync queue ---
            # w*_all[p, ic*256 + j] = w*T[ic*128 + p, j]
            wq_all = wpool.tile([128, 2048], BF16, name="wq_all", tag="wq")
            wk_all = wpool.tile([128, 2048], BF16, name="wk_all", tag="wk")
            wv_all = wpool.tile([128, 2048], BF16, name="wv_all", tag="wv")
            # wo_all[p, j*1024 + c] = woT[j*128 + p, c]
            wo_all = wpool.tile([128, 2048], BF16, name="wo_all", tag="wo")
            _wseen = set()

            def wqkv(t, ic, ft):
                return t[:, ic * 256 + ft * 128: ic * 256 + (ft + 1) * 128]

            mb = [wpool.tile([128, 16], F32, name=f"mbt{s}", tag=f"mbt{s}")
                  for s in range(2)]

            def emit_wo_mb():
                for j in range(2):
                    nc.sync.dma_start(
                        out=wo_all[:, j * 1024:(j + 1) * 1024],
                        in_=d_in["woT"].ap()[j * 128:(j + 1) * 128, :])
                for s in range(2):
                    nc.sync.dma_start(out=mb[s][:], in_=d_in[f"mb{s}"].ap())
            ones = wpool.tile([128, 64], BF16, name="ones", tag="ones")
            nc.vector.memset(ones[:], 1.0)

            # --- persistent projection outputs ---
            qTp = [[persist.tile([128, NQ[s] * 512], BF16, name=f"qTp{s}_{p}",
                                 tag=f"qTp{s}_{p}")
                    for p in range(2)] for s in range(2)]
            kTp = [[persist.tile([128, NSCK[s] * 512], BF16, name=f"kTp{s}_{p}",
                                 tag=f"kTp{s}_{p}")
                    for p in range(2)] for s in range(2)]
            vp = [[persist.tile([128, 256], BF16, name=f"vp{s}_{st}", tag=f"vp{s}_{st}")
                   for st in range(NK[s])] for s in range(2)]

            # DMA queue rotation for input streams (keep Scalar clean once
            # attention starts; Sync carries the weights up front)
            s0_queues = [nc.scalar, nc.gpsimd, nc.sync]
            s1_queues = [nc.sync, nc.gpsimd]
            _qi = [0]

            def dma_rot(queues, out, in_):
                q = queues[_qi[0] % len(queues)]
                _qi[0] += 1
                q.dma_start(out=out, in_=in_)

            def emit_w(t, dname, ic):
                # interleave weight-chunk DMAs with the input stream so the
                # first matmuls aren't gated on the full weight load
                if (dname, ic) in _wseen:
                    return
                _wseen.add((dname, ic))
                dma_rot(s0_queues, t[:, ic * 256:(ic + 1) * 256],
                        d_in[dname].ap()[ic * 128:(ic + 1) * 128, :])

            def emit_proj_streamed(s, pool):
                """ic-outer projections with streamed inputs (slot 0)."""
                for kind, wts, dname, nsc, outtiles, cw in (
                        ("q", wq_all, f"qT{s}", NQ[s], qTp[s], qcw),
                        ("k", wk_all, f"kT{s}", NSCK[s], kTp[s], kcw)):
                    wname = "wqT" if kind == "q" else "wkT"
                    ps = [[pool.tile([128, 512], F32,
                                     name=f"pj{kind}{s}_{ft}_{sc}",
                                     tag=f"pj_{ft}_{sc}")
                           for sc in range(nsc)] for ft in range(2)]
                    tw = sum(cw(s, sc) for sc in range(nsc))
                    for ic in range(8):
                        emit_w(wts, wname, ic)
                        it = inp.tile([128, 2048], BF16,
                                      name=f"in{kind}{s}_{ic}", tag="inp")
                        dma_rot(s0_queues,
                                it[:, 0:tw],
                                d_in[dname].ap()[ic * 128:(ic + 1) * 128, 0:tw])
                        for ft in range(2):
                            for sc in range(nsc):
                                w = cw(s, sc)
                                nc.tensor.matmul(
                                    out=ps[ft][sc][:, 0:w],
                                    lhsT=wqkv(wts, ic, ft),
                                    rhs=it[:, sc * 512: sc * 512 + w],
                                    start=(ic == 0), stop=(ic == 7))
                    for ft in range(2):
                        for sc in range(nsc):
                            w = cw(s, sc)
                            if kind == "q":
                                nc.vector.tensor_copy(
                                    outtiles[ft][:, sc * 512: sc * 512 + w],
                                    ps[ft][sc][:, 0:w])
                            else:
                                nc.scalar.copy(
                                    outtiles[ft][:, sc * 512: sc * 512 + w],
                                    ps[ft][sc][:, 0:w])

            def emit_v_prefetch0():
                tiles = []
                for ic in range(8):
                    emit_w(wv_all, "wvT", ic)
                    it = in8.tile([128, KW[0]], BF16, name=f"v0in_{ic}",
                                  tag=f"v0in{ic}")
                    dma_rot(s0_queues, it[:],
                            d_in["vT0"].ap()[ic * 128:(ic + 1) * 128, 0:KW[0]])
                    tiles.append(it)
                return tiles

            def emit_v_resident(s, pool, tiles):
                for st0 in range(0, NK[s], 8):
                    sts = range(st0, min(st0 + 8, NK[s]))
                    psv = {st: pool.tile([128, 256], F32, name=f"pjv{s}_{st}",
                                         tag=f"pj_{(st - st0) // 4}_{(st - st0) % 4}")
                           for st in sts}
                    for ic in range(8):
                        for st in sts:
                            nc.tensor.matmul(
                                out=psv[st][:],
                                lhsT=tiles[ic][:, st * 128:(st + 1) * 128],
                                rhs=wv_all[:, ic * 256: ic * 256 + 256],
                                start=(ic == 0), stop=(ic == 7))
                    for st in sts:
                        if st % 2:
                            nc.scalar.copy(vp[s][st][:], psv[st][:])
                        else:
                            nc.vector.tensor_copy(vp[s][st][:], psv[st][:])

            def emit_prefetch(s):
                """Issue all of slot s's input DMAs into dedicated tiles."""
                tiles = {}
                for kind, dname, w in (("v", f"vT{s}", KW[s]),
                                       ("k", f"kT{s}", KW[s]),
                                       ("q", f"qT{s}", QW[s])):
                    for ic in range(8):
                        it = in8.tile([128, w], BF16, name=f"pf{kind}{s}_{ic}",
                                      tag=f"pf{kind}{ic}")
                        dma_rot(s1_queues,
                                it[:],
                                d_in[dname].ap()[ic * 128:(ic + 1) * 128, 0:w])
                        tiles[(kind, ic)] = it
                return tiles

            def proj_units(s, pool, tiles, nkit, kinds=("v", "k", "q")):
                """Deadline-tagged projection units for slot s, consumed by
                interleaving into the attention kc-loop. Deadline = global
                iteration index ((qc*2)+p)*NK + kc of first use."""
                units = []

                def q_unit(ft, sc):
                    def emit():
                        w = qcw(s, sc)
                        pj = pool.tile([128, 512], F32,
                                       name=f"rpjq{s}_{ft}_{sc}", tag="y")
                        for ic in range(8):
                            nc.tensor.matmul(
                                out=pj[:, 0:w],
                                lhsT=wqkv(wq_all, ic, ft),
                                rhs=tiles[("q", ic)][:, sc * 512: sc * 512 + w],
                                start=(ic == 0), stop=(ic == 7))
                        nc.vector.tensor_copy(
                            qTp[s][ft][:, sc * 512: sc * 512 + w], pj[:, 0:w])
                    return emit

                def k_unit(ft, sc):
                    def emit():
                        w = kcw(s, sc)
                        pj = pool.tile([128, 512], F32,
                                       name=f"rpjk{s}_{ft}_{sc}", tag="y")
                        for ic in range(8):
                            nc.tensor.matmul(
                                out=pj[:, 0:w],
                                lhsT=wqkv(wk_all, ic, ft),
                                rhs=tiles[("k", ic)][:, sc * 512: sc * 512 + w],
                                start=(ic == 0), stop=(ic == 7))
                        nc.vector.tensor_copy(
                            kTp[s][ft][:, sc * 512: sc * 512 + w], pj[:, 0:w])
                    return emit

                def v_unit(st):
                    def emit():
                        pj = pool.tile([128, 512], F32,
                                       name=f"rpjv{s}_{st}", tag="y")
                        for ic in range(8):
                            nc.tensor.matmul(
                                out=pj[:, 0:256],
                                lhsT=tiles[("v", ic)][:, st * 128:(st + 1) * 128],
                                rhs=wv_all[:, ic * 256: ic * 256 + 256],
                                start=(ic == 0), stop=(ic == 7))
                        nc.vector.tensor_copy(vp[s][st][:], pj[:, 0:256])
                    return emit

                vk = []
                if "v" in kinds:
                    for st in range(NK[s]):
                        vk.append(v_unit(st))
                if "k" in kinds:
                    for ft in range(2):
                        for sc in range(NSCK[s]):
                            vk.append(k_unit(ft, sc))
                if "q" in kinds:
                    for ft in range(2):
                        for sc in range(NQ[s]):
                            if sc == 0:
                                # first q chunks ride in the PREVIOUS slot's
                                # attention window (inputs land well before)
                                vk.append(q_unit(ft, sc))
                            else:
                                units.append(
                                    (max(0, (sc * 2 + ft) * nkit - nkit + 3),
                                     q_unit(ft, sc)))
                units.sort(key=lambda u: u[0])
                return vk, units

            def emit_attention(s, psc, pat, pz, pwo, units=None, carry_wo=None,
                               lead=4):
                units = list(units) if units else []
                wo_pending = list(carry_wo) if carry_wo else []
                nkit = NK[s]

                def drain_units(cur_idx, pre_group=False):
                    # hard deadlines only: consumers of these units follow
                    # in the in-order PE stream
                    while units and units[0][0] <= cur_idx:
                        units.pop(0)[1]()

                def drain_early(cur_idx):
                    # opportunistic emission after the iteration's PV/Z, so
                    # units never delay the scores->exp pipeline
                    n = 0
                    while units and n < 2 and units[0][0] <= cur_idx + lead:
                        units.pop(0)[1]()
                        n += 1
                    m = 0
                    while wo_pending and wo_pending[0][0] <= cur_idx and m < 2:
                        wo_pending.pop(0)[1](None)
                        m += 1

                def make_wo_unit(qc, ot, ab_pair, W):
                    def emit(tag):
                        if tag is None:
                            yps = pwo.tile([128, 512], F32,
                                           name=f"yp{s}_{qc}_{ot}", tag="y")
                            ypv = yps[:, 0:W]
                        else:
                            yps = psc.tile([128, 1024], F32,
                                           name=f"yp{s}_{qc}_{ot}", tag="sc")
                            ypv = yps[:, 0:W]
                        for j in range(2):
                            nc.tensor.matmul(
                                out=ypv,
                                lhsT=wo_all[:, j * 1024 + ot * 128: j * 1024 + (ot + 1) * 128],
                                rhs=ab_pair[j][:, 0:W],
                                start=(j == 0), stop=(j == 1))
                        ysb = ystp.tile([128, 512], BF16,
                                        name=f"ysb{s}_{qc}_{ot}", tag="ysb")
                        if tag is not None:
                            nc.scalar.copy(ysb[:, 0:W], ypv)
                        else:
                            nc.vector.tensor_copy(ysb[:, 0:W], ypv)
                        (nc.gpsimd if ot % 2 else nc.sync).dma_start(
                            out=d_out[s].ap()[ot * 128:(ot + 1) * 128,
                                              qc * 512: qc * 512 + W],
                            in_=ysb[:, 0:W])
                    return emit

                kqr_t = {}

                def emit_kqr(qc):
                    w = qcw(s, qc)
                    t = small.tile([128, 512], F32, name=f"kqr{s}_{qc}",
                                   tag="kqr")
                    nc.gpsimd.dma_start(
                        out=t[:, 0:w],
                        in_=bass.AP(tensor=d_in[f"kq{s}"], offset=qc * 512,
                                    ap=[[0, 128], [1, w]]))
                    kqr_t[qc] = t

                emit_kqr(0)

                def emit_scores_g(qc, p, kc, W):
                    sc_ps = psc.tile([128, 1024], F32,
                                     name=f"s{s}_{qc}_{p}_{kc}", tag="sc")
                    pr = probsp.tile([128, 1024], BF16,
                                     name=f"pr{s}_{qc}_{p}_{kc}", tag="pr")
                    for hh in range(2):
                        hsl = slice(hh * 64, hh * 64 + 64)
                        # 512-strided blocks: the two row-group-concurrent
                        # matmuls must drain to different PSUM banks
                        nc.tensor.matmul(
                            out=sc_ps[:, hh * 512: hh * 512 + W],
                            lhsT=kTp[s][p][hsl, kc * 128:(kc + 1) * 128],
                            rhs=qTp[s][p][hsl, qc * 512: qc * 512 + W],
                            start=True, stop=True)
                    if W == 512:
                        exp_in = sc_ps[:, 0:1024]
                        exp_out = pr[:, 0:1024]
                    else:
                        exp_in = sc_ps[:].rearrange(
                            "p (b w) -> p b w", b=2)[:, :, 0:W]
                        exp_out = pr[:, 0:2 * W].rearrange(
                            "p (b w) -> p b w", b=2)
                    nc.scalar.activation(
                        out=exp_out, in_=exp_in,
                        func=mybir.ActivationFunctionType.Exp,
                        bias=mb[s][:, kc:kc + 1],
                        scale=1.0 / math.sqrt(DK))
                    return pr

                # groups software-pipelined ACROSS boundaries: the next
                # group's first scores/exp is emitted during the current
                # group's last iteration, so ScalarE never refills cold
                groups = [(qc, p) for qc in range(NQ[s]) for p in range(2)]
                att_sb = []
                sc_q = []

                def push_scores(abs_it):
                    # emit scores for absolute iteration abs_it (depth-2
                    # lookahead, rolling across group boundaries)
                    g2, kc2 = divmod(abs_it, nkit)
                    if g2 < len(groups):
                        qc2, p2 = groups[g2]
                        sc_q.append(emit_scores_g(qc2, p2, kc2, qcw(s, qc2)))

                drain_units(0)
                push_scores(0)
                push_scores(1)
                for gi, (qc, p) in enumerate(groups):
                    W = qcw(s, qc)
                    base_idx = gi * nkit
                    if p == 0:
                        if qc + 1 < NQ[s]:
                            emit_kqr(qc + 1)
                        att_sb = []
                    kqr = kqr_t[qc]
                    drain_units(base_idx)
                    attn = pat.tile([128, 512], F32,
                                    name=f"at{s}_{qc}_{p}", tag="at")
                    zps = pz.tile([128, 512], F32,
                                  name=f"z{s}_{qc}_{p}", tag="z")
                    for kc in range(nkit):
                        drain_units(base_idx + kc)
                        first, last = kc == 0, kc == nkit - 1
                        pr = sc_q.pop(0)
                        push_scores(base_idx + kc + 2)
                        for hh in range(2):
                            hsl = slice(hh * 64, hh * 64 + 64)
                            nc.tensor.matmul(
                                out=attn[hsl, 0:W],
                                lhsT=vp[s][kc][:, p * 128 + hh * 64:p * 128 + (hh + 1) * 64],
                                rhs=pr[:, hh * W:(hh + 1) * W],
                                start=first, stop=last)
                            nc.tensor.matmul(
                                out=zps[hsl, 0:W],
                                lhsT=ones[:, :], rhs=pr[:, hh * W:(hh + 1) * W],
                                start=first, stop=last)
                        drain_early(base_idx + kc)
                    # normalize: attn * keepq / Z
                    rz = small.tile([128, 512], F32,
                                    name=f"rz{s}_{qc}_{p}", tag="rz")
                    nc.vector.reciprocal_approx_fast(out=rz[:, 0:W],
                                                     in_=zps[:, 0:W])
                    nc.vector.tensor_mul(rz[:, 0:W], rz[:, 0:W], kqr[:, 0:W])
                    ab = attp.tile([128, 512], BF16,
                                   name=f"ab{s}_{qc}_{p}", tag=f"ab{p}")
                    nc.vector.tensor_mul(ab[:, 0:W], attn[:, 0:W], rz[:, 0:W])
                    att_sb.append(ab)
                    if p == 1:
                        kqr_t.pop(qc)
                        # Wo deferred: one unit per iteration of the NEXT
                        # group, never gating the scores/exp pipeline
                        for ot in range(8):
                            wo_pending.append(((qc + 1) * 2 * nkit + 1 + 2 * ot,
                                               make_wo_unit(qc, ot, att_sb, W)))
                # flush stragglers; alternate between the 'y' bank and the
                # now-idle scores slots so the tail chain pipelines
                while units:
                    units.pop(0)[1]()
                return wo_pending

            # slot 0 projections use the full PSUM (released afterwards)
            with tc.tile_pool(name="pproj", bufs=1, space="PSUM") as pproj:
                emit_proj_streamed(0, pproj)
                v0tiles = emit_v_prefetch0()
                emit_v_resident(0, pproj, v0tiles)
            emit_wo_mb()
            tiles1 = emit_prefetch(1)
            with tc.tile_pool(name="psc", bufs=2, space="PSUM") as psc, \
                 tc.tile_pool(name="pat", bufs=2, space="PSUM") as pat, \
                 tc.tile_pool(name="pz", bufs=1, space="PSUM") as pz, \
                 tc.tile_pool(name="pwo", bufs=1, space="PSUM") as pwo:
                vk1, units1 = proj_units(1, pwo, tiles1, NK[1])
                n0 = NQ[0] * 2 * NK[0]
                sp = max(1, (n0 - 6) // max(1, len(vk1)))
                units0 = [(2 + sp * i, fn) for i, fn in enumerate(vk1)]
                left0 = emit_attention(0, psc, pat, pz, pwo, units=units0)
                carry = [(i + 1, fn) for i, (_, fn) in enumerate(left0)]
                left1 = emit_attention(1, psc, pat, pz, pwo, units=units1,
                                       carry_wo=carry)
                for i, (_, fn) in enumerate(left1):
                    fn("sc" if i % 2 == 0 else None)
    nc.compile()
    return nc


def _get_program(NQ, NK, VQ):
    key = (tuple(NQ), tuple(NK), tuple(VQ))
    if key not in _prog_cache:
        _prog_cache[key] = _build_program(list(NQ), list(NK), list(VQ))
    return _prog_cache[key]


def kernel(value, key, query, padding_mask, Wq, Wk, Wv, Wo):
    value = np.asarray(value)
    key = np.asarray(key)
    query = np.asarray(query)
    padding_mask = np.asarray(padding_mask)
    Wq, Wk, Wv, Wo = (np.asarray(a) for a in (Wq, Wk, Wv, Wo))

    lengths = (~padding_mask).sum(axis=0).astype(int)  # (B,)

    # --- batch pairing: assign batches to (group, slot) minimizing baked work ---
    def slot_counts(assign):
        nq = [max((int(lengths[assign[g][sl]]) + 511) // 512 for g in range(2))
              for sl in range(2)]
        nk = [max((int(lengths[assign[g][sl]]) + 127) // 128 for g in range(2))
              for sl in range(2)]
        return nq, nk

    best = None
    for perm in permutations(range(B)):
        a = ((perm[0], perm[1]), (perm[2], perm[3]))
        nq, nk = slot_counts(a)
        c = nq[0] * nk[0] + nq[1] * nk[1]
        if best is None or c < best[0]:
            best = (c, a)
    assign = best[1]
    nq, nk = slot_counts(assign)
    # slot 0 should be the smaller workload (its projections can't overlap)
    if nq[0] * nk[0] > nq[1] * nk[1]:
        assign = tuple((g[1], g[0]) for g in assign)
        nq, nk = slot_counts(assign)
    NQ, NK = nq, nk
    # trimmed width of the last q chunk per slot (multiple of 32)
    VQ = []
    for sl in range(2):
        maxlen = max(int(lengths[assign[g][sl]]) for g in range(2))
        v = maxlen - (NQ[sl] - 1) * 512
        VQ.append(min(512, (v + 31) // 32 * 32))

    nc = _get_program(NQ, NK, VQ)

    # --- per-core inputs ---
    WqT = np.ascontiguousarray(Wq.T).astype(NPBF16)
    WkT = np.ascontiguousarray(Wk.T).astype(NPBF16)
    WvT = np.ascontiguousarray(Wv.T).astype(NPBF16)
    WoT = np.ascontiguousarray(Wo.T).astype(NPBF16)

    batch_qT, batch_kT, batch_vT, batch_mb, batch_kq = {}, {}, {}, {}, {}
    for b in range(B):
        batch_qT[b] = np.ascontiguousarray(query[:, b, :].T).astype(NPBF16)
        batch_kT[b] = np.ascontiguousarray(key[:, b, :].T).astype(NPBF16)
        batch_vT[b] = np.ascontiguousarray(value[:, b, :].T).astype(NPBF16)
        kpos = np.arange(S).reshape(16, 128)  # [kchunk, kpos]
        mbv = np.where(kpos >= lengths[b], np.float32(MASK_BIAS), np.float32(0.0))
        batch_mb[b] = np.ascontiguousarray(mbv.T).astype(np.float32)  # [128, 16]
        batch_kq[b] = (np.arange(S).reshape(4, 512) < lengths[b]).astype(np.float32)

    in_maps = []
    for c in range(N_CORES):
        g, hq = c // 4, c % 4
        f0 = hq * 256
        m = {
            "wqT": np.ascontiguousarray(WqT[:, f0:f0 + 256]),
            "wkT": np.ascontiguousarray(WkT[:, f0:f0 + 256]),
            "wvT": np.ascontiguousarray(WvT[:, f0:f0 + 256]),
            "woT": np.ascontiguousarray(WoT[f0:f0 + 256, :]),
        }
        for sl in range(2):
            b = assign[g][sl]
            m[f"qT{sl}"] = batch_qT[b]
            m[f"kT{sl}"] = batch_kT[b]
            m[f"vT{sl}"] = batch_vT[b]
            m[f"mb{sl}"] = batch_mb[b]
            m[f"kq{sl}"] = batch_kq[b]
        in_maps.append(m)

    res = run_bass_kernel_spmd(nc, in_maps, list(range(N_CORES)))

    # --- gather: sum 4 head-quad partials per batch, transpose ---
    out = np.zeros((S, B, H), dtype=np.float32)
    for g in range(2):
        for sl in range(2):
            b = assign[g][sl]
            acc = np.zeros((H, S), dtype=np.float32)
            for hq in range(4):
                c = g * 4 + hq
                acc += res.results[c][f"y{sl}"].astype(np.float32)
            out[:, b, :] = acc.T
    return out


# revision 40
# speedup vs baseline: 1.3013x; 1.0022x over previous
"""Multi-head attention (S=2048, B=4, H=1024, NH=16) on 8 Trainium2 NeuronCores.

Sharding: each core handles 2 batches x 4 heads (batch pairs balanced by
valid length; tensor-parallel over heads). Within a core everything is bf16
matmul / fp32 accumulate:
  1. q,k projected d-major (qT/kT: [dims, seq]), v seq-major ([seq, dims])
  2. scoresT[k,q] per head-pair via row-tiled matmuls (row_grp concurrency)
  3. mask+scale+exp fused on ScalarE (per-partition bias; PAD keys -> exp 0)
  4. PV col-tiled accumulates attnT; Z row-sums via all-ones stationary
     matmul (col_grp-concurrent with PV)
  5. attnT normalized by 1/Z; keepq applied before the Wo output multiply
  6. Wo projection -> yT partial [H, S]; host sums 4 partials/batch

Schedule: weight-chunk DMAs interleave with slot0's q/k input streams over
the three DMA-capable queues (sync/scalar/gpsimd), so the first projection
matmul issues ~2 transfers in; slot0 v inputs prefetch into dedicated tiles
(issue never blocks on tile recycling). Attention is emitted as a
software-pipelined kc-loop (scores(kc+1) before PV(kc)); the two scores
matmuls use 512-strided PSUM blocks so their row-group-concurrent drains hit
different banks (same-bank PE+PE drain is a fatal collision when trimmed),
with a strided-AP exp over both blocks. PV+Z pairs run col-group-concurrent.
Wo is deferred into per-ot units drained one-per-iteration of the NEXT
query-group (1-bank PSUM tag shared with projection units), so the output
chain never gates the scores/exp pipeline; the final group's units flush
alternating into the idle scores banks. Slot1's q/k/v projections become
deadline-scheduled units: v/k spread through attention0, q half-a-group
early in attention1. The last q-chunk of each slot is width-trimmed to the
valid length (multiple of 32), shrinking exp/matmul/DVE/DMA work on padded
queries.
"""
import sys

if "/opt/trn_rl_repo" not in sys.path:
    sys.path.insert(0, "/opt/trn_rl_repo")

import math
from itertools import permutations

import ml_dtypes
import numpy as np

import concourse.bass as bass
import concourse.mybir as mybir
import concourse.tile as tile
from concourse import bacc
from concourse.bass_utils import run_bass_kernel_spmd

S, B, H, NH, DK = 2048, 4, 1024, 16, 64
N_CORES = 8
BF16 = mybir.dt.bfloat16
F32 = mybir.dt.float32
NPBF16 = ml_dtypes.bfloat16
MASK_BIAS = -30000.0

_prog_cache: dict = {}


def _build_program(NQ, NK, VQ):
    """One SPMD program. Per batch-slot s: NQ[s] 512-wide q chunks (last one
    VQ[s] wide), NK[s] 128-wide k chunks. Slot 0 is the smaller workload."""
    NSCK = [(nk * 128 + 511) // 512 for nk in NK]
    KW = [nk * 128 for nk in NK]                    # k/v valid width
    QW = [(NQ[s] - 1) * 512 + VQ[s] for s in range(2)]  # q valid width
    nc = bacc.Bacc("TRN2", target_bir_lowering=False, debug=False,
                   num_devices=N_CORES)

    d_in = {}
    for s in range(2):
        d_in[f"qT{s}"] = nc.dram_tensor(f"qT{s}", [H, S], BF16, kind="ExternalInput")
        d_in[f"kT{s}"] = nc.dram_tensor(f"kT{s}", [H, S], BF16, kind="ExternalInput")
        d_in[f"vT{s}"] = nc.dram_tensor(f"vT{s}", [H, S], BF16, kind="ExternalInput")
        d_in[f"mb{s}"] = nc.dram_tensor(f"mb{s}", [128, 16], F32, kind="ExternalInput")
        d_in[f"kq{s}"] = nc.dram_tensor(f"kq{s}", [4, 512], F32, kind="ExternalInput")
    d_in["wqT"] = nc.dram_tensor("wqT", [H, 256], BF16, kind="ExternalInput")
    d_in["wkT"] = nc.dram_tensor("wkT", [H, 256], BF16, kind="ExternalInput")
    d_in["wvT"] = nc.dram_tensor("wvT", [H, 256], BF16, kind="ExternalInput")
    d_in["woT"] = nc.dram_tensor("woT", [256, H], BF16, kind="ExternalInput")
    d_out = [nc.dram_tensor(f"y{s}", [H, S], BF16, kind="ExternalOutput")
             for s in range(2)]

    def qcw(s, sc):
        return 512 if sc < NQ[s] - 1 else VQ[s]

    def kcw(s, sc):
        return min(512, KW[s] - sc * 512)

    with tile.TileContext(nc) as tc:
        with tc.tile_pool(name="wpool", bufs=1) as wpool, \
             tc.tile_pool(name="inp", bufs=6) as inp, \
             tc.tile_pool(name="in8", bufs=1) as in8, \
             tc.tile_pool(name="persist", bufs=1) as persist, \
             tc.tile_pool(name="probs", bufs=3) as probsp, \
             tc.tile_pool(name="small", bufs=2) as small, \
             tc.tile_pool(name="att", bufs=3) as attp, \
             tc.tile_pool(name="yst", bufs=3) as ystp:

            # --- weights: consolidated DMAs on the sync queue ---
            # w*_all[p, ic*256 + j] = w*T[ic*128 + p, j]
            wq_all = wpool.tile([128, 2048], BF16, name="wq_all", tag="wq")
            wk_all = wpool.tile([128, 2048], BF16, name="wk_all", tag="wk")
            wv_all = wpool.tile([128, 2048], BF16, name="wv_all", tag="wv")
            # wo_all[p, j*1024 + c] = woT[j*128 + p, c]
            wo_all = wpool.tile([128, 2048], BF16, name="wo_all", tag="wo")
            _wseen = set()

            def wqkv(t, ic, ft):
                return t[:, ic * 256 + ft * 128: ic * 256 + (ft + 1) * 128]

            mb = [wpool.tile([128, 16], F32, name=f"mbt{s}", tag=f"mbt{s}")
                  for s in range(2)]

            def emit_wo_mb():
                for j in range(2):
                    nc.sync.dma_start(
                        out=wo_all[:, j * 1024:(j + 1) * 1024],
                        in_=d_in["woT"].ap()[j * 128:(j + 1) * 128, :])
                for s in range(2):
                    nc.sync.dma_start(out=mb[s][:], in_=d_in[f"mb{s}"].ap())
            ones = wpool.tile([128, 64], BF16, name="ones", tag="ones")
            nc.vector.memset(ones[:], 1.0)

            # --- persistent projection outputs ---
            qTp = [[persist.tile([128, NQ[s] * 512], BF16, name=f"qTp{s}_{p}",
                                 tag=f"qTp{s}_{p}")
                    for p in range(2)] for s in range(2)]
            kTp = [[persist.tile([128, NSCK[s] * 512], BF16, name=f"kTp{s}_{p}",
                                 tag=f"kTp{s}_{p}")
                    for p in range(2)] for s in range(2)]
            vp = [[persist.tile([128, 256], BF16, name=f"vp{s}_{st}", tag=f"vp{s}_{st}")
                   for st in range(NK[s])] for s in range(2)]

            # DMA queue rotation for input streams (keep Scalar clean once
            # attention starts; Sync carries the weights up front)
            s0_queues = [nc.scalar, nc.gpsimd, nc.sync]
            s1_queues = [nc.sync, nc.gpsimd]
            _qi = [0]

            def dma_rot(queues, out, in_):
                q = queues[_qi[0] % len(queues)]
                _qi[0] += 1
                q.dma_start(out=out, in_=in_)

            def emit_w(t, dname, ic):
                # interleave weight-chunk DMAs with the input stream so the
                # first matmuls aren't gated on the full weight load
                if (dname, ic) in _wseen:
                    return
                _wseen.add((dname, ic))
                dma_rot(s0_queues, t[:, ic * 256:(ic + 1) * 256],
                        d_in[dname].ap()[ic * 128:(ic + 1) * 128, :])

            def emit_proj_streamed(s, pool):
                """ic-outer projections with streamed inputs (slot 0)."""
                for kind, wts, dname, nsc, outtiles, cw in (
                        ("q", wq_all, f"qT{s}", NQ[s], qTp[s], qcw),
                        ("k", wk_all, f"kT{s}", NSCK[s], kTp[s], kcw)):
                    wname = "wqT" if kind == "q" else "wkT"
                    ps = [[pool.tile([128, 512], F32,
                                     name=f"pj{kind}{s}_{ft}_{sc}",
                                     tag=f"pj_{ft}_{sc}")
                           for sc in range(nsc)] for ft in range(2)]
                    tw = sum(cw(s, sc) for sc in range(nsc))
                    for ic in range(8):
                        emit_w(wts, wname, ic)
                        it = inp.tile([128, 2048], BF16,
                                      name=f"in{kind}{s}_{ic}", tag="inp")
                        dma_rot(s0_queues,
                                it[:, 0:tw],
                                d_in[dname].ap()[ic * 128:(ic + 1) * 128, 0:tw])
                        for ft in range(2):
                            for sc in range(nsc):
                                w = cw(s, sc)
                                nc.tensor.matmul(
                                    out=ps[ft][sc][:, 0:w],
                                    lhsT=wqkv(wts, ic, ft),
                                    rhs=it[:, sc * 512: sc * 512 + w],
                                    start=(ic == 0), stop=(ic == 7))
                    for ft in range(2):
                        for sc in range(nsc):
                            w = cw(s, sc)
                            if kind == "q":
                                nc.vector.tensor_copy(
                                    outtiles[ft][:, sc * 512: sc * 512 + w],
                                    ps[ft][sc][:, 0:w])
                            else:
                                nc.scalar.copy(
                                    outtiles[ft][:, sc * 512: sc * 512 + w],
                                    ps[ft][sc][:, 0:w])

            def emit_v_prefetch0():
                tiles = []
                for ic in range(8):
                    emit_w(wv_all, "wvT", ic)
                    it = in8.tile([128, KW[0]], BF16, name=f"v0in_{ic}",
                                  tag=f"v0in{ic}")
                    dma_rot(s0_queues, it[:],
                            d_in["vT0"].ap()[ic * 128:(ic + 1) * 128, 0:KW[0]])
                    tiles.append(it)
                return tiles

            def emit_v_resident(s, pool, tiles):
                for st0 in range(0, NK[s], 8):
                    sts = range(st0, min(st0 + 8, NK[s]))
                    psv = {st: pool.tile([128, 256], F32, name=f"pjv{s}_{st}",
                                         tag=f"pj_{(st - st0) // 4}_{(st - st0) % 4}")
                           for st in sts}
                    for ic in range(8):
                        for st in sts:
                            nc.tensor.matmul(
                                out=psv[st][:],
                                lhsT=tiles[ic][:, st * 128:(st + 1) * 128],
                                rhs=wv_all[:, ic * 256: ic * 256 + 256],
                                start=(ic == 0), stop=(ic == 7))
                    for st in sts:
                        if st % 2:
                            nc.scalar.copy(vp[s][st][:], psv[st][:])
                        else:
                            nc.vector.tensor_copy(vp[s][st][:], psv[st][:])

            def emit_prefetch(s):
                """Issue all of slot s's input DMAs into dedicated tiles."""
                tiles = {}
                for kind, dname, w in (("v", f"vT{s}", KW[s]),
                                       ("k", f"kT{s}", KW[s]),
                                       ("q", f"qT{s}", QW[s])):
                    for ic in range(8):
                        it = in8.tile([128, w], BF16, name=f"pf{kind}{s}_{ic}",
                                      tag=f"pf{kind}{ic}")
                        dma_rot(s1_queues,
                                it[:],
                                d_in[dname].ap()[ic * 128:(ic + 1) * 128, 0:w])
                        tiles[(kind, ic)] = it
                return tiles

            def proj_units(s, pool, tiles, nkit, kinds=("v", "k", "q")):
                """Deadline-tagged projection units for slot s, consumed by
                interleaving into the attention kc-loop. Deadline = global
                iteration index ((qc*2)+p)*NK + kc of first use."""
                units = []

                def q_unit(ft, sc):
                    def emit():
                        w = qcw(s, sc)
                        pj = pool.tile([128, 512], F32,
                                       name=f"rpjq{s}_{ft}_{sc}", tag="y")
                        for ic in range(8):
                            nc.tensor.matmul(
                                out=pj[:, 0:w],
                                lhsT=wqkv(wq_all, ic, ft),
                                rhs=tiles[("q", ic)][:, sc * 512: sc * 512 + w],
                                start=(ic == 0), stop=(ic == 7))
                        nc.vector.tensor_copy(
                            qTp[s][ft][:, sc * 512: sc * 512 + w], pj[:, 0:w])
                    return emit

                def k_unit(ft, sc):
                    def emit():
                        w = kcw(s, sc)
                        pj = pool.tile([128, 512], F32,
                                       name=f"rpjk{s}_{ft}_{sc}", tag="y")
                        for ic in range(8):
                            nc.tensor.matmul(
                                out=pj[:, 0:w],
                                lhsT=wqkv(wk_all, ic, ft),
                                rhs=tiles[("k", ic)][:, sc * 512: sc * 512 + w],
                                start=(ic == 0), stop=(ic == 7))
                        nc.vector.tensor_copy(
                            kTp[s][ft][:, sc * 512: sc * 512 + w], pj[:, 0:w])
                    return emit

                def v_unit(st):
                    def emit():
                        pj = pool.tile([128, 512], F32,
                                       name=f"rpjv{s}_{st}", tag="y")
                        for ic in range(8):
                            nc.tensor.matmul(
                                out=pj[:, 0:256],
                                lhsT=tiles[("v", ic)][:, st * 128:(st + 1) * 128],
                                rhs=wv_all[:, ic * 256: ic * 256 + 256],
                                start=(ic == 0), stop=(ic == 7))
                        nc.vector.tensor_copy(vp[s][st][:], pj[:, 0:256])
                    return emit

                vk = []
                if "v" in kinds:
                    for st in range(NK[s]):
                        vk.append(v_unit(st))
                if "k" in kinds:
                    for ft in range(2):
                        for sc in range(NSCK[s]):
                            vk.append(k_unit(ft, sc))
                if "q" in kinds:
                    for ft in range(2):
                        for sc in range(NQ[s]):
                            if sc == 0:
                                # first q chunks ride in the PREVIOUS slot's
                                # attention window (inputs land well before)
                                vk.append(q_unit(ft, sc))
                            else:
                                units.append(
                                    (max(0, (sc * 2 + ft) * nkit - nkit + 3),
                                     q_unit(ft, sc)))
                units.sort(key=lambda u: u[0])
                return vk, units

            def emit_attention(s, psc, pat, pz, pwo, units=None, carry_wo=None,
                               lead=4):
                units = list(units) if units else []
                wo_pending = list(carry_wo) if carry_wo else []
                nkit = NK[s]

                def drain_units(cur_idx, pre_group=False):
                    # hard deadlines only: consumers of these units follow
                    # in the in-order PE stream
                    while units and units[0][0] <= cur_idx:
                        units.pop(0)[1]()

                def drain_early(cur_idx):
                    # opportunistic emission after the iteration's PV/Z, so
                    # units never delay the scores->exp pipeline; one unit
                    # per iteration max, so consecutive units never stall on
                    # the shared 1-bank PSUM tag rotation
                    if units and units[0][0] <= cur_idx + lead:
                        units.pop(0)[1]()
                    elif wo_pending and wo_pending[0][0] <= cur_idx:
                        wo_pending.pop(0)[1](None)

                def make_wo_unit(qc, ot, ab_pair, W):
                    def emit(tag):
                        if tag is None:
                            yps = pwo.tile([128, 512], F32,
                                           name=f"yp{s}_{qc}_{ot}", tag="y")
                            ypv = yps[:, 0:W]
                        else:
                            yps = psc.tile([128, 1024], F32,
                                           name=f"yp{s}_{qc}_{ot}", tag="sc")
                            ypv = yps[:, 0:W]
                        for j in range(2):
                            nc.tensor.matmul(
                                out=ypv,
                                lhsT=wo_all[:, j * 1024 + ot * 128: j * 1024 + (ot + 1) * 128],
                                rhs=ab_pair[j][:, 0:W],
                                start=(j == 0), stop=(j == 1))
                        ysb = ystp.tile([128, 512], BF16,
                                        name=f"ysb{s}_{qc}_{ot}", tag="ysb")
                        if tag is not None:
                            nc.scalar.copy(ysb[:, 0:W], ypv)
                        else:
                            nc.vector.tensor_copy(ysb[:, 0:W], ypv)
                        (nc.gpsimd if ot % 2 else nc.sync).dma_start(
                            out=d_out[s].ap()[ot * 128:(ot + 1) * 128,
                                              qc * 512: qc * 512 + W],
                            in_=ysb[:, 0:W])
                    return emit

                kqr_t = {}

                def emit_kqr(qc):
                    w = qcw(s, qc)
                    t = small.tile([128, 512], F32, name=f"kqr{s}_{qc}",
                                   tag="kqr")
                    nc.gpsimd.dma_start(
                        out=t[:, 0:w],
                        in_=bass.AP(tensor=d_in[f"kq{s}"], offset=qc * 512,
                                    ap=[[0, 128], [1, w]]))
                    kqr_t[qc] = t

                emit_kqr(0)

                def emit_scores_g(qc, p, kc, W):
                    sc_ps = psc.tile([128, 1024], F32,
                                     name=f"s{s}_{qc}_{p}_{kc}", tag="sc")
                    pr = probsp.tile([128, 1024], BF16,
                                     name=f"pr{s}_{qc}_{p}_{kc}", tag="pr")
                    for hh in range(2):
                        hsl = slice(hh * 64, hh * 64 + 64)
                        # 512-strided blocks: the two row-group-concurrent
                        # matmuls must drain to different PSUM banks
                        nc.tensor.matmul(
                            out=sc_ps[:, hh * 512: hh * 512 + W],
                            lhsT=kTp[s][p][hsl, kc * 128:(kc + 1) * 128],
                            rhs=qTp[s][p][hsl, qc * 512: qc * 512 + W],
                            start=True, stop=True)
                    if W == 512:
                        exp_in = sc_ps[:, 0:1024]
                        exp_out = pr[:, 0:1024]
                    else:
                        exp_in = sc_ps[:].rearrange(
                            "p (b w) -> p b w", b=2)[:, :, 0:W]
                        exp_out = pr[:, 0:2 * W].rearrange(
                            "p (b w) -> p b w", b=2)
                    nc.scalar.activation(
                        out=exp_out, in_=exp_in,
                        func=mybir.ActivationFunctionType.Exp,
                        bias=mb[s][:, kc:kc + 1],
                        scale=1.0 / math.sqrt(DK))
                    return pr

                # groups software-pipelined ACROSS boundaries: the next
                # group's first scores/exp is emitted during the current
                # group's last iteration, so ScalarE never refills cold
                groups = [(qc, p) for qc in range(NQ[s]) for p in range(2)]
                att_sb = []
                sc_q = []

                def push_scores(abs_it):
                    # emit scores for absolute iteration abs_it (depth-2
                    # lookahead, rolling across group boundaries)
                    g2, kc2 = divmod(abs_it, nkit)
                    if g2 < len(groups):
                        qc2, p2 = groups[g2]
                        sc_q.append(emit_scores_g(qc2, p2, kc2, qcw(s, qc2)))

                drain_units(0)
                push_scores(0)
                push_scores(1)
                for gi, (qc, p) in enumerate(groups):
                    W = qcw(s, qc)
                    base_idx = gi * nkit
                    if p == 0:
                        if qc + 1 < NQ[s]:
                            emit_kqr(qc + 1)
                        att_sb = []
                    kqr = kqr_t[qc]
                    drain_units(base_idx)
                    attn = pat.tile([128, 512], F32,
                                    name=f"at{s}_{qc}_{p}", tag="at")
                    zps = pz.tile([128, 512], F32,
                                  name=f"z{s}_{qc}_{p}", tag="z")
                    for kc in range(nkit):
                        drain_units(base_idx + kc)
                        first, last = kc == 0, kc == nkit - 1
                        pr = sc_q.pop(0)
                        push_scores(base_idx + kc + 2)
                        for hh in range(2):
                            hsl = slice(hh * 64, hh * 64 + 64)
                            nc.tensor.matmul(
                                out=attn[hsl, 0:W],
                                lhsT=vp[s][kc][:, p * 128 + hh * 64:p * 128 + (hh + 1) * 64],
                                rhs=pr[:, hh * W:(hh + 1) * W],
                                start=first, stop=last)
                            nc.tensor.matmul(
                                out=zps[hsl, 0:W],
                                lhsT=ones[:, :], rhs=pr[:, hh * W:(hh + 1) * W],
                                start=first, stop=last)
                        drain_early(base_idx + kc)
                    # normalize: attn * keepq / Z
                    rz = small.tile([128, 512], F32,
                                    name=f"rz{s}_{qc}_{p}", tag="rz")
                    nc.vector.reciprocal_approx_fast(out=rz[:, 0:W],
                                                     in_=zps[:, 0:W])
                    nc.vector.tensor_mul(rz[:, 0:W], rz[:, 0:W], kqr[:, 0:W])
                    ab = attp.tile([128, 512], BF16,
                                   name=f"ab{s}_{qc}_{p}", tag=f"ab{p}")
                    nc.vector.tensor_mul(ab[:, 0:W], attn[:, 0:W], rz[:, 0:W])
                    att_sb.append(ab)
                    if p == 1:
                        kqr_t.pop(qc)
                        # Wo deferred: one unit per iteration of the NEXT
                        # group, never gating the scores/exp pipeline
                        for ot in range(8):
                            wo_pending.append(((qc + 1) * 2 * nkit + 1 + 2 * ot,
                                               make_wo_unit(qc, ot, att_sb, W)))
                # flush stragglers; alternate between the 'y' bank and the
                # now-idle scores slots so the tail chain pipelines
                while units:
                    units.pop(0)[1]()
                return wo_pending

            # slot 0 projections use the full PSUM (released afterwards)
            with tc.tile_pool(name="pproj", bufs=1, space="PSUM") as pproj:
                emit_proj_streamed(0, pproj)
                v0tiles = emit_v_prefetch0()
                emit_v_resident(0, pproj, v0tiles)
            emit_wo_mb()
            tiles1 = emit_prefetch(1)
            with tc.tile_pool(name="psc", bufs=2, space="PSUM") as psc, \
                 tc.tile_pool(name="pat", bufs=2, space="PSUM") as pat, \
                 tc.tile_pool(name="pz", bufs=1, space="PSUM") as pz, \
                 tc.tile_pool(name="pwo", bufs=1, space="PSUM") as pwo:
                vk1, units1 = proj_units(1, pwo, tiles1, NK[1])
                n0 = NQ[0] * 2 * NK[0]
                sp = max(1, (n0 - 6) // max(1, len(vk1)))
                units0 = [(2 + sp * i, fn) for i, fn in enumerate(vk1)]
                left0 = emit_attention(0, psc, pat, pz, pwo, units=units0)
                carry = [(i + 1, fn) for i, (_, fn) in enumerate(left0)]
                left1 = emit_attention(1, psc, pat, pz, pwo, units=units1,
                                       carry_wo=carry)
                for i, (_, fn) in enumerate(left1):
                    fn("sc" if i % 2 == 0 else None)
    nc.compile()
    return nc


def _get_program(NQ, NK, VQ):
    key = (tuple(NQ), tuple(NK), tuple(VQ))
    if key not in _prog_cache:
        _prog_cache[key] = _build_program(list(NQ), list(NK), list(VQ))
    return _prog_cache[key]


def kernel(value, key, query, padding_mask, Wq, Wk, Wv, Wo):
    value = np.asarray(value)
    key = np.asarray(key)
    query = np.asarray(query)
    padding_mask = np.asarray(padding_mask)
    Wq, Wk, Wv, Wo = (np.asarray(a) for a in (Wq, Wk, Wv, Wo))

    lengths = (~padding_mask).sum(axis=0).astype(int)  # (B,)

    # --- batch pairing: assign batches to (group, slot) minimizing baked work ---
    def slot_counts(assign):
        nq = [max((int(lengths[assign[g][sl]]) + 511) // 512 for g in range(2))
              for sl in range(2)]
        nk = [max((int(lengths[assign[g][sl]]) + 127) // 128 for g in range(2))
              for sl in range(2)]
        return nq, nk

    best = None
    for perm in permutations(range(B)):
        a = ((perm[0], perm[1]), (perm[2], perm[3]))
        nq, nk = slot_counts(a)
        c = nq[0] * nk[0] + nq[1] * nk[1]
        if best is None or c < best[0]:
            best = (c, a)
    assign = best[1]
    nq, nk = slot_counts(assign)
    # slot 0 should be the smaller workload (its projections can't overlap)
    if nq[0] * nk[0] > nq[1] * nk[1]:
        assign = tuple((g[1], g[0]) for g in assign)
        nq, nk = slot_counts(assign)
    NQ, NK = nq, nk
    # trimmed width of the last q chunk per slot (multiple of 32)
    VQ = []
    for sl in range(2):
        maxlen = max(int(lengths[assign[g][sl]]) for g in range(2))
        v = maxlen - (NQ[sl] - 1) * 512
        VQ.append(min(512, (v + 31) // 32 * 32))

    nc = _get_program(NQ, NK, VQ)

    # --- per-core inputs ---
    WqT = np.ascontiguousarray(Wq.T).astype(NPBF16)
    WkT = np.ascontiguousarray(Wk.T).astype(NPBF16)
    WvT = np.ascontiguousarray(Wv.T).astype(NPBF16)
    WoT = np.ascontiguousarray(Wo.T).astype(NPBF16)

    batch_qT, batch_kT, batch_vT, batch_mb, batch_kq = {}, {}, {}, {}, {}
    for b in range(B):
        batch_qT[b] = np.ascontiguousarray(query[:, b, :].T).astype(NPBF16)
        batch_kT[b] = np.ascontiguousarray(key[:, b, :].T).astype(NPBF16)
        batch_vT[b] = np.ascontiguousarray(value[:, b, :].T).astype(NPBF16)
        kpos = np.arange(S).reshape(16, 128)  # [kchunk, kpos]
        mbv = np.where(kpos >= lengths[b], np.float32(MASK_BIAS), np.float32(0.0))
        batch_mb[b] = np.ascontiguousarray(mbv.T).astype(np.float32)  # [128, 16]
        batch_kq[b] = (np.arange(S).reshape(4, 512) < lengths[b]).astype(np.float32)

    in_maps = []
    for c in range(N_CORES):
        g, hq = c // 4, c % 4
        f0 = hq * 256
        m = {
            "wqT": np.ascontiguousarray(WqT[:, f0:f0 + 256]),
            "wkT": np.ascontiguousarray(WkT[:, f0:f0 + 256]),
            "wvT": np.ascontiguousarray(WvT[:, f0:f0 + 256]),
            "woT": np.ascontiguousarray(WoT[f0:f0 + 256, :]),
        }
        for sl in range(2):
            b = assign[g][sl]
            m[f"qT{sl}"] = batch_qT[b]
            m[f"kT{sl}"] = batch_kT[b]
            m[f"vT{sl}"] = batch_vT[b]
            m[f"mb{sl}"] = batch_mb[b]
            m[f"kq{sl}"] = batch_kq[b]
        in_maps.append(m)

    res = run_bass_kernel_spmd(nc, in_maps, list(range(N_CORES)))

    # --- gather: sum 4 head-quad partials per batch, transpose ---
    out = np.zeros((S, B, H), dtype=np.float32)
    for g in range(2):
        for sl in range(2):
            b = assign[g][sl]
            acc = np.zeros((H, S), dtype=np.float32)
            for hq in range(4):
                c = g * 4 + hq
                acc += res.results[c][f"y{sl}"].astype(np.float32)
            out[:, b, :] = acc.T
    return out
